# revision 1
# baseline (speedup 1.0000x reference)
"""Trainium2 Bass kernel for nn_MultiHeadAttention_78864189489198.

Symmetric-scores fp8 variant.

S = Q Q^T is symmetric, so exp(S) is too: compute scores/exp only for
blocks (v, u) with u >= v (136/256 of the work), and realize each
lower-triangle contribution directly from the stored upper block:

  row use   (YT):  YT[:, u]  += Qnat_v^T @ E[v, u...]     (fp16 matmul)
  mirror use (Y):  Y[u, :]   += E[v, u]^T @ [Qnat_v | 1]  (fp16 matmul)
                   -> transposed on PE into YT layout, ones column
                      accumulates the missing row-sum part of r.

This halves the ACT exp stream (the v2 pacer) at the cost of ~15% more
PE matmul columns and a PE transpose per mirror block.

Everything else as v2.1: fp8 hi/lo DoubleRow Qproj, fp8 DR scores from
qt8 [64,2,...], fp16 PV/outproj, fp16 partial outputs summed on host.
"""

import os

import numpy as np
import ml_dtypes

import concourse.bass as bass
import concourse.mybir as mybir
import concourse.tile as tile
from concourse import bacc
from concourse.bass_utils import run_bass_kernel_spmd
from concourse.masks import make_identity
from contextlib import ExitStack

P = 128
N = 2048
D = 2048
KP = 8
HG = 4
HD = 128
HCOLS = HG * HD
SP = N // 512
NCH = N // P
SCALE = HD ** -0.5
C_BIAS = 9.0
WQS = 64.0
Q8S = 1.0 / 16.0

f32 = mybir.dt.float32
f32r = mybir.dt.float32r
f16 = mybir.dt.float16
e4 = mybir.dt.float8e4
DR = mybir.MatmulPerfMode.DoubleRow

_CACHE = {}


def build_nc():
    nc = bacc.Bacc("TRN2", target_bir_lowering=False, debug=False)
    xh = nc.dram_tensor("xh", [P, SP, KP, 2, 512], e4, kind="ExternalInput")
    xl = nc.dram_tensor("xl", [P, SP, KP, 2, 512], e4, kind="ExternalInput")
    wqh = nc.dram_tensor("wqh", [P, KP, 2, HCOLS], e4, kind="ExternalInput")
    wql = nc.dram_tensor("wql", [P, KP, 2, HCOLS], e4, kind="ExternalInput")
    wo8h = nc.dram_tensor("wo8h", [P, HG, D], e4, kind="ExternalInput")
    wo8l = nc.dram_tensor("wo8l", [P, HG, D], e4, kind="ExternalInput")
    out = nc.dram_tensor("out", [N, D], f16, kind="ExternalOutput")
    out3 = out.rearrange("(a p) n -> p a n", p=P)

    with (
        nc.allow_low_precision(reason="fp8/fp16 dataflow is intentional"),
        tile.TileContext(nc) as tc,
        ExitStack() as ctx,
    ):
        const_pool = ctx.enter_context(tc.tile_pool(name="const", bufs=1))
        qt8_pool = ctx.enter_context(tc.tile_pool(name="qt8", bufs=1))
        qn_pool = ctx.enter_context(tc.tile_pool(name="qn", bufs=1))
        es_pool = ctx.enter_context(tc.tile_pool(name="es", bufs=19))
        rr_pool = ctx.enter_context(tc.tile_pool(name="rr", bufs=3))
        rbc_pool = ctx.enter_context(tc.tile_pool(name="rbc", bufs=2))
        yt_pool = ctx.enter_context(tc.tile_pool(name="yt", bufs=2))
        o_pool = ctx.enter_context(tc.tile_pool(name="osb", bufs=3))
        ynm_pool = ctx.enter_context(tc.tile_pool(name="ynm", bufs=4))
        ps_s = ctx.enter_context(tc.tile_pool(name="ps_s", bufs=2, space="PSUM"))
        ps_b = ctx.enter_context(tc.tile_pool(name="ps_b", bufs=4, space="PSUM"))

        idr = const_pool.tile([P, P], f32r, tag="idr")
        id16 = const_pool.tile([P, P], f16, tag="id16")
        cbias = const_pool.tile([P, 1], f32, tag="cbias")
        nc.gpsimd.memset(cbias[:], -C_BIAS)

        qt8 = qt8_pool.tile([64, 2, HG, N], e4, tag="qt8")
        # qn[:, a, c, 0:128] = Qnat chunk a of head c; col 128 = ones
        qn_sb = qn_pool.tile([P, NCH, HG, 129], f16, tag="qn")
        nc.gpsimd.memset(qn_sb[:, :, :, 128:129], 1.0)
        yts = {}
        rrechs = {}
        rmirs = {}
        ess = {}

        def new_head_state(c):
            rrechs[c] = rr_pool.tile([P, NCH, 2], f32, tag="rrech", name=f"rr{c}")
            rmirs[c] = rr_pool.tile([P, NCH], f32, tag="rmir", name=f"rm{c}")
            nc.vector.memset(rrechs[c][:], 0.0)
            nc.vector.memset(rmirs[c][:], 0.0)
            yts[c] = yt_pool.tile([P, N], f16, tag="yt", name=f"yt{c}")
            nc.gpsimd.memset(yts[c][:, 0:P], 0.0)
            ess[c] = []

        def scores_piece(c, v, h, es):
            # cols [max(128v, 1024h), 1024(h+1))
            lo = max(P * v, 1024 * h)
            hi = 1024 * (h + 1)
            if lo >= hi:
                return
            ps = ps_s.tile([P, 1024], f32, tag="s")
            col = lo
            while col < hi:
                w = min(512 - col % 512, hi - col)
                nc.tensor.matmul(
                    ps[:, col - 1024 * h:col - 1024 * h + w],
                    qt8[:, :, c, v * P:(v + 1) * P],
                    qt8[:, :, c, col:col + w],
                    start=True,
                    stop=True,
                    perf_mode=DR,
                )
                col += w
            nc.scalar.activation(
                es[:, lo:hi],
                ps[:, lo - 1024 * h:1024],
                mybir.ActivationFunctionType.Exp,
                bias=cbias[:, 0:1],
                scale=SCALE / 16.0,
                accum_out=rrechs[c][:, v, h:h + 1],
            )

        def scores_row(c, v):
            es = es_pool.tile([P, N], f16, tag="es")
            scores_piece(c, v, 0, es)
            scores_piece(c, v, 1, es)
            ess[c].append(es)

        def mirror_unit(c, u):
            # Y[u-chunk, :] += sum_{v<u} E[v, u-block]^T @ [Qnat_v | 1]
            pm = ps_b.tile([P, 132], f32, tag="b", name=f"pm{c}_{u}")
            for v in range(u):
                nc.tensor.matmul(
                    pm[:, 0:129],
                    ess[c][v][:, u * P:(u + 1) * P],
                    qn_sb[:, v, c, :],
                    start=(v == 0),
                    stop=(v == u - 1),
                )
            ynm = ynm_pool.tile([P, P], f16, tag="ynm")
            nc.vector.tensor_copy(ynm[:], pm[:, 0:P])
            nc.vector.tensor_copy(rmirs[c][:, u:u + 1], pm[:, 128:129])
            ptm = ps_b.tile([P, P], f16, tag="b", name=f"ptm{c}_{u}")
            nc.tensor.transpose(ptm[:], ynm[:], id16[:])
            nc.vector.tensor_copy(yts[c][:, u * P:(u + 1) * P], ptm[:])

        def r_chain(c):
            rrec = rr_pool.tile([P, NCH], f32, tag="rrec")
            nc.vector.tensor_reduce(
                rrec[:], rrechs[c][:], mybir.AxisListType.X, mybir.AluOpType.add
            )
            nc.vector.tensor_tensor(
                rrec[:], rrec[:], rmirs[c][:], mybir.AluOpType.add
            )
            rrec2 = rr_pool.tile([P, NCH], f32r, tag="rrec2")
            nc.vector.reciprocal(rrec2[:], rrec[:])
            prt = ps_b.tile([NCH, P], f32r, tag="b")
            nc.tensor.transpose(prt[:], rrec2[:], idr[:])
            rt16 = rr_pool.tile([NCH, P], f16, tag="rt16")
            nc.vector.tensor_scalar_mul(rt16[:], prt[:], 16.0)
            rbc = rbc_pool.tile([P, N], f16, tag="rbc")
            nc.sync.dma_start(rbc[0:1, :], rt16[:, :])
            nc.gpsimd.partition_broadcast(rbc[:], rbc[0:1, :])
            return rbc

        with (
            tc.tile_pool(name="xt", bufs=5) as xt_pool,
            tc.tile_pool(name="wq", bufs=1) as wq_pool,
            tc.tile_pool(name="q16r", bufs=4) as q16r_pool,
            tc.tile_pool(name="q8f", bufs=4) as q8f_pool,
        ):
            make_identity(nc, id16[:])
            nc.vector.tensor_copy(idr[:], id16[:])

            wqh_sb = wq_pool.tile([P, KP, 2, HCOLS], e4, tag="wqh")
            wql_sb = wq_pool.tile([P, KP, 2, HCOLS], e4, tag="wql")
            xsp = {}
            for t in range(SP):
                xsp[t] = (
                    xt_pool.tile([P, KP, 2, 512], e4, tag="xt", name=f"xh{t}"),
                    xt_pool.tile([P, KP, 2, 512], e4, tag="xt", name=f"xl{t}"),
                )
            nc.sync.dma_start(wqh_sb[:, 0:4], wqh[:, 0:4])
            nc.scalar.dma_start(xsp[0][0][:], xh[:, 0])
            nc.sync.dma_start(wqh_sb[:, 4:8], wqh[:, 4:8])
            nc.scalar.dma_start(wql_sb[:], wql[:])
            nc.gpsimd.dma_start(xsp[0][1][:], xl[:, 0])
            nc.scalar.dma_start(xsp[1][1][:], xl[:, 1])
            nc.gpsimd.dma_start(xsp[1][0][:], xh[:, 1])
            nc.sync.dma_start(xsp[2][0][:], xh[:, 2])
            nc.scalar.dma_start(xsp[3][1][:], xl[:, 3])
            nc.gpsimd.dma_start(xsp[2][1][:], xl[:, 2])
            nc.sync.dma_start(xsp[3][0][:], xh[:, 3])
            edum = rr_pool.tile([P, 1], f32, tag="edum")
            nc.scalar.activation(
                edum[:], cbias[:, 0:1], mybir.ActivationFunctionType.Exp
            )

            q8flats = {}
            pend_tr = []

            def qproj_unit(c, t):
                ps = ps_b.tile([P, 512], f32, tag="b")
                th, tl = xsp[t]
                combos = [(wqh_sb, th, kp) for kp in range(KP)]
                combos += [(wql_sb, th, kp) for kp in range(KP)]
                combos += [(wqh_sb, tl, kp) for kp in range(KP)]
                for i, (w, x, kp) in enumerate(combos):
                    nc.tensor.matmul(
                        ps[:],
                        w[:, kp, :, c * P:(c + 1) * P],
                        x[:, kp],
                        start=(i == 0),
                        stop=(i == len(combos) - 1),
                        perf_mode=DR,
                    )
                q16 = q16r_pool.tile([P, 512], f16, tag="q16")
                nc.vector.tensor_copy(q16[:], ps[:])
                if c not in q8flats:
                    q8flats[c] = q8f_pool.tile([P, N], e4, tag="q8f", name=f"q8f{c}")
                nc.gpsimd.tensor_scalar_mul(
                    q8flats[c][:, t * 512:(t + 1) * 512], q16[:], 1.0 / 16.0
                )
                if pend_tr:
                    pend_tr.pop(0)()

                def transposes(c=c, t=t, q16=q16):
                    for j in range(4):
                        pt = ps_b.tile([P, P], f16, tag="b")
                        nc.tensor.transpose(
                            pt[:], q16[:, j * P:(j + 1) * P], id16[:]
                        )
                        nc.vector.tensor_scalar_mul(
                            qn_sb[:, t * 4 + j, c, 0:P], pt[:], 1.0 / 1024.0
                        )

                pend_tr.append(transposes)

            def fold(c, lo_t, hi_t):
                nc.sync.dma_start(
                    qt8[:, :, c, lo_t * 512:hi_t * 512],
                    q8flats[c][:, lo_t * 512:hi_t * 512],
                )

            for t in range(2):
                for c in range(HG):
                    qproj_unit(c, t)
            fold(0, 0, 2)

            while pend_tr:
                pend_tr.pop(0)()
            new_head_state(0)
            # phase 1: h=0 pieces for rows v<8 (need spans 0-1 only)
            for v in range(8):
                es = es_pool.tile([P, N], f16, tag="es")
                scores_piece(0, v, 0, es)
                ess[0].append(es)

            for c in range(HG):
                qproj_unit(c, 2)
            qproj_unit(0, 3)
            fold(0, 2, 4)
            fold(1, 0, 3)

            tailq = []
            for c in range(1, HG):
                tailq.append(lambda c=c: qproj_unit(c, 3))
                if c == 1:
                    tailq.append(lambda: fold(1, 3, 4))
                else:
                    tailq.append(lambda c=c: fold(c, 0, 4))

            # phase 2: finish head-0 rows; mirrors trail 2 rows behind so
            # their psum slots rotate through freed qproj/transpose slots
            for v in range(NCH):
                if v < 8:
                    scores_piece(0, v, 1, ess[0][v])
                else:
                    scores_row(0, v)
                if v >= 2:
                    mirror_unit(0, v - 1)
                npop = (len(tailq) + NCH - 1 - v) // (NCH - v)
                for _ in range(npop):
                    if tailq:
                        tailq.pop(0)()
            while tailq:
                tailq.pop(0)()
            while pend_tr:
                pend_tr.pop(0)()
            mirror_unit(0, NCH - 1)

        with (
            tc.tile_pool(name="wo8", bufs=1) as wo8_pool,
            tc.tile_pool(name="y8", bufs=1) as y8p,
        ):
            wo8h_sb = wo8_pool.tile([P, HG, D], e4, tag="wo8h")
            wo8l_sb = wo8_pool.tile([P, HG, D], e4, tag="wo8l")
            nc.sync.dma_start(wo8h_sb[:], wo8h[:])
            nc.gpsimd.dma_start(wo8l_sb[:], wo8l[:])
            y8h = y8p.tile([P, HG, N], e4, tag="y8h")
            y8l = y8p.tile([P, HG, N], e4, tag="y8l")

            def outproj_unit(a):
                ot = o_pool.tile([P, D], f16, tag="ot")
                for d4 in range(SP):
                    ps = ps_b.tile([P, 512], f32, tag="b")
                    combos = []
                    for cc in (0, 2):
                        combos += [
                            (y8h, wo8h_sb, cc),
                            (y8h, wo8l_sb, cc),
                            (y8l, wo8h_sb, cc),
                        ]
                    for i, (yy, ww, cc) in enumerate(combos):
                        nc.tensor.matmul(
                            ps[:],
                            yy[:, cc:cc + 2, a * P:(a + 1) * P],
                            ww[:, cc:cc + 2, d4 * 512:(d4 + 1) * 512],
                            start=(i == 0),
                            stop=(i == len(combos) - 1),
                            perf_mode=DR,
                        )
                    if d4 % 2 == 0:
                        nc.vector.tensor_scalar_mul(
                            ot[:, d4 * 512:(d4 + 1) * 512], ps[:], 1.0 / 64.0
                        )
                    else:
                        nc.scalar.mul(ot[:, d4 * 512:(d4 + 1) * 512], ps[:], 1.0 / 64.0)
                if a < 14:
                    eng = nc.sync if a % 2 == 0 else nc.gpsimd
                    eng.dma_start(out3[:, a, :], ot[:])
                else:
                    nc.sync.dma_start(out3[:, a, 0:1024], ot[:, 0:1024])
                    nc.gpsimd.dma_start(out3[:, a, 1024:2048], ot[:, 1024:2048])

            # ================= heads pipeline =================
            # head c iteration: scores+mirror of head c, PV of head pc=c-1
            # with early per-quarter evac + norm.
            for c in range(1, HG + 1):
                pc = c - 1
                rbc = r_chain(pc)
                if c < HG:
                    new_head_state(c)
                psy = {}
                for s in range(SP):
                    psy[s] = ps_b.tile([P, 512], f32, tag="b", name=f"psy{c}_{s}")
                mirrorq = []
                pes = ess[pc]

                def pv_row(v):
                    for s in range(v // 4, SP):
                        lo = max(s * 512, v * P)
                        hi = (s + 1) * 512
                        nc.tensor.matmul(
                            psy[s][:, lo - s * 512:hi - s * 512],
                            qn_sb[:, v, pc, 0:P],
                            pes[v][:, lo:hi],
                            start=(v == 0),
                            stop=(v == 4 * s + 3),
                        )

                def evac_norm(s):
                    yt = yts[pc]
                    nc.vector.tensor_tensor(
                        yt[:, s * 512:(s + 1) * 512],
                        psy[s][:],
                        yt[:, s * 512:(s + 1) * 512],
                        mybir.AluOpType.add,
                    )
                    nc.vector.tensor_tensor(
                        yt[:, s * 512:(s + 1) * 512],
                        yt[:, s * 512:(s + 1) * 512],
                        rbc[:, s * 512:(s + 1) * 512],
                        mybir.AluOpType.mult,
                    )
                    nc.gpsimd.tensor_copy(
                        y8h[:, pc, s * 512:(s + 1) * 512],
                        yt[:, s * 512:(s + 1) * 512],
                    )
                    nc.gpsimd.tensor_tensor(
                        y8l[:, pc, s * 512:(s + 1) * 512],
                        yt[:, s * 512:(s + 1) * 512],
                        y8h[:, pc, s * 512:(s + 1) * 512],
                        mybir.AluOpType.subtract,
                    )

                for v in range(NCH):
                    pv_row(v)
                    if c < HG:
                        scores_row(c, v)
                        if v >= 1:
                            mirrorq.append(v)
                    if v % 4 == 3:
                        evac_norm(v // 4)
                        while mirrorq:
                            mirror_unit(c, mirrorq.pop(0))
                        if c == HG:
                            for a in range(v - 3, v + 1):
                                outproj_unit(a)

    nc.compile()
    return nc


def _hi_lo(arr):
    hi = arr.astype(ml_dtypes.float8_e4m3)
    lo = (arr - hi.astype(np.float32)).astype(ml_dtypes.float8_e4m3)
    return hi, lo


def _pack_x(xt_hl):
    return np.ascontiguousarray(
        xt_hl.reshape(KP, 2, P, SP, 512).transpose(2, 3, 0, 1, 4)
    )


def kernel(x, Wq, Wo, bo):
    x = np.asarray(x, dtype=np.float32)
    Wq = np.asarray(Wq, dtype=np.float32)
    Wo = np.asarray(Wo, dtype=np.float32)
    bo = np.asarray(bo, dtype=np.float32)
    B = x.shape[0]
    assert B == 2 and x.shape == (B, N, D)
    assert Wq.shape == (D, D) and Wo.shape == (D, D)

    if "nc" not in _CACHE:
        _CACHE["nc"] = build_nc()
    nc = _CACHE["nc"]

    packed_x = []
    for b in range(B):
        hi, lo = _hi_lo(np.ascontiguousarray(x[b].T))
        packed_x.append((_pack_x(hi), _pack_x(lo)))
    in_maps = []
    for core in range(8):
        b, hg = core // 4, core % 4
        wq_s = WQS * Wq[:, hg * HCOLS:(hg + 1) * HCOLS]
        wqh, wql = _hi_lo(np.ascontiguousarray(wq_s))
        wo8h_a, wo8l_a = _hi_lo(
            np.ascontiguousarray(WQS * Wo[hg * HCOLS:(hg + 1) * HCOLS, :])
        )
        in_maps.append(
            {
                "xh": packed_x[b][0],
                "xl": packed_x[b][1],
                "wqh": np.ascontiguousarray(
                    wqh.reshape(KP, 2, P, HCOLS).transpose(2, 0, 1, 3)
                ),
                "wql": np.ascontiguousarray(
                    wql.reshape(KP, 2, P, HCOLS).transpose(2, 0, 1, 3)
                ),
                "wo8h": np.ascontiguousarray(
                    wo8h_a.reshape(HG, P, D).transpose(1, 0, 2)
                ),
                "wo8l": np.ascontiguousarray(
                    wo8l_a.reshape(HG, P, D).transpose(1, 0, 2)
                ),
            }
        )

    res = run_bass_kernel_spmd(nc, in_maps, list(range(8)))
    _CACHE["last_res"] = res
    out = np.zeros((B, N, D), dtype=np.float32)
    for core in range(8):
        b = core // 4
        out[b] += res.results[core]["out"].astype(np.float32)
    out += bo
    return out



# revision 2
# speedup vs baseline: 1.0287x; 1.0287x over previous
"""Trainium2 Bass kernel for nn_MultiHeadAttention_78864189489198 — v3.

fp8 paired-es variant.

E = exp(S - B8) is stored in fp8 (e4m3) chunk-PAIRED tiles es8[t] of
shape [P, 2, N] holding row-chunks (2t, 2t+1).  This lets both the PV
(attn @ V) and mirror matmuls run in DoubleRow perf mode (contraction
256 = two chunks at once, 0.5 cyc/col), cutting their PE cost ~4x/3.3x
vs the fp16 v2 path.

The huge diagonal E_nn (softmax is diagonally dominant here) cannot
live in fp8: the scores diagonal is masked to 0 in PSUM before exp
(DVE multiply by (1-I)), and the diagonal contribution is restored
exactly:  S_nn = rowsum(q16^2) via per-chunk ones-matmuls,
E_nn = exp(S_nn - 9), Y += (E_nn / r) * Q with r assembled from the
exp accumulators + mirror ones-column + E_nn (consistent numerator /
denominator, so S_nn quantization error cancels).

Everything else as v2: fp8 hi/lo DoubleRow qproj/outproj, host-side
batch x head-group sharding (8 cores), fp16 partial outputs summed on
host.
"""

import numpy as np
import ml_dtypes

import concourse.bass as bass
import concourse.mybir as mybir
import concourse.tile as tile
from concourse import bacc
from concourse.bass_utils import run_bass_kernel_spmd
from concourse.masks import make_identity
from contextlib import ExitStack

P = 128
N = 2048
D = 2048
KP = 8
HG = 4
HD = 128
HCOLS = HG * HD
SP = N // 512
NCH = N // P
NPAIR = NCH // 2
SCALE = HD ** -0.5
C_BIAS = 9.0
B8 = 3.5
WQS = 64.0

f32 = mybir.dt.float32
f32r = mybir.dt.float32r
f16 = mybir.dt.float16
e4 = mybir.dt.float8e4
DR = mybir.MatmulPerfMode.DoubleRow

_CACHE = {}

EXP = mybir.ActivationFunctionType.Exp
ADD = mybir.AluOpType.add
SUB = mybir.AluOpType.subtract
MUL = mybir.AluOpType.mult


def build_nc():
    nc = bacc.Bacc("TRN2", target_bir_lowering=False, debug=False)
    xh = nc.dram_tensor("xh", [P, SP, KP, 2, 512], e4, kind="ExternalInput")
    xl = nc.dram_tensor("xl", [P, SP, KP, 2, 512], e4, kind="ExternalInput")
    wqh = nc.dram_tensor("wqh", [P, KP, 2, HCOLS], e4, kind="ExternalInput")
    wql = nc.dram_tensor("wql", [P, KP, 2, HCOLS], e4, kind="ExternalInput")
    wo8h = nc.dram_tensor("wo8h", [P, HG, D], e4, kind="ExternalInput")
    wo8l = nc.dram_tensor("wo8l", [P, HG, D], e4, kind="ExternalInput")
    out = nc.dram_tensor("out", [N, D], f16, kind="ExternalOutput")
    out3 = out.rearrange("(a p) n -> p a n", p=P)
    outA = nc.dram_tensor("outA", [N, D], f16, kind="ExternalOutput")
    outA3 = outA.rearrange("(a p) n -> p a n", p=P)

    EB8 = float(np.exp(-B8))          # masked-diag residue per row
    EB9 = float(np.exp(B8 - C_BIAS))  # es8 scale -> e^-9 scale

    with (
        nc.allow_low_precision(reason="fp8/fp16 dataflow is intentional"),
        tile.TileContext(nc) as tc,
        ExitStack() as ctx,
    ):
        const_pool = ctx.enter_context(tc.tile_pool(name="const", bufs=1))
        qt8_pool = ctx.enter_context(tc.tile_pool(name="qt8", bufs=1))
        qn8_pool = ctx.enter_context(tc.tile_pool(name="qn8", bufs=1))
        q16_pool = ctx.enter_context(tc.tile_pool(name="q16f", bufs=4))
        es_pool = ctx.enter_context(tc.tile_pool(name="es8", bufs=16))
        rr_pool = ctx.enter_context(tc.tile_pool(name="rr", bufs=4))
        tmp_pool = ctx.enter_context(tc.tile_pool(name="tmp", bufs=3))
        snn_pool = ctx.enter_context(tc.tile_pool(name="snn", bufs=4))
        rbc_pool = ctx.enter_context(tc.tile_pool(name="rbc", bufs=4))
        yt_pool = ctx.enter_context(tc.tile_pool(name="yt", bufs=2))
        ynm_pool = ctx.enter_context(tc.tile_pool(name="ynm", bufs=4))
        ps_s = ctx.enter_context(tc.tile_pool(name="ps_s", bufs=2, space="PSUM"))
        ps_b = ctx.enter_context(tc.tile_pool(name="ps_b", bufs=4, space="PSUM"))

        idr = const_pool.tile([P, P], f32r, tag="idr")
        id16 = const_pool.tile([P, P], f16, tag="id16")
        # DR-layout fp8 identities: ia8p^T @ ia8n = -4096*I (diag canceller)
        ia8p = const_pool.tile([64, 2, P], e4, tag="ia8p")
        ia8n = const_pool.tile([64, 2, P], e4, tag="ia8n")
        nb8 = const_pool.tile([P, 1], f32, tag="nb8")
        nc.gpsimd.memset(nb8[:], -B8)
        nb9 = const_pool.tile([P, 1], f32, tag="nb9")
        nc.gpsimd.memset(nb9[:], -C_BIAS)
        ones1 = const_pool.tile([P, 1], f16, tag="ones1")
        nc.gpsimd.memset(ones1[:], 1.0)

        # qt8[p, r, c, n] : 4*Q^T head c, DR layout (hd = 2p+r-ish)
        qt8 = qt8_pool.tile([64, 2, HG, N], e4, tag="qt8")
        # qn8[p, t, k, c, :] = Q[(2t+k)*P+p, head c, :]/16
        # (128-multiple strides: fp8 Ldweights restriction)
        qn8 = qn8_pool.tile([P, NPAIR, 2, HG, P], e4, tag="qn8")
        # fp8 ones for the mirror r-column matmuls
        on8 = const_pool.tile([P, 2, 1], e4, tag="on8")
        nc.gpsimd.memset(on8[:], 1.0)
        # q16full[c] = 4*Q^T head c, f16, persistent (diag path + src of qt8/qn8)
        q16full = {}
        for c in range(HG):
            q16full[c] = q16_pool.tile([P, N], f16, tag="q16f", name=f"q16f{c}")

        yts = {}
        rrechs = {}
        rmirs = {}
        snns = {}
        ess = {}       # ess[c][t] = es8 pair tile [P, 2, N]

        def new_head_state(c):
            rrechs[c] = rr_pool.tile([P, NCH, 3], f32, tag="rrech", name=f"rr{c}")
            rmirs[c] = rr_pool.tile([P, NCH], f32, tag="rmir", name=f"rm{c}")
            nc.vector.memset(rrechs[c][:], 0.0)
            nc.vector.memset(rmirs[c][:], 0.0)
            yts[c] = yt_pool.tile([P, N], f16, tag="yt", name=f"yt{c}")
            nc.gpsimd.memset(yts[c][:, 0:P], 0.0)
            ess[c] = {}

        def scores_piece(c, v, h, es, k, lo=None, hi=None, slot=None):
            # scores cols [max(128v, 1024h), 1024(h+1)) for row-chunk v
            if lo is None:
                lo = max(P * v, 1024 * h)
            if hi is None:
                hi = 1024 * (h + 1)
            if slot is None:
                slot = h
            if lo >= hi:
                return
            ps = ps_s.tile([P, 1024], f32, tag="s")
            col = lo
            while col < hi:
                w = min(512 - col % 512, hi - col)
                nc.tensor.matmul(
                    ps[:, col - 1024 * h:col - 1024 * h + w],
                    qt8[:, :, c, v * P:(v + 1) * P],
                    qt8[:, :, c, col:col + w],
                    start=True,
                    stop=True,
                    perf_mode=DR,
                )
                col += w
            if lo == v * P:
                # diag block leads this piece: add -4096*I so exp of the
                # exact diagonal flushes to 0 in fp8 (residue ~1e-4, ignored)
                nc.tensor.matmul(
                    ps[:, lo - 1024 * h:lo - 1024 * h + P],
                    ia8p[:],
                    ia8n[:],
                    start=False,
                    stop=True,
                    perf_mode=DR,
                )
            nc.scalar.activation(
                es[:, k, lo:hi],
                ps[:, lo - 1024 * h:hi - 1024 * h],
                EXP,
                bias=nb8[:, 0:1],
                scale=SCALE / 16.0,
                accum_out=rrechs[c][:, v, slot:slot + 1],
            )

        def scores_row(c, v):
            t, k = v // 2, v % 2
            if k == 0:
                ess[c][t] = es_pool.tile([P, 2, N], e4, tag="es", name=f"e{c}_{t}")
            es = ess[c][t]
            scores_piece(c, v, 0, es, k)
            scores_piece(c, v, 1, es, k)

        def mirror_units(c, us):
            # Y[u-chunk, :] += sum_{v<u} E8[v, u-block]^T @ [Q_v/16 | 1]
            # us: 1-2 consecutive chunk indices sharing one PSUM tile
            nu = len(us)
            w = 128 * nu
            pm = ps_b.tile([P, w + nu], f32, tag="b", name=f"pm{c}_{us[0]}")
            for j, u in enumerate(us):
                npairs = u // 2
                tail = [(t, None) for t in range(npairs)]
                if u % 2 == 1:
                    tail.append((npairs, 0))
                for i, (t, k) in enumerate(tail):
                    st = (i == 0)
                    sp = (i == len(tail) - 1)
                    if k is None:
                        nc.tensor.matmul(
                            pm[:, j * P:(j + 1) * P],
                            ess[c][t][:, :, u * P:(u + 1) * P],
                            qn8[:, t, :, c, :],
                            start=st, stop=sp, perf_mode=DR,
                        )
                        nc.tensor.matmul(
                            pm[:, w + j:w + j + 1],
                            ess[c][t][:, :, u * P:(u + 1) * P],
                            on8[:],
                            start=st, stop=sp, perf_mode=DR,
                        )
                    else:
                        nc.tensor.matmul(
                            pm[:, j * P:(j + 1) * P],
                            ess[c][t][:, k, u * P:(u + 1) * P],
                            qn8[:, t, k, c, :],
                            start=st, stop=sp,
                        )
                        nc.tensor.matmul(
                            pm[:, w + j:w + j + 1],
                            ess[c][t][:, k, u * P:(u + 1) * P],
                            on8[:, 0],
                            start=st, stop=sp,
                        )
            ynm = ynm_pool.tile([P, w], f16, tag="ynm")
            nc.vector.tensor_copy(ynm[:], pm[:, 0:w])
            nc.vector.tensor_copy(rmirs[c][:, us[0]:us[0] + nu], pm[:, w:w + nu])
            ptm = ps_b.tile([P, w], f16, tag="b", name=f"ptm{c}_{us[0]}")
            for j in range(nu):
                nc.tensor.transpose(
                    ptm[:, j * P:(j + 1) * P], ynm[:, j * P:(j + 1) * P], id16[:]
                )
            nc.vector.tensor_copy(yts[c][:, us[0] * P:(us[0] + nu) * P], ptm[:])

        def snn_unit(c, t):
            # S_nn for chunks 4t..4t+3 of head c: colsums of (q16)^2 blocks
            qsq = tmp_pool.tile([P, 512], f16, tag="qsq")
            nc.vector.tensor_tensor(
                qsq[:], q16full[c][:, t * 512:(t + 1) * 512],
                q16full[c][:, t * 512:(t + 1) * 512], MUL,
            )
            pq = ps_b.tile([P, 4], f32, tag="b", name=f"pq{c}_{t}")
            for j in range(4):
                nc.tensor.matmul(
                    pq[:, j:j + 1],
                    qsq[:, j * P:(j + 1) * P],
                    ones1[:],
                    start=True, stop=True,
                )
            nc.vector.tensor_copy(snns[c][:, t * 4:(t + 1) * 4], pq[:])

        def r_chain(c):
            # r9 = (sum_h rrech + rmir - e^-B8) * e^(B8-9) + Enn9   [P, NCH]
            enn = tmp_pool.tile([P, NCH], f32, tag="enn")
            nc.scalar.activation(
                enn[:], snns[c][:], EXP, bias=nb9[:, 0:1], scale=SCALE / 16.0,
            )
            rr = tmp_pool.tile([P, NCH], f32, tag="rrec")
            nc.vector.tensor_reduce(
                rr[:], rrechs[c][:], mybir.AxisListType.X, ADD
            )
            nc.vector.tensor_tensor(rr[:], rr[:], rmirs[c][:], ADD)
            nc.vector.tensor_scalar_mul(rr[:], rr[:], EB9)
            nc.vector.tensor_tensor(rr[:], rr[:], enn[:], ADD)
            rcp = tmp_pool.tile([P, NCH], f32r, tag="rcp")
            nc.vector.reciprocal(rcp[:], rr[:])
            # broadcast vectors: rbc = 16*e^(B8-9)/r9 ; zbc = Enn9/(4*r9)
            # (evac: yt = (psy + yt_mirror)*rbc + q16*zbc ; zbc <= 0.25)
            offv = tmp_pool.tile([P, 2, NCH], f32r, tag="offv")
            nc.vector.tensor_scalar_mul(offv[:, 0], rcp[:], 16.0 * EB9)
            nc.vector.tensor_tensor(offv[:, 1], rcp[:], enn[:], MUL)
            nc.vector.tensor_scalar_mul(offv[:, 1], offv[:, 1], 0.25)
            pt = ps_b.tile([NCH, 2 * P], f32r, tag="b", name=f"rt{c}")
            nc.tensor.transpose(pt[:, 0:P], offv[:, 0], idr[:])
            nc.tensor.transpose(pt[:, P:2 * P], offv[:, 1], idr[:])
            both16 = tmp_pool.tile([NCH, 2 * P], f16, tag="both16")
            nc.vector.tensor_copy(both16[:], pt[:])
            rbc = rbc_pool.tile([P, N], f16, tag="rbc", name=f"rb{c}")
            zbc = rbc_pool.tile([P, N], f16, tag="zbc", name=f"zb{c}")
            nc.sync.dma_start(rbc[0:1, :], both16[:, 0:P])
            nc.sync.dma_start(zbc[0:1, :], both16[:, P:2 * P])
            nc.gpsimd.partition_broadcast(rbc[:], rbc[0:1, :])
            nc.gpsimd.partition_broadcast(zbc[:], zbc[0:1, :])
            return rbc, zbc

        with (
            tc.tile_pool(name="xt", bufs=4) as xt_pool,
            tc.tile_pool(name="wq", bufs=1) as wq_pool,
            tc.tile_pool(name="q8st", bufs=2) as q8st_pool,
            tc.tile_pool(name="qn8st", bufs=2) as qn8st_pool,
        ):
            make_identity(nc, id16[:])
            nc.vector.tensor_copy(idr[:], id16[:])
            i8st = q8st_pool.tile([P, P], e4, tag="q8s", name="i8p")
            nc.vector.tensor_scalar_mul(i8st[:], id16[:], 64.0)
            nc.sync.dma_start(ia8p[:], i8st[:])
            i8st2 = q8st_pool.tile([P, P], e4, tag="q8s", name="i8n")
            nc.vector.tensor_scalar_mul(i8st2[:], id16[:], -64.0)
            nc.sync.dma_start(ia8n[:], i8st2[:])

            for c in range(HG):
                snns[c] = snn_pool.tile([P, NCH], f32, tag="snn", name=f"sn{c}")

            wqh_sb = wq_pool.tile([P, KP, 2, HCOLS], e4, tag="wqh")
            wql_sb = wq_pool.tile([P, KP, 2, HCOLS], e4, tag="wql")
            xsp = {}

            def load_x(t, engs):
                xsp[t] = (
                    xt_pool.tile([P, KP, 2, 512], e4, tag="xt", name=f"xh{t}"),
                    xt_pool.tile([P, KP, 2, 512], e4, tag="xt", name=f"xl{t}"),
                )
                engs[0].dma_start(xsp[t][0][:], xh[:, t])
                engs[1].dma_start(xsp[t][1][:], xl[:, t])

            # prologue loads: fine-grained halves so qproj starts early
            xsp[0] = (
                xt_pool.tile([P, KP, 2, 512], e4, tag="xt", name="xh0"),
                xt_pool.tile([P, KP, 2, 512], e4, tag="xt", name="xl0"),
            )
            nc.sync.dma_start(wqh_sb[:, 0:2], wqh[:, 0:2])
            nc.scalar.dma_start(xsp[0][0][:, 0:2], xh[:, 0, 0:2])
            nc.gpsimd.dma_start(xsp[0][1][:, 0:2], xl[:, 0, 0:2])
            nc.sync.dma_start(wqh_sb[:, 2:5], wqh[:, 2:5])
            nc.scalar.dma_start(xsp[0][0][:, 2:5], xh[:, 0, 2:5])
            nc.gpsimd.dma_start(xsp[0][1][:, 2:5], xl[:, 0, 2:5])
            nc.sync.dma_start(wqh_sb[:, 5:8], wqh[:, 5:8])
            nc.scalar.dma_start(xsp[0][0][:, 5:8], xh[:, 0, 5:8])
            nc.gpsimd.dma_start(xsp[0][1][:, 5:8], xl[:, 0, 5:8])
            nc.sync.dma_start(wql_sb[:, 0:4], wql[:, 0:4])
            nc.scalar.dma_start(wql_sb[:, 4:8], wql[:, 4:8])
            load_x(1, (nc.scalar, nc.gpsimd))
            edum = rr_pool.tile([P, 1], f32, tag="edum")
            nc.scalar.activation(edum[:], nb8[:, 0:1], EXP)

            pend_tr = []

            def qproj_unit(c, t):
                ps = ps_b.tile([P, 512], f32, tag="b")
                th, tl = xsp[t]
                combos = [(wqh_sb, th, kp) for kp in range(KP)]
                combos += [(wql_sb, th, kp) for kp in range(KP)]
                combos += [(wqh_sb, tl, kp) for kp in range(KP)]
                for i, (w, x, kp) in enumerate(combos):
                    nc.tensor.matmul(
                        ps[:],
                        w[:, kp, :, c * P:(c + 1) * P],
                        x[:, kp],
                        start=(i == 0),
                        stop=(i == len(combos) - 1),
                        perf_mode=DR,
                    )
                # q16full stores 4*Q^T (f16)
                nc.vector.tensor_scalar_mul(
                    q16full[c][:, t * 512:(t + 1) * 512], ps[:], 1.0 / 16.0
                )
                # qt8 = e4m3(4*Q^T), staged then DR-repacked via DMA
                q8s = q8st_pool.tile([P, 512], e4, tag="q8s")
                nc.gpsimd.tensor_copy(q8s[:], q16full[c][:, t * 512:(t + 1) * 512])
                nc.sync.dma_start(qt8[:, :, c, t * 512:(t + 1) * 512], q8s[:])
                if pend_tr:
                    pend_tr.pop(0)()

                def transposes(c=c, t=t):
                    # Q natural fp8 (Q/16) into paired DR layout
                    pt_all = ps_b.tile([P, 512], f16, tag="b")
                    for j in range(4):
                        nc.tensor.transpose(
                            pt_all[:, j * P:(j + 1) * P],
                            q16full[c][:, (t * 4 + j) * P:(t * 4 + j + 1) * P],
                            id16[:],
                        )
                    qn8s = qn8st_pool.tile([P, 512], e4, tag="qn8s")
                    nc.vector.tensor_scalar_mul(qn8s[:], pt_all[:], 1.0 / 64.0)
                    nc.sync.dma_start(
                        qn8[:, 2 * t:2 * t + 2, :, c, :], qn8s[:]
                    )
                    snn_unit(c, t)

                pend_tr.append(transposes)

            # Boot interleave: stream head-0 and head-1 score pieces (feeding
            # the ACT exp pipeline) between qproj units so ACT never starves.
            sc = []
            sc1 = []

            def emit(n):
                for _ in range(n):
                    if sc:
                        sc.pop(0)()
                    elif sc1:
                        sc1.pop(0)()

            def ph_piece(c, v, h, **kw):
                t, k = v // 2, v % 2
                if k == 0 and t not in ess[c]:
                    ess[c][t] = es_pool.tile(
                        [P, 2, N], e4, tag="es", name=f"e{c}_{t}"
                    )
                scores_piece(c, v, h, ess[c][t], k, **kw)

            qproj_unit(0, 0)
            new_head_state(0)
            # rows 0-3 cols [vP,512): only needs the (0,0) qt8 fold -> ACT
            # exp stream starts while qproj is still warming up
            for v in range(4):
                sc.append(lambda v=v: ph_piece(0, v, 0, hi=512, slot=2))
            qproj_unit(1, 0)
            emit(1)
            qproj_unit(2, 0)
            emit(1)
            qproj_unit(3, 0)
            emit(2)
            load_x(2, (nc.sync, nc.sync))
            qproj_unit(0, 1)
            for v in range(8):
                sc.append(lambda v=v: ph_piece(0, v, 0, lo=max(v * P, 512)))
            qproj_unit(1, 1)
            emit(2)
            qproj_unit(2, 1)
            emit(2)
            qproj_unit(3, 1)
            emit(2)
            load_x(3, (nc.sync, nc.sync))
            qproj_unit(0, 2)
            emit(2)
            qproj_unit(0, 3)
            # head-0 phase 2 (qt8[0] complete)
            for v in range(NCH):
                sc.append(lambda v=v: ph_piece(0, v, 1))
                if v >= 3 and v % 2 == 1:
                    sc.append(lambda v=v: mirror_units(0, [v - 2, v - 1]))
            sc.append(lambda: mirror_units(0, [NCH - 1]))
            qproj_unit(1, 2)
            emit(3)
            qproj_unit(1, 3)
            # head 1 (qt8[1] complete)
            sc1.append(lambda: new_head_state(1))
            for v in range(NCH):
                sc1.append(lambda v=v: ph_piece(1, v, 0))
                sc1.append(lambda v=v: ph_piece(1, v, 1))
                if v >= 3 and v % 2 == 1:
                    sc1.append(lambda v=v: mirror_units(1, [v - 2, v - 1]))
            sc1.append(lambda: mirror_units(1, [NCH - 1]))
            qproj_unit(2, 2)
            emit(3)
            qproj_unit(2, 3)
            emit(3)
            qproj_unit(3, 2)
            emit(3)
            qproj_unit(3, 3)
            while pend_tr:
                pend_tr.pop(0)()

        with (
            tc.tile_pool(name="wo8", bufs=1) as wo8_pool,
            tc.tile_pool(name="y8", bufs=1) as y8p,
            tc.tile_pool(name="osb", bufs=3) as o_pool,
        ):
            wo8h_sb = wo8_pool.tile([P, HG, D], e4, tag="wo8h")
            wo8l_sb = wo8_pool.tile([P, HG, D], e4, tag="wo8l")
            nc.sync.dma_start(wo8h_sb[:], wo8h[:])
            nc.gpsimd.dma_start(wo8l_sb[:], wo8l[:])
            y8h = y8p.tile([P, HG, N], e4, tag="y8h")
            y8l = y8p.tile([P, HG, N], e4, tag="y8l")
            rbz = {}

            def outproj_unit(a, grp):
                # grp 0: heads 0-1 partial -> outA ; grp 1: heads 2-3 -> out
                # grp 2: all heads (combined) -> out
                dst = outA3 if grp == 0 else out3
                ot = o_pool.tile([P, D], f16, tag="ot")
                for d4 in range(SP):
                    ps = ps_b.tile([P, 512], f32, tag="b")
                    combos = []
                    for cc in ((0,), (2,), (0, 2))[grp]:
                        combos += [
                            (y8h, wo8h_sb, cc),
                            (y8h, wo8l_sb, cc),
                            (y8l, wo8h_sb, cc),
                        ]
                    for i, (yy, ww, cc) in enumerate(combos):
                        nc.tensor.matmul(
                            ps[:],
                            yy[:, cc:cc + 2, a * P:(a + 1) * P],
                            ww[:, cc:cc + 2, d4 * 512:(d4 + 1) * 512],
                            start=(i == 0),
                            stop=(i == len(combos) - 1),
                            perf_mode=DR,
                        )
                    if d4 % 2 == 0:
                        nc.vector.tensor_scalar_mul(
                            ot[:, d4 * 512:(d4 + 1) * 512], ps[:], 1.0 / 64.0
                        )
                    else:
                        nc.scalar.mul(
                            ot[:, d4 * 512:(d4 + 1) * 512], ps[:], 1.0 / 64.0
                        )
                        # stream each 1KB half out as soon as it's ready
                        half = d4 // 2
                        eng = nc.sync
                        eng.dma_start(
                            dst[:, a, half * 1024:(half + 1) * 1024],
                            ot[:, half * 1024:(half + 1) * 1024],
                        )

            def pv_mm(pc, s):
                # one span of YT via paired DR matmuls
                psy = ps_b.tile([P, 512], f32, tag="b", name=f"psy{pc}_{s}")
                lo_s = s * 512
                mms = []
                for t in range(2 * s):
                    mms.append((t, None, lo_s))
                mms.append((2 * s, 0, lo_s))                # solo chunk 4s
                mms.append((2 * s, None, lo_s + 128))       # pair (4s,4s+1)
                mms.append((2 * s + 1, 0, lo_s + 256))      # solo chunk 4s+2
                mms.append((2 * s + 1, None, lo_s + 384))   # pair (4s+2,4s+3)
                for i, (t, k, lo) in enumerate(mms):
                    st = (i == 0)
                    sp = (i == len(mms) - 1)
                    hi = (s + 1) * 512
                    if k is None:
                        nc.tensor.matmul(
                            psy[:, lo - lo_s:hi - lo_s],
                            qn8[:, t, :, pc, 0:P],
                            ess[pc][t][:, :, lo:hi],
                            start=st, stop=sp, perf_mode=DR,
                        )
                    else:
                        nc.tensor.matmul(
                            psy[:, lo - lo_s:lo - lo_s + 128],
                            qn8[:, t, k, pc, 0:P],
                            ess[pc][t][:, k, lo:lo + 128],
                            start=st, stop=sp,
                        )
                return psy

            def pv_evac(pc, s, psy, rbc, zbc):
                # yt = ((psy + yt_mirror) + q16*zrbc) * rbc  -> y8 hi/lo
                yt = yts[pc]
                sl = slice(s * 512, (s + 1) * 512)
                nc.vector.tensor_tensor(yt[:, sl], psy[:], yt[:, sl], ADD)
                nc.vector.tensor_tensor(yt[:, sl], yt[:, sl], rbc[:, sl], MUL)
                dg = tmp_pool.tile([P, 512], f16, tag="dg", name=f"dg{pc}_{s}")
                nc.vector.tensor_tensor(
                    dg[:], q16full[pc][:, sl], zbc[:, sl], MUL
                )
                nc.vector.tensor_tensor(yt[:, sl], yt[:, sl], dg[:], ADD)
                nc.gpsimd.tensor_copy(y8h[:, pc, sl], yt[:, sl])
                nc.gpsimd.tensor_tensor(
                    y8l[:, pc, sl], yt[:, sl], y8h[:, pc, sl], SUB
                )

            def pv_span(pc, s, rbc, zbc):
                pv_evac(pc, s, pv_mm(pc, s), rbc, zbc)

            # boot tail: drain head-0 stream, then overlap the head-1 exp
            # tail with PV(0) spans and the r-chains
            while sc:
                sc.pop(0)()
            rbz[0] = r_chain(0)
            s0 = 0
            while sc1:
                for _ in range(6):
                    if sc1:
                        sc1.pop(0)()
                if s0 < SP:
                    pv_span(0, s0, *rbz[0])
                    s0 += 1
            while s0 < SP:
                pv_span(0, s0, *rbz[0])
                s0 += 1
            rbz[1] = r_chain(1)

            # ============ stage 1: scores(2) || PV(1) ============
            new_head_state(2)
            for v in range(NCH):
                scores_row(2, v)
                if v >= 3 and v % 2 == 1:
                    mirror_units(2, [v - 2, v - 1])
                    if v >= 9:
                        pv_span(1, (v - 9) // 2, *rbz[1])
            mirror_units(2, [NCH - 1])
            rbz[2] = r_chain(2)

            # ===== stage 2: scores(3) || PV(2) + outproj-A (heads 0-1) =====
            new_head_state(3)
            for v in range(NCH):
                scores_row(3, v)
                if v >= 3 and v % 2 == 1:
                    mirror_units(3, [v - 2, v - 1])
                if v % 4 == 2:
                    pv_span(2, v // 4, *rbz[2])
            mirror_units(3, [NCH - 1])
            rbz[3] = r_chain(3)

            # lull fill: heads-0/1 partials are independent of r_chain(3)
            for a in range(6):
                outproj_unit(a, 0)

            # == stage 3: PV(3) + outproj (B for a<6, combined for a>=6) ==
            psys = {0: pv_mm(3, 0), 1: pv_mm(3, 1)}
            for s in range(SP):
                pv_evac(3, s, psys[s], *rbz[3])
                if s + 2 < SP:
                    psys[s + 2] = pv_mm(3, s + 2)
                for a in range(4 * s, 4 * s + 4):
                    outproj_unit(a, 1 if a < 6 else 2)

    nc.compile()
    return nc


def _hi_lo(arr):
    hi = arr.astype(ml_dtypes.float8_e4m3)
    lo = (arr - hi.astype(np.float32)).astype(ml_dtypes.float8_e4m3)
    return hi, lo


def _pack_x(xt_hl):
    return np.ascontiguousarray(
        xt_hl.reshape(KP, 2, P, SP, 512).transpose(2, 3, 0, 1, 4)
    )


def kernel(x, Wq, Wo, bo):
    x = np.asarray(x, dtype=np.float32)
    Wq = np.asarray(Wq, dtype=np.float32)
    Wo = np.asarray(Wo, dtype=np.float32)
    bo = np.asarray(bo, dtype=np.float32)
    B = x.shape[0]
    assert B == 2 and x.shape == (B, N, D)
    assert Wq.shape == (D, D) and Wo.shape == (D, D)

    if "nc" not in _CACHE:
        _CACHE["nc"] = build_nc()
    nc = _CACHE["nc"]

    packed_x = []
    for b in range(B):
        hi, lo = _hi_lo(np.ascontiguousarray(x[b].T))
        packed_x.append((_pack_x(hi), _pack_x(lo)))
    in_maps = []
    for core in range(8):
        b, hg = core // 4, core % 4
        wq_s = WQS * Wq[:, hg * HCOLS:(hg + 1) * HCOLS]
        wqh, wql = _hi_lo(np.ascontiguousarray(wq_s))
        wo8h_a, wo8l_a = _hi_lo(
            np.ascontiguousarray(WQS * Wo[hg * HCOLS:(hg + 1) * HCOLS, :])
        )
        in_maps.append(
            {
                "xh": packed_x[b][0],
                "xl": packed_x[b][1],
                "wqh": np.ascontiguousarray(
                    wqh.reshape(KP, 2, P, HCOLS).transpose(2, 0, 1, 3)
                ),
                "wql": np.ascontiguousarray(
                    wql.reshape(KP, 2, P, HCOLS).transpose(2, 0, 1, 3)
                ),
                "wo8h": np.ascontiguousarray(
                    wo8h_a.reshape(HG, P, D).transpose(1, 0, 2)
                ),
                "wo8l": np.ascontiguousarray(
                    wo8l_a.reshape(HG, P, D).transpose(1, 0, 2)
                ),
            }
        )

    res = run_bass_kernel_spmd(nc, in_maps, list(range(8)))
    _CACHE["last_res"] = res
    out = np.zeros((B, N, D), dtype=np.float32)
    for core in range(8):
        b = core // 4
        out[b] += res.results[core]["out"].astype(np.float32)
        out[b][0:768] += res.results[core]["outA"][0:768].astype(np.float32)
    out += bo
    return out


# revision 3
# speedup vs baseline: 1.0491x; 1.0199x over previous
"""Trainium2 Bass kernel for nn_MultiHeadAttention_78864189489198 — v3.

fp8 paired-es variant.

E = exp(S - B8) is stored in fp8 (e4m3) chunk-PAIRED tiles es8[t] of
shape [P, 2, N] holding row-chunks (2t, 2t+1).  This lets both the PV
(attn @ V) and mirror matmuls run in DoubleRow perf mode (contraction
256 = two chunks at once, 0.5 cyc/col), cutting their PE cost ~4x/3.3x
vs the fp16 v2 path.

The huge diagonal E_nn (softmax is diagonally dominant here) cannot
live in fp8: the scores diagonal is masked to 0 in PSUM before exp
(DVE multiply by (1-I)), and the diagonal contribution is restored
exactly:  S_nn = rowsum(q16^2) via per-chunk ones-matmuls,
E_nn = exp(S_nn - 9), Y += (E_nn / r) * Q with r assembled from the
exp accumulators + mirror ones-column + E_nn (consistent numerator /
denominator, so S_nn quantization error cancels).

Everything else as v2: fp8 hi/lo DoubleRow qproj/outproj, host-side
batch x head-group sharding (8 cores), fp16 partial outputs summed on
host.
"""

import numpy as np
import ml_dtypes

import concourse.bass as bass
import concourse.mybir as mybir
import concourse.tile as tile
from concourse import bacc
from concourse.bass_utils import run_bass_kernel_spmd
from concourse.masks import make_identity
from contextlib import ExitStack

P = 128
N = 2048
D = 2048
KP = 8
HG = 4
HD = 128
HCOLS = HG * HD
SP = N // 512
NCH = N // P
NPAIR = NCH // 2
SCALE = HD ** -0.5
C_BIAS = 9.0
B8 = 3.5
WQS = 64.0

f32 = mybir.dt.float32
f32r = mybir.dt.float32r
f16 = mybir.dt.float16
e4 = mybir.dt.float8e4
DR = mybir.MatmulPerfMode.DoubleRow

_CACHE = {}

EXP = mybir.ActivationFunctionType.Exp
ADD = mybir.AluOpType.add
SUB = mybir.AluOpType.subtract
MUL = mybir.AluOpType.mult


def build_nc():
    nc = bacc.Bacc("TRN2", target_bir_lowering=False, debug=False)
    xh = nc.dram_tensor("xh", [P, SP, KP, 2, 512], e4, kind="ExternalInput")
    xl = nc.dram_tensor("xl", [P, SP, KP, 2, 512], e4, kind="ExternalInput")
    wqh = nc.dram_tensor("wqh", [P, KP, 2, HCOLS], e4, kind="ExternalInput")
    wql = nc.dram_tensor("wql", [P, KP, 2, HCOLS], e4, kind="ExternalInput")
    wo8h = nc.dram_tensor("wo8h", [P, HG, D], e4, kind="ExternalInput")
    wo8l = nc.dram_tensor("wo8l", [P, HG, D], e4, kind="ExternalInput")
    out = nc.dram_tensor("out", [N, D], f16, kind="ExternalOutput")
    out3 = out.rearrange("(a p) n -> p a n", p=P)
    outA = nc.dram_tensor("outA", [N, D], f16, kind="ExternalOutput")
    outA3 = outA.rearrange("(a p) n -> p a n", p=P)

    EB8 = float(np.exp(-B8))          # masked-diag residue per row
    EB9 = float(np.exp(B8 - C_BIAS))  # es8 scale -> e^-9 scale

    with (
        nc.allow_low_precision(reason="fp8/fp16 dataflow is intentional"),
        tile.TileContext(nc) as tc,
        ExitStack() as ctx,
    ):
        const_pool = ctx.enter_context(tc.tile_pool(name="const", bufs=1))
        qt8_pool = ctx.enter_context(tc.tile_pool(name="qt8", bufs=1))
        qn8_pool = ctx.enter_context(tc.tile_pool(name="qn8", bufs=1))
        q16_pool = ctx.enter_context(tc.tile_pool(name="q16f", bufs=4))
        es_pool = ctx.enter_context(tc.tile_pool(name="es8", bufs=16))
        rr_pool = ctx.enter_context(tc.tile_pool(name="rr", bufs=4))
        tmp_pool = ctx.enter_context(tc.tile_pool(name="tmp", bufs=3))
        snn_pool = ctx.enter_context(tc.tile_pool(name="snn", bufs=4))
        rbc_pool = ctx.enter_context(tc.tile_pool(name="rbc", bufs=4))
        yt_pool = ctx.enter_context(tc.tile_pool(name="yt", bufs=2))
        ynm_pool = ctx.enter_context(tc.tile_pool(name="ynm", bufs=4))
        ps_s = ctx.enter_context(tc.tile_pool(name="ps_s", bufs=2, space="PSUM"))
        ps_b = ctx.enter_context(tc.tile_pool(name="ps_b", bufs=4, space="PSUM"))

        idr = const_pool.tile([P, P], f32r, tag="idr")
        id16 = const_pool.tile([P, P], f16, tag="id16")
        # DR-layout fp8 identities: ia8p^T @ ia8n = -4096*I (diag canceller)
        ia8p = const_pool.tile([64, 2, P], e4, tag="ia8p")
        ia8n = const_pool.tile([64, 2, P], e4, tag="ia8n")
        nb8 = const_pool.tile([P, 1], f32, tag="nb8")
        nc.gpsimd.memset(nb8[:], -B8)
        nb9 = const_pool.tile([P, 1], f32, tag="nb9")
        nc.gpsimd.memset(nb9[:], -C_BIAS)
        ones1 = const_pool.tile([P, 1], f16, tag="ones1")
        nc.gpsimd.memset(ones1[:], 1.0)

        # qt8[p, r, c, n] : 4*Q^T head c, DR layout (hd = 2p+r-ish)
        qt8 = qt8_pool.tile([64, 2, HG, N], e4, tag="qt8")
        # qn8[p, t, k, c, :] = Q[(2t+k)*P+p, head c, :]/16
        # (128-multiple strides: fp8 Ldweights restriction)
        qn8 = qn8_pool.tile([P, NPAIR, 2, HG, P], e4, tag="qn8")
        # fp8 ones for the mirror r-column matmuls
        on8 = const_pool.tile([P, 2, 1], e4, tag="on8")
        nc.gpsimd.memset(on8[:], 1.0)
        # q16full[c] = 4*Q^T head c, f16, persistent (diag path + src of qt8/qn8)
        q16full = {}
        for c in range(HG):
            q16full[c] = q16_pool.tile([P, N], f16, tag="q16f", name=f"q16f{c}")

        yts = {}
        rrechs = {}
        rmirs = {}
        snns = {}
        ess = {}       # ess[c][t] = es8 pair tile [P, 2, N]

        def new_head_state(c):
            rrechs[c] = rr_pool.tile([P, NCH, 3], f32, tag="rrech", name=f"rr{c}")
            rmirs[c] = rr_pool.tile([P, NCH], f32, tag="rmir", name=f"rm{c}")
            nc.vector.memset(rrechs[c][:], 0.0)
            nc.vector.memset(rmirs[c][:], 0.0)
            yts[c] = yt_pool.tile([P, N], f16, tag="yt", name=f"yt{c}")
            nc.gpsimd.memset(yts[c][:, 0:P], 0.0)
            ess[c] = {}

        def scores_piece(c, v, h, es, k, lo=None, hi=None, slot=None):
            # scores cols [max(128v, 1024h), 1024(h+1)) for row-chunk v
            if lo is None:
                lo = max(P * v, 1024 * h)
            if hi is None:
                hi = 1024 * (h + 1)
            if slot is None:
                slot = h
            if lo >= hi:
                return
            ps = ps_s.tile([P, 1024], f32, tag="s")
            col = lo
            while col < hi:
                w = min(512 - col % 512, hi - col)
                nc.tensor.matmul(
                    ps[:, col - 1024 * h:col - 1024 * h + w],
                    qt8[:, :, c, v * P:(v + 1) * P],
                    qt8[:, :, c, col:col + w],
                    start=True,
                    stop=True,
                    perf_mode=DR,
                )
                col += w
            if lo == v * P:
                # diag block leads this piece: add -4096*I so exp of the
                # exact diagonal flushes to 0 in fp8 (residue ~1e-4, ignored)
                nc.tensor.matmul(
                    ps[:, lo - 1024 * h:lo - 1024 * h + P],
                    ia8p[:],
                    ia8n[:],
                    start=False,
                    stop=True,
                    perf_mode=DR,
                )
            nc.scalar.activation(
                es[:, k, lo:hi],
                ps[:, lo - 1024 * h:hi - 1024 * h],
                EXP,
                bias=nb8[:, 0:1],
                scale=SCALE / 16.0,
                accum_out=rrechs[c][:, v, slot:slot + 1],
            )

        def scores_row(c, v):
            t, k = v // 2, v % 2
            if k == 0:
                ess[c][t] = es_pool.tile([P, 2, N], e4, tag="es", name=f"e{c}_{t}")
            es = ess[c][t]
            scores_piece(c, v, 0, es, k)
            scores_piece(c, v, 1, es, k)

        def mirror_units(c, us):
            # Y[u-chunk, :] += sum_{v<u} E8[v, u-block]^T @ [Q_v/16 | 1]
            # us: 1-2 consecutive chunk indices sharing one PSUM tile
            nu = len(us)
            w = 128 * nu
            pm = ps_b.tile([P, w + nu], f32, tag="b", name=f"pm{c}_{us[0]}")
            for j, u in enumerate(us):
                npairs = u // 2
                tail = [(t, None) for t in range(npairs)]
                if u % 2 == 1:
                    tail.append((npairs, 0))
                for i, (t, k) in enumerate(tail):
                    st = (i == 0)
                    sp = (i == len(tail) - 1)
                    if k is None:
                        nc.tensor.matmul(
                            pm[:, j * P:(j + 1) * P],
                            ess[c][t][:, :, u * P:(u + 1) * P],
                            qn8[:, t, :, c, :],
                            start=st, stop=sp, perf_mode=DR,
                        )
                        nc.tensor.matmul(
                            pm[:, w + j:w + j + 1],
                            ess[c][t][:, :, u * P:(u + 1) * P],
                            on8[:],
                            start=st, stop=sp, perf_mode=DR,
                        )
                    else:
                        nc.tensor.matmul(
                            pm[:, j * P:(j + 1) * P],
                            ess[c][t][:, k, u * P:(u + 1) * P],
                            qn8[:, t, k, c, :],
                            start=st, stop=sp,
                        )
                        nc.tensor.matmul(
                            pm[:, w + j:w + j + 1],
                            ess[c][t][:, k, u * P:(u + 1) * P],
                            on8[:, 0],
                            start=st, stop=sp,
                        )
            ynm = ynm_pool.tile([P, w], f16, tag="ynm")
            nc.vector.tensor_copy(ynm[:], pm[:, 0:w])
            nc.vector.tensor_copy(rmirs[c][:, us[0]:us[0] + nu], pm[:, w:w + nu])
            ptm = ps_b.tile([P, w], f16, tag="b", name=f"ptm{c}_{us[0]}")
            for j in range(nu):
                nc.tensor.transpose(
                    ptm[:, j * P:(j + 1) * P], ynm[:, j * P:(j + 1) * P], id16[:]
                )
            nc.vector.tensor_copy(yts[c][:, us[0] * P:(us[0] + nu) * P], ptm[:])

        def snn_unit(c, t):
            # S_nn for chunks 4t..4t+3 of head c: colsums of (q16)^2 blocks
            qsq = tmp_pool.tile([P, 512], f16, tag="qsq")
            nc.vector.tensor_tensor(
                qsq[:], q16full[c][:, t * 512:(t + 1) * 512],
                q16full[c][:, t * 512:(t + 1) * 512], MUL,
            )
            pq = ps_b.tile([P, 4], f32, tag="b", name=f"pq{c}_{t}")
            for j in range(4):
                nc.tensor.matmul(
                    pq[:, j:j + 1],
                    qsq[:, j * P:(j + 1) * P],
                    ones1[:],
                    start=True, stop=True,
                )
            nc.vector.tensor_copy(snns[c][:, t * 4:(t + 1) * 4], pq[:])

        def r_chain(c):
            # r9 = (sum_h rrech + rmir - e^-B8) * e^(B8-9) + Enn9   [P, NCH]
            enn = tmp_pool.tile([P, NCH], f32, tag="enn")
            nc.scalar.activation(
                enn[:], snns[c][:], EXP, bias=nb9[:, 0:1], scale=SCALE / 16.0,
            )
            rr = tmp_pool.tile([P, NCH], f32, tag="rrec")
            nc.vector.tensor_reduce(
                rr[:], rrechs[c][:], mybir.AxisListType.X, ADD
            )
            nc.vector.tensor_tensor(rr[:], rr[:], rmirs[c][:], ADD)
            nc.vector.tensor_scalar_mul(rr[:], rr[:], EB9)
            nc.vector.tensor_tensor(rr[:], rr[:], enn[:], ADD)
            rcp = tmp_pool.tile([P, NCH], f32r, tag="rcp")
            nc.vector.reciprocal(rcp[:], rr[:])
            # broadcast vectors: rbc = 16*e^(B8-9)/r9 ; zbc = Enn9/(4*r9)
            # (evac: yt = (psy + yt_mirror)*rbc + q16*zbc ; zbc <= 0.25)
            offv = tmp_pool.tile([P, 2, NCH], f32r, tag="offv")
            nc.vector.tensor_scalar_mul(offv[:, 0], rcp[:], 16.0 * EB9)
            nc.vector.tensor_tensor(offv[:, 1], rcp[:], enn[:], MUL)
            nc.vector.tensor_scalar_mul(offv[:, 1], offv[:, 1], 0.25)
            pt = ps_b.tile([NCH, 2 * P], f32r, tag="b", name=f"rt{c}")
            nc.tensor.transpose(pt[:, 0:P], offv[:, 0], idr[:])
            nc.tensor.transpose(pt[:, P:2 * P], offv[:, 1], idr[:])
            both16 = tmp_pool.tile([NCH, 2 * P], f16, tag="both16")
            nc.vector.tensor_copy(both16[:], pt[:])
            rbc = rbc_pool.tile([P, N], f16, tag="rbc", name=f"rb{c}")
            zbc = rbc_pool.tile([P, N], f16, tag="zbc", name=f"zb{c}")
            nc.sync.dma_start(rbc[0:1, :], both16[:, 0:P])
            nc.sync.dma_start(zbc[0:1, :], both16[:, P:2 * P])
            nc.gpsimd.partition_broadcast(rbc[:], rbc[0:1, :])
            nc.gpsimd.partition_broadcast(zbc[:], zbc[0:1, :])
            return rbc, zbc

        with (
            tc.tile_pool(name="xt", bufs=4) as xt_pool,
            tc.tile_pool(name="wq", bufs=1) as wq_pool,
            tc.tile_pool(name="q8st", bufs=2) as q8st_pool,
            tc.tile_pool(name="qn8st", bufs=2) as qn8st_pool,
        ):
            make_identity(nc, id16[:])
            nc.vector.tensor_copy(idr[:], id16[:])
            i8st = q8st_pool.tile([P, P], e4, tag="q8s", name="i8p")
            nc.vector.tensor_scalar_mul(i8st[:], id16[:], 64.0)
            nc.sync.dma_start(ia8p[:], i8st[:])
            i8st2 = q8st_pool.tile([P, P], e4, tag="q8s", name="i8n")
            nc.vector.tensor_scalar_mul(i8st2[:], id16[:], -64.0)
            nc.sync.dma_start(ia8n[:], i8st2[:])

            for c in range(HG):
                snns[c] = snn_pool.tile([P, NCH], f32, tag="snn", name=f"sn{c}")

            wqh_sb = wq_pool.tile([P, KP, 2, HCOLS], e4, tag="wqh")
            wql_sb = wq_pool.tile([P, KP, 2, HCOLS], e4, tag="wql")
            xsp = {}

            def load_x(t, engs):
                xsp[t] = (
                    xt_pool.tile([P, KP, 2, 512], e4, tag="xt", name=f"xh{t}"),
                    xt_pool.tile([P, KP, 2, 512], e4, tag="xt", name=f"xl{t}"),
                )
                engs[0].dma_start(xsp[t][0][:], xh[:, t])
                engs[1].dma_start(xsp[t][1][:], xl[:, t])

            # prologue loads: fine-grained halves so qproj starts early
            xsp[0] = (
                xt_pool.tile([P, KP, 2, 512], e4, tag="xt", name="xh0"),
                xt_pool.tile([P, KP, 2, 512], e4, tag="xt", name="xl0"),
            )
            nc.sync.dma_start(wqh_sb[:, 0:2], wqh[:, 0:2])
            nc.scalar.dma_start(xsp[0][0][:, 0:2], xh[:, 0, 0:2])
            nc.gpsimd.dma_start(xsp[0][1][:, 0:2], xl[:, 0, 0:2])
            nc.sync.dma_start(wqh_sb[:, 2:5], wqh[:, 2:5])
            nc.scalar.dma_start(xsp[0][0][:, 2:5], xh[:, 0, 2:5])
            nc.gpsimd.dma_start(xsp[0][1][:, 2:5], xl[:, 0, 2:5])
            nc.sync.dma_start(wqh_sb[:, 5:8], wqh[:, 5:8])
            nc.scalar.dma_start(xsp[0][0][:, 5:8], xh[:, 0, 5:8])
            nc.gpsimd.dma_start(xsp[0][1][:, 5:8], xl[:, 0, 5:8])
            nc.sync.dma_start(wql_sb[:, 0:4], wql[:, 0:4])
            nc.scalar.dma_start(wql_sb[:, 4:8], wql[:, 4:8])
            load_x(1, (nc.scalar, nc.gpsimd))
            edum = rr_pool.tile([P, 1], f32, tag="edum")
            nc.scalar.activation(edum[:], nb8[:, 0:1], EXP)

            pend_tr = []

            def qproj_unit(c, t):
                ps = ps_b.tile([P, 512], f32, tag="b")
                th, tl = xsp[t]
                combos = [(wqh_sb, th, kp) for kp in range(KP)]
                combos += [(wql_sb, th, kp) for kp in range(KP)]
                combos += [(wqh_sb, tl, kp) for kp in range(KP)]
                for i, (w, x, kp) in enumerate(combos):
                    nc.tensor.matmul(
                        ps[:],
                        w[:, kp, :, c * P:(c + 1) * P],
                        x[:, kp],
                        start=(i == 0),
                        stop=(i == len(combos) - 1),
                        perf_mode=DR,
                    )
                # q16full stores 4*Q^T (f16)
                nc.vector.tensor_scalar_mul(
                    q16full[c][:, t * 512:(t + 1) * 512], ps[:], 1.0 / 16.0
                )
                # qt8 = e4m3(4*Q^T), staged then DR-repacked via DMA
                q8s = q8st_pool.tile([P, 512], e4, tag="q8s")
                nc.gpsimd.tensor_copy(q8s[:], q16full[c][:, t * 512:(t + 1) * 512])
                nc.sync.dma_start(qt8[:, :, c, t * 512:(t + 1) * 512], q8s[:])
                if pend_tr:
                    pend_tr.pop(0)()

                def transposes(c=c, t=t):
                    # Q natural fp8 (Q/16) into paired DR layout
                    pt_all = ps_b.tile([P, 512], f16, tag="b")
                    for j in range(4):
                        nc.tensor.transpose(
                            pt_all[:, j * P:(j + 1) * P],
                            q16full[c][:, (t * 4 + j) * P:(t * 4 + j + 1) * P],
                            id16[:],
                        )
                    qn8s = qn8st_pool.tile([P, 512], e4, tag="qn8s")
                    nc.vector.tensor_scalar_mul(qn8s[:], pt_all[:], 1.0 / 64.0)
                    nc.sync.dma_start(
                        qn8[:, 2 * t:2 * t + 2, :, c, :], qn8s[:]
                    )
                    snn_unit(c, t)

                pend_tr.append(transposes)

            # Boot interleave: stream head-0 and head-1 score pieces (feeding
            # the ACT exp pipeline) between qproj units so ACT never starves.
            sc = []
            sc1 = []

            def emit(n):
                for _ in range(n):
                    if sc:
                        sc.pop(0)()
                    elif sc1:
                        sc1.pop(0)()

            def ph_piece(c, v, h, **kw):
                t, k = v // 2, v % 2
                if k == 0 and t not in ess[c]:
                    ess[c][t] = es_pool.tile(
                        [P, 2, N], e4, tag="es", name=f"e{c}_{t}"
                    )
                scores_piece(c, v, h, ess[c][t], k, **kw)

            qproj_unit(0, 0)
            new_head_state(0)
            # rows 0-3 cols [vP,512): only needs the (0,0) qt8 fold -> ACT
            # exp stream starts while qproj is still warming up
            for v in range(4):
                sc.append(lambda v=v: ph_piece(0, v, 0, hi=512, slot=2))
            qproj_unit(1, 0)
            emit(1)
            qproj_unit(2, 0)
            emit(1)
            qproj_unit(3, 0)
            emit(2)
            load_x(2, (nc.sync, nc.sync))
            qproj_unit(0, 1)
            for v in range(8):
                sc.append(lambda v=v: ph_piece(0, v, 0, lo=max(v * P, 512)))
            qproj_unit(1, 1)
            emit(2)
            qproj_unit(2, 1)
            emit(2)
            qproj_unit(3, 1)
            emit(2)
            load_x(3, (nc.sync, nc.sync))
            qproj_unit(0, 2)
            emit(2)
            qproj_unit(1, 2)
            emit(2)
            qproj_unit(0, 3)
            # head-0 phase 2 (qt8[0] complete)
            for v in range(NCH):
                sc.append(lambda v=v: ph_piece(0, v, 1))
                if v >= 4 and v % 2 == 0:
                    sc.append(lambda v=v: mirror_units(0, [v - 3, v - 2]))
            sc.append(lambda: mirror_units(0, [NCH - 3, NCH - 2]))
            sc.append(lambda: mirror_units(0, [NCH - 1]))
            qproj_unit(1, 3)
            # head 1 (qt8[1] complete)
            sc1.append(lambda: new_head_state(1))
            for v in range(NCH):
                sc1.append(lambda v=v: ph_piece(1, v, 0))
                sc1.append(lambda v=v: ph_piece(1, v, 1))
                if v >= 4 and v % 2 == 0:
                    sc1.append(lambda v=v: mirror_units(1, [v - 3, v - 2]))
            sc1.append(lambda: mirror_units(1, [NCH - 3, NCH - 2]))
            sc1.append(lambda: mirror_units(1, [NCH - 1]))
            qproj_unit(2, 2)
            emit(4)
            qproj_unit(2, 3)
            emit(4)
            qproj_unit(3, 2)
            emit(4)
            qproj_unit(3, 3)
            while pend_tr:
                pend_tr.pop(0)()

        with (
            tc.tile_pool(name="wo8", bufs=1) as wo8_pool,
            tc.tile_pool(name="y8", bufs=1) as y8p,
            tc.tile_pool(name="osb", bufs=3) as o_pool,
        ):
            wo8h_sb = wo8_pool.tile([P, HG, D], e4, tag="wo8h")
            wo8l_sb = wo8_pool.tile([P, HG, D], e4, tag="wo8l")
            nc.sync.dma_start(wo8h_sb[:], wo8h[:])
            nc.gpsimd.dma_start(wo8l_sb[:], wo8l[:])
            y8h = y8p.tile([P, HG, N], e4, tag="y8h")
            y8l = y8p.tile([P, HG, N], e4, tag="y8l")
            rbz = {}

            def outproj_unit(a, grp):
                # grp 0: heads 0-1 partial -> outA ; grp 1: heads 2-3 -> out
                # grp 2: all heads (combined) -> out
                dst = outA3 if grp == 0 else out3
                ot = o_pool.tile([P, D], f16, tag="ot")
                for d4 in range(SP):
                    pool = ps_s if d4 % 2 else ps_b
                    ps = pool.tile([P, 512], f32, tag="s" if d4 % 2 else "b")
                    combos = []
                    for cc in ((0,), (2,), (0, 2))[grp]:
                        combos += [
                            (y8h, wo8h_sb, cc),
                            (y8h, wo8l_sb, cc),
                            (y8l, wo8h_sb, cc),
                        ]
                    for i, (yy, ww, cc) in enumerate(combos):
                        nc.tensor.matmul(
                            ps[:],
                            yy[:, cc:cc + 2, a * P:(a + 1) * P],
                            ww[:, cc:cc + 2, d4 * 512:(d4 + 1) * 512],
                            start=(i == 0),
                            stop=(i == len(combos) - 1),
                            perf_mode=DR,
                        )
                    if d4 % 2 == 0:
                        nc.vector.tensor_scalar_mul(
                            ot[:, d4 * 512:(d4 + 1) * 512], ps[:], 1.0 / 64.0
                        )
                    else:
                        nc.scalar.mul(
                            ot[:, d4 * 512:(d4 + 1) * 512], ps[:], 1.0 / 64.0
                        )
                        # stream each 1KB half out as soon as it's ready
                        half = d4 // 2
                        eng = nc.sync
                        eng.dma_start(
                            dst[:, a, half * 1024:(half + 1) * 1024],
                            ot[:, half * 1024:(half + 1) * 1024],
                        )

            def pv_mm(pc, s):
                # one span of YT via paired DR matmuls
                psy = ps_b.tile([P, 512], f32, tag="b", name=f"psy{pc}_{s}")
                lo_s = s * 512
                mms = []
                for t in range(2 * s):
                    mms.append((t, None, lo_s))
                mms.append((2 * s, 0, lo_s))                # solo chunk 4s
                mms.append((2 * s, None, lo_s + 128))       # pair (4s,4s+1)
                mms.append((2 * s + 1, 0, lo_s + 256))      # solo chunk 4s+2
                mms.append((2 * s + 1, None, lo_s + 384))   # pair (4s+2,4s+3)
                for i, (t, k, lo) in enumerate(mms):
                    st = (i == 0)
                    sp = (i == len(mms) - 1)
                    hi = (s + 1) * 512
                    if k is None:
                        nc.tensor.matmul(
                            psy[:, lo - lo_s:hi - lo_s],
                            qn8[:, t, :, pc, 0:P],
                            ess[pc][t][:, :, lo:hi],
                            start=st, stop=sp, perf_mode=DR,
                        )
                    else:
                        nc.tensor.matmul(
                            psy[:, lo - lo_s:lo - lo_s + 128],
                            qn8[:, t, k, pc, 0:P],
                            ess[pc][t][:, k, lo:lo + 128],
                            start=st, stop=sp,
                        )
                return psy

            def pv_evac(pc, s, psy, rbc, zbc):
                # yt = ((psy + yt_mirror) + q16*zrbc) * rbc  -> y8 hi/lo
                yt = yts[pc]
                sl = slice(s * 512, (s + 1) * 512)
                nc.vector.tensor_tensor(yt[:, sl], psy[:], yt[:, sl], ADD)
                nc.vector.tensor_tensor(yt[:, sl], yt[:, sl], rbc[:, sl], MUL)
                dg = tmp_pool.tile([P, 512], f16, tag="dg", name=f"dg{pc}_{s}")
                nc.vector.tensor_tensor(
                    dg[:], q16full[pc][:, sl], zbc[:, sl], MUL
                )
                nc.vector.tensor_tensor(yt[:, sl], yt[:, sl], dg[:], ADD)
                nc.gpsimd.tensor_copy(y8h[:, pc, sl], yt[:, sl])
                nc.gpsimd.tensor_tensor(
                    y8l[:, pc, sl], yt[:, sl], y8h[:, pc, sl], SUB
                )

            def pv_span(pc, s, rbc, zbc):
                pv_evac(pc, s, pv_mm(pc, s), rbc, zbc)

            # boot tail: drain head-0 stream, then overlap the head-1 exp
            # tail with PV(0) spans and the r-chains
            while sc:
                sc.pop(0)()
            rbz[0] = r_chain(0)
            s0 = 0
            while sc1:
                for _ in range(6):
                    if sc1:
                        sc1.pop(0)()
                if s0 < SP:
                    pv_span(0, s0, *rbz[0])
                    s0 += 1
            while s0 < SP:
                pv_span(0, s0, *rbz[0])
                s0 += 1
            rbz[1] = r_chain(1)

            # ============ stage 1: scores(2) || PV(1) ============
            new_head_state(2)
            for v in range(NCH):
                scores_row(2, v)
                if v >= 4 and v % 2 == 0:
                    mirror_units(2, [v - 3, v - 2])
                    if v >= 8:
                        pv_span(1, (v - 8) // 2, *rbz[1])
            mirror_units(2, [NCH - 3, NCH - 2])
            mirror_units(2, [NCH - 1])
            rbz[2] = r_chain(2)

            # ===== stage 2: scores(3) || PV(2) + outproj-A (heads 0-1) =====
            new_head_state(3)
            for v in range(NCH):
                scores_row(3, v)
                if v >= 4 and v % 2 == 0:
                    mirror_units(3, [v - 3, v - 2])
                if v % 4 == 2:
                    pv_span(2, v // 4, *rbz[2])
            mirror_units(3, [NCH - 3, NCH - 2])
            mirror_units(3, [NCH - 1])
            rbz[3] = r_chain(3)

            # lull fill: heads-0/1 partials are independent of r_chain(3)
            for a in range(6):
                outproj_unit(a, 0)

            # == stage 3: PV(3) + outproj (B for a<6, combined for a>=6) ==
            psys = {0: pv_mm(3, 0), 1: pv_mm(3, 1)}
            for s in range(SP):
                pv_evac(3, s, psys[s], *rbz[3])
                if s + 2 < SP:
                    psys[s + 2] = pv_mm(3, s + 2)
                for a in range(4 * s, 4 * s + 4):
                    outproj_unit(a, 1 if a < 6 else 2)

    nc.compile()
    return nc


def _hi_lo(arr):
    hi = arr.astype(ml_dtypes.float8_e4m3)
    lo = (arr - hi.astype(np.float32)).astype(ml_dtypes.float8_e4m3)
    return hi, lo


def _pack_x(xt_hl):
    return np.ascontiguousarray(
        xt_hl.reshape(KP, 2, P, SP, 512).transpose(2, 3, 0, 1, 4)
    )


def kernel(x, Wq, Wo, bo):
    x = np.asarray(x, dtype=np.float32)
    Wq = np.asarray(Wq, dtype=np.float32)
    Wo = np.asarray(Wo, dtype=np.float32)
    bo = np.asarray(bo, dtype=np.float32)
    B = x.shape[0]
    assert B == 2 and x.shape == (B, N, D)
    assert Wq.shape == (D, D) and Wo.shape == (D, D)

    if "nc" not in _CACHE:
        _CACHE["nc"] = build_nc()
    nc = _CACHE["nc"]

    packed_x = []
    for b in range(B):
        hi, lo = _hi_lo(np.ascontiguousarray(x[b].T))
        packed_x.append((_pack_x(hi), _pack_x(lo)))
    in_maps = []
    for core in range(8):
        b, hg = core // 4, core % 4
        wq_s = WQS * Wq[:, hg * HCOLS:(hg + 1) * HCOLS]
        wqh, wql = _hi_lo(np.ascontiguousarray(wq_s))
        wo8h_a, wo8l_a = _hi_lo(
            np.ascontiguousarray(WQS * Wo[hg * HCOLS:(hg + 1) * HCOLS, :])
        )
        in_maps.append(
            {
                "xh": packed_x[b][0],
                "xl": packed_x[b][1],
                "wqh": np.ascontiguousarray(
                    wqh.reshape(KP, 2, P, HCOLS).transpose(2, 0, 1, 3)
                ),
                "wql": np.ascontiguousarray(
                    wql.reshape(KP, 2, P, HCOLS).transpose(2, 0, 1, 3)
                ),
                "wo8h": np.ascontiguousarray(
                    wo8h_a.reshape(HG, P, D).transpose(1, 0, 2)
                ),
                "wo8l": np.ascontiguousarray(
                    wo8l_a.reshape(HG, P, D).transpose(1, 0, 2)
                ),
            }
        )

    res = run_bass_kernel_spmd(nc, in_maps, list(range(8)))
    _CACHE["last_res"] = res
    out = np.zeros((B, N, D), dtype=np.float32)
    for core in range(8):
        b = core // 4
        out[b] += res.results[core]["out"].astype(np.float32)
        out[b][0:768] += res.results[core]["outA"][0:768].astype(np.float32)
    out += bo
    return out


# revision 5
# speedup vs baseline: 1.0610x; 1.0113x over previous
"""Trainium2 Bass kernel for nn_MultiHeadAttention_78864189489198 — v3.

fp8 paired-es variant.

E = exp(S - B8) is stored in fp8 (e4m3) chunk-PAIRED tiles es8[t] of
shape [P, 2, N] holding row-chunks (2t, 2t+1).  This lets both the PV
(attn @ V) and mirror matmuls run in DoubleRow perf mode (contraction
256 = two chunks at once, 0.5 cyc/col), cutting their PE cost ~4x/3.3x
vs the fp16 v2 path.

The huge diagonal E_nn (softmax is diagonally dominant here) cannot
live in fp8: the scores diagonal is masked to 0 in PSUM before exp
(DVE multiply by (1-I)), and the diagonal contribution is restored
exactly:  S_nn = rowsum(q16^2) via per-chunk ones-matmuls,
E_nn = exp(S_nn - 9), Y += (E_nn / r) * Q with r assembled from the
exp accumulators + mirror ones-column + E_nn (consistent numerator /
denominator, so S_nn quantization error cancels).

Everything else as v2: fp8 hi/lo DoubleRow qproj/outproj, host-side
batch x head-group sharding (8 cores), fp16 partial outputs summed on
host.
"""

import numpy as np
import ml_dtypes

import concourse.bass as bass
import concourse.mybir as mybir
import concourse.tile as tile
from concourse import bacc
from concourse.bass_utils import run_bass_kernel_spmd
from concourse.masks import make_identity
from contextlib import ExitStack

P = 128
N = 2048
D = 2048
KP = 8
HG = 4
HD = 128
HCOLS = HG * HD
SP = N // 512
NCH = N // P
NPAIR = NCH // 2
SCALE = HD ** -0.5
C_BIAS = 9.0
B8 = 3.5
WQS = 64.0

f32 = mybir.dt.float32
f32r = mybir.dt.float32r
f16 = mybir.dt.float16
e4 = mybir.dt.float8e4
DR = mybir.MatmulPerfMode.DoubleRow

_CACHE = {}

EXP = mybir.ActivationFunctionType.Exp
ADD = mybir.AluOpType.add
SUB = mybir.AluOpType.subtract
MUL = mybir.AluOpType.mult


def build_nc():
    nc = bacc.Bacc("TRN2", target_bir_lowering=False, debug=False)
    xh = nc.dram_tensor("xh", [P, SP, KP, 2, 512], e4, kind="ExternalInput")
    xl = nc.dram_tensor("xl", [P, SP, KP, 2, 512], e4, kind="ExternalInput")
    wqh = nc.dram_tensor("wqh", [P, KP, 2, HCOLS], e4, kind="ExternalInput")
    wql = nc.dram_tensor("wql", [P, KP, 2, HCOLS], e4, kind="ExternalInput")
    wo8h = nc.dram_tensor("wo8h", [P, HG, D], e4, kind="ExternalInput")
    wo8l = nc.dram_tensor("wo8l", [P, HG, D], e4, kind="ExternalInput")
    out = nc.dram_tensor("out", [N, D], f16, kind="ExternalOutput")
    out3 = out.rearrange("(a p) n -> p a n", p=P)
    outA = nc.dram_tensor("outA", [N, D], f16, kind="ExternalOutput")
    outA3 = outA.rearrange("(a p) n -> p a n", p=P)

    EB8 = float(np.exp(-B8))          # masked-diag residue per row
    EB9 = float(np.exp(B8 - C_BIAS))  # es8 scale -> e^-9 scale

    with (
        nc.allow_low_precision(reason="fp8/fp16 dataflow is intentional"),
        tile.TileContext(nc) as tc,
        ExitStack() as ctx,
    ):
        const_pool = ctx.enter_context(tc.tile_pool(name="const", bufs=1))
        qt8_pool = ctx.enter_context(tc.tile_pool(name="qt8", bufs=1))
        qn8_pool = ctx.enter_context(tc.tile_pool(name="qn8", bufs=1))
        q16_pool = ctx.enter_context(tc.tile_pool(name="q16f", bufs=4))
        es_pool = ctx.enter_context(tc.tile_pool(name="es8", bufs=16))
        rr_pool = ctx.enter_context(tc.tile_pool(name="rr", bufs=4))
        tmp_pool = ctx.enter_context(tc.tile_pool(name="tmp", bufs=3))
        snn_pool = ctx.enter_context(tc.tile_pool(name="snn", bufs=4))
        rbc_pool = ctx.enter_context(tc.tile_pool(name="rbc", bufs=4))
        yt_pool = ctx.enter_context(tc.tile_pool(name="yt", bufs=2))
        ynm_pool = ctx.enter_context(tc.tile_pool(name="ynm", bufs=4))
        q8st_pool = ctx.enter_context(tc.tile_pool(name="q8st", bufs=2))
        qn8st_pool = ctx.enter_context(tc.tile_pool(name="qn8st", bufs=2))
        ps_s = ctx.enter_context(tc.tile_pool(name="ps_s", bufs=2, space="PSUM"))
        ps_b = ctx.enter_context(tc.tile_pool(name="ps_b", bufs=4, space="PSUM"))

        idr = const_pool.tile([P, P], f32r, tag="idr")
        id16 = const_pool.tile([P, P], f16, tag="id16")
        # DR-layout fp8 identities: ia8p^T @ ia8n = -4096*I (diag canceller)
        ia8p = const_pool.tile([64, 2, P], e4, tag="ia8p")
        ia8n = const_pool.tile([64, 2, P], e4, tag="ia8n")
        nb8 = const_pool.tile([P, 1], f32, tag="nb8")
        nc.gpsimd.memset(nb8[:], -B8)
        nb9 = const_pool.tile([P, 1], f32, tag="nb9")
        nc.gpsimd.memset(nb9[:], -C_BIAS)
        ones1 = const_pool.tile([P, 1], f16, tag="ones1")
        nc.gpsimd.memset(ones1[:], 1.0)

        # qt8[p, r, c, n] : 4*Q^T head c, DR layout (hd = 2p+r-ish)
        qt8 = qt8_pool.tile([64, 2, HG, N], e4, tag="qt8")
        # qn8[p, t, k, c, :] = Q[(2t+k)*P+p, head c, :]/16
        # (128-multiple strides: fp8 Ldweights restriction)
        qn8 = qn8_pool.tile([P, NPAIR, 2, HG, P], e4, tag="qn8")
        # fp8 ones for the mirror r-column matmuls
        on8 = const_pool.tile([P, 2, 1], e4, tag="on8")
        nc.gpsimd.memset(on8[:], 1.0)
        # q16full[c] = 4*Q^T head c, f16, persistent (diag path + src of qt8/qn8)
        q16full = {}
        for c in range(HG):
            q16full[c] = q16_pool.tile([P, N], f16, tag="q16f", name=f"q16f{c}")

        yts = {}
        rrechs = {}
        rmirs = {}
        snns = {}
        ess = {}       # ess[c][t] = es8 pair tile [P, 2, N]

        def new_head_state(c):
            rrechs[c] = rr_pool.tile([P, NCH, 3], f32, tag="rrech", name=f"rr{c}")
            rmirs[c] = rr_pool.tile([P, NCH], f32, tag="rmir", name=f"rm{c}")
            nc.vector.memset(rrechs[c][:], 0.0)
            nc.vector.memset(rmirs[c][:], 0.0)
            yts[c] = yt_pool.tile([P, N], f16, tag="yt", name=f"yt{c}")
            nc.gpsimd.memset(yts[c][:, 0:P], 0.0)
            ess[c] = {}

        def scores_piece(c, v, h, es, k, lo=None, hi=None, slot=None):
            # scores cols [max(128v, 1024h), 1024(h+1)) for row-chunk v
            if lo is None:
                lo = max(P * v, 1024 * h)
            if hi is None:
                hi = 1024 * (h + 1)
            if slot is None:
                slot = h
            if lo >= hi:
                return
            ps = ps_s.tile([P, 1024], f32, tag="s")
            col = lo
            while col < hi:
                w = min(512 - col % 512, hi - col)
                nc.tensor.matmul(
                    ps[:, col - 1024 * h:col - 1024 * h + w],
                    qt8[:, :, c, v * P:(v + 1) * P],
                    qt8[:, :, c, col:col + w],
                    start=True,
                    stop=True,
                    perf_mode=DR,
                )
                col += w
            if lo == v * P:
                # diag block leads this piece: add -4096*I so exp of the
                # exact diagonal flushes to 0 in fp8 (residue ~1e-4, ignored)
                nc.tensor.matmul(
                    ps[:, lo - 1024 * h:lo - 1024 * h + P],
                    ia8p[:],
                    ia8n[:],
                    start=False,
                    stop=True,
                    perf_mode=DR,
                )
            nc.scalar.activation(
                es[:, k, lo:hi],
                ps[:, lo - 1024 * h:hi - 1024 * h],
                EXP,
                bias=nb8[:, 0:1],
                scale=SCALE / 16.0,
                accum_out=rrechs[c][:, v, slot:slot + 1],
            )

        def scores_row(c, v):
            t, k = v // 2, v % 2
            if k == 0:
                ess[c][t] = es_pool.tile([P, 2, N], e4, tag="es", name=f"e{c}_{t}")
            es = ess[c][t]
            scores_piece(c, v, 0, es, k)
            scores_piece(c, v, 1, es, k)

        def mirror_units(c, us):
            # Y[u-chunk, :] += sum_{v<u} E8[v, u-block]^T @ [Q_v/16 | 1]
            # us: 1-2 consecutive chunk indices sharing one PSUM tile
            nu = len(us)
            w = 128 * nu
            pm = ps_b.tile([P, w + nu], f32, tag="b", name=f"pm{c}_{us[0]}")
            for j, u in enumerate(us):
                npairs = u // 2
                tail = [(t, None) for t in range(npairs)]
                if u % 2 == 1:
                    tail.append((npairs, 0))
                for i, (t, k) in enumerate(tail):
                    st = (i == 0)
                    sp = (i == len(tail) - 1)
                    if k is None:
                        nc.tensor.matmul(
                            pm[:, j * P:(j + 1) * P],
                            ess[c][t][:, :, u * P:(u + 1) * P],
                            qn8[:, t, :, c, :],
                            start=st, stop=sp, perf_mode=DR,
                        )
                        nc.tensor.matmul(
                            pm[:, w + j:w + j + 1],
                            ess[c][t][:, :, u * P:(u + 1) * P],
                            on8[:],
                            start=st, stop=sp, perf_mode=DR,
                        )
                    else:
                        nc.tensor.matmul(
                            pm[:, j * P:(j + 1) * P],
                            ess[c][t][:, k, u * P:(u + 1) * P],
                            qn8[:, t, k, c, :],
                            start=st, stop=sp,
                        )
                        nc.tensor.matmul(
                            pm[:, w + j:w + j + 1],
                            ess[c][t][:, k, u * P:(u + 1) * P],
                            on8[:, 0],
                            start=st, stop=sp,
                        )
            ynm = ynm_pool.tile([P, w], f16, tag="ynm")
            nc.vector.tensor_copy(ynm[:], pm[:, 0:w])
            nc.vector.tensor_copy(rmirs[c][:, us[0]:us[0] + nu], pm[:, w:w + nu])
            ptm = ps_b.tile([P, w], f16, tag="b", name=f"ptm{c}_{us[0]}")
            for j in range(nu):
                nc.tensor.transpose(
                    ptm[:, j * P:(j + 1) * P], ynm[:, j * P:(j + 1) * P], id16[:]
                )
            nc.vector.tensor_copy(yts[c][:, us[0] * P:(us[0] + nu) * P], ptm[:])

        def snn_unit(c, t):
            # S_nn for chunks 4t..4t+3 of head c: colsums of (q16)^2 blocks
            qsq = tmp_pool.tile([P, 512], f16, tag="qsq")
            nc.vector.tensor_tensor(
                qsq[:], q16full[c][:, t * 512:(t + 1) * 512],
                q16full[c][:, t * 512:(t + 1) * 512], MUL,
            )
            pq = ps_b.tile([P, 4], f32, tag="b", name=f"pq{c}_{t}")
            for j in range(4):
                nc.tensor.matmul(
                    pq[:, j:j + 1],
                    qsq[:, j * P:(j + 1) * P],
                    ones1[:],
                    start=True, stop=True,
                )
            nc.vector.tensor_copy(snns[c][:, t * 4:(t + 1) * 4], pq[:])

        def r_chain(c):
            # r9 = (sum_h rrech + rmir - e^-B8) * e^(B8-9) + Enn9   [P, NCH]
            enn = tmp_pool.tile([P, NCH], f32, tag="enn")
            nc.scalar.activation(
                enn[:], snns[c][:], EXP, bias=nb9[:, 0:1], scale=SCALE / 16.0,
            )
            rr = tmp_pool.tile([P, NCH], f32, tag="rrec")
            nc.vector.tensor_reduce(
                rr[:], rrechs[c][:], mybir.AxisListType.X, ADD
            )
            nc.vector.tensor_tensor(rr[:], rr[:], rmirs[c][:], ADD)
            nc.vector.tensor_scalar_mul(rr[:], rr[:], EB9)
            nc.vector.tensor_tensor(rr[:], rr[:], enn[:], ADD)
            rcp = tmp_pool.tile([P, NCH], f32r, tag="rcp")
            nc.vector.reciprocal(rcp[:], rr[:])
            # broadcast vectors: rbc = 16*e^(B8-9)/r9 ; zbc = Enn9/(4*r9)
            # (evac: yt = (psy + yt_mirror)*rbc + q16*zbc ; zbc <= 0.25)
            offv = tmp_pool.tile([P, 2, NCH], f32r, tag="offv")
            nc.vector.tensor_scalar_mul(offv[:, 0], rcp[:], 16.0 * EB9)
            nc.vector.tensor_tensor(offv[:, 1], rcp[:], enn[:], MUL)
            nc.vector.tensor_scalar_mul(offv[:, 1], offv[:, 1], 0.25)
            pt = ps_b.tile([NCH, 2 * P], f32r, tag="b", name=f"rt{c}")
            nc.tensor.transpose(pt[:, 0:P], offv[:, 0], idr[:])
            nc.tensor.transpose(pt[:, P:2 * P], offv[:, 1], idr[:])
            both16 = tmp_pool.tile([NCH, 2 * P], f16, tag="both16")
            nc.vector.tensor_copy(both16[:], pt[:])
            rbc = rbc_pool.tile([P, N], f16, tag="rbc", name=f"rb{c}")
            zbc = rbc_pool.tile([P, N], f16, tag="zbc", name=f"zb{c}")
            nc.sync.dma_start(rbc[0:1, :], both16[:, 0:P])
            nc.sync.dma_start(zbc[0:1, :], both16[:, P:2 * P])
            nc.gpsimd.partition_broadcast(rbc[:], rbc[0:1, :])
            nc.gpsimd.partition_broadcast(zbc[:], zbc[0:1, :])
            return rbc, zbc

        with (
            tc.tile_pool(name="xt", bufs=4) as xt_pool,
            tc.tile_pool(name="wq", bufs=1) as wq_pool,
        ):
            make_identity(nc, id16[:])
            nc.vector.tensor_copy(idr[:], id16[:])
            i8st = q8st_pool.tile([P, P], e4, tag="q8s", name="i8p")
            nc.vector.tensor_scalar_mul(i8st[:], id16[:], 64.0)
            nc.sync.dma_start(ia8p[:], i8st[:])
            i8st2 = q8st_pool.tile([P, P], e4, tag="q8s", name="i8n")
            nc.vector.tensor_scalar_mul(i8st2[:], id16[:], -64.0)
            nc.sync.dma_start(ia8n[:], i8st2[:])

            for c in range(HG):
                snns[c] = snn_pool.tile([P, NCH], f32, tag="snn", name=f"sn{c}")

            wqh_sb = wq_pool.tile([P, KP, 2, HCOLS], e4, tag="wqh")
            wql_sb = wq_pool.tile([P, KP, 2, HCOLS], e4, tag="wql")
            xsp = {}

            def load_x(t, engs):
                xsp[t] = (
                    xt_pool.tile([P, KP, 2, 512], e4, tag="xt", name=f"xh{t}"),
                    xt_pool.tile([P, KP, 2, 512], e4, tag="xt", name=f"xl{t}"),
                )
                engs[0].dma_start(xsp[t][0][:], xh[:, t])
                engs[1].dma_start(xsp[t][1][:], xl[:, t])

            # prologue loads: fine-grained halves so qproj starts early
            xsp[0] = (
                xt_pool.tile([P, KP, 2, 512], e4, tag="xt", name="xh0"),
                xt_pool.tile([P, KP, 2, 512], e4, tag="xt", name="xl0"),
            )
            nc.sync.dma_start(wqh_sb[:, 0:2], wqh[:, 0:2])
            nc.scalar.dma_start(xsp[0][0][:, 0:2], xh[:, 0, 0:2])
            nc.gpsimd.dma_start(xsp[0][1][:, 0:2], xl[:, 0, 0:2])
            nc.sync.dma_start(wqh_sb[:, 2:5], wqh[:, 2:5])
            nc.scalar.dma_start(xsp[0][0][:, 2:5], xh[:, 0, 2:5])
            nc.gpsimd.dma_start(xsp[0][1][:, 2:5], xl[:, 0, 2:5])
            nc.sync.dma_start(wqh_sb[:, 5:8], wqh[:, 5:8])
            nc.scalar.dma_start(xsp[0][0][:, 5:8], xh[:, 0, 5:8])
            nc.gpsimd.dma_start(xsp[0][1][:, 5:8], xl[:, 0, 5:8])
            nc.sync.dma_start(wql_sb[:, 0:4], wql[:, 0:4])
            nc.scalar.dma_start(wql_sb[:, 4:8], wql[:, 4:8])
            load_x(1, (nc.scalar, nc.gpsimd))
            edum = rr_pool.tile([P, 1], f32, tag="edum")
            nc.scalar.activation(edum[:], nb8[:, 0:1], EXP)

            pend_tr = []
            late_tr = []

            def qproj_unit(c, t):
                ps = ps_b.tile([P, 512], f32, tag="b")
                th, tl = xsp[t]
                combos = [(wqh_sb, th, kp) for kp in range(KP)]
                combos += [(wql_sb, th, kp) for kp in range(KP)]
                combos += [(wqh_sb, tl, kp) for kp in range(KP)]
                for i, (w, x, kp) in enumerate(combos):
                    nc.tensor.matmul(
                        ps[:],
                        w[:, kp, :, c * P:(c + 1) * P],
                        x[:, kp],
                        start=(i == 0),
                        stop=(i == len(combos) - 1),
                        perf_mode=DR,
                    )
                # q16full stores 4*Q^T (f16)
                nc.vector.tensor_scalar_mul(
                    q16full[c][:, t * 512:(t + 1) * 512], ps[:], 1.0 / 16.0
                )
                # qt8 = e4m3(4*Q^T), staged then DR-repacked via DMA
                q8s = q8st_pool.tile([P, 512], e4, tag="q8s")
                nc.gpsimd.tensor_copy(q8s[:], q16full[c][:, t * 512:(t + 1) * 512])
                nc.sync.dma_start(qt8[:, :, c, t * 512:(t + 1) * 512], q8s[:])
                if pend_tr:
                    pend_tr.pop(0)()

                def transposes(c=c, t=t):
                    # Q natural fp8 (Q/16) into paired DR layout
                    pt_all = ps_b.tile([P, 512], f16, tag="b")
                    for j in range(4):
                        nc.tensor.transpose(
                            pt_all[:, j * P:(j + 1) * P],
                            q16full[c][:, (t * 4 + j) * P:(t * 4 + j + 1) * P],
                            id16[:],
                        )
                    qn8s = qn8st_pool.tile([P, 512], e4, tag="qn8s")
                    nc.vector.tensor_scalar_mul(qn8s[:], pt_all[:], 1.0 / 64.0)
                    nc.sync.dma_start(
                        qn8[:, 2 * t:2 * t + 2, :, c, :], qn8s[:]
                    )
                    snn_unit(c, t)

                if c < 2:
                    pend_tr.append(transposes)
                else:
                    late_tr.append(transposes)

            # Boot interleave: stream head-0 and head-1 score pieces (feeding
            # the ACT exp pipeline) between qproj units so ACT never starves.
            sc = []
            sc1 = []

            def emit(n):
                for _ in range(n):
                    if sc:
                        sc.pop(0)()
                    elif sc1:
                        sc1.pop(0)()

            def ph_piece(c, v, h, **kw):
                t, k = v // 2, v % 2
                if k == 0 and t not in ess[c]:
                    ess[c][t] = es_pool.tile(
                        [P, 2, N], e4, tag="es", name=f"e{c}_{t}"
                    )
                scores_piece(c, v, h, ess[c][t], k, **kw)

            qproj_unit(0, 0)
            new_head_state(0)
            # rows 0-3 cols [vP,512): only needs the (0,0) qt8 fold -> ACT
            # exp stream starts while qproj is still warming up
            for v in range(4):
                sc.append(lambda v=v: ph_piece(0, v, 0, hi=512, slot=2))
            qproj_unit(1, 0)
            emit(1)
            qproj_unit(2, 0)
            emit(1)
            qproj_unit(3, 0)
            emit(2)
            load_x(2, (nc.sync, nc.sync))
            qproj_unit(0, 1)
            for v in range(8):
                sc.append(lambda v=v: ph_piece(0, v, 0, lo=max(v * P, 512)))
            qproj_unit(1, 1)
            new_head_state(1)
            for v in range(8):
                sc.append(lambda v=v: ph_piece(1, v, 0))
            emit(2)
            qproj_unit(2, 1)
            emit(2)
            qproj_unit(3, 1)
            emit(2)
            load_x(3, (nc.sync, nc.sync))
            qproj_unit(0, 2)
            emit(2)
            qproj_unit(1, 2)
            emit(2)
            qproj_unit(0, 3)
            # head-0 phase 2 (qt8[0] complete)
            for v in range(NCH):
                sc.append(lambda v=v: ph_piece(0, v, 1))
                if v >= 4 and v % 2 == 0:
                    sc.append(lambda v=v: mirror_units(0, [v - 3, v - 2]))
            sc.append(lambda: mirror_units(0, [NCH - 3, NCH - 2]))
            sc.append(lambda: mirror_units(0, [NCH - 1]))
            qproj_unit(1, 3)
            # head 1 (qt8[1] complete)
            for v in range(NCH):
                if v >= 8:
                    sc1.append(lambda v=v: ph_piece(1, v, 0))
                sc1.append(lambda v=v: ph_piece(1, v, 1))
                if v >= 4 and v % 2 == 0:
                    sc1.append(lambda v=v: mirror_units(1, [v - 3, v - 2]))
            sc1.append(lambda: mirror_units(1, [NCH - 3, NCH - 2]))
            sc1.append(lambda: mirror_units(1, [NCH - 1]))
            qproj_unit(2, 2)
            emit(4)
            qproj_unit(2, 3)
            emit(4)
            qproj_unit(3, 2)
            emit(4)
            qproj_unit(3, 3)
            while pend_tr:
                pend_tr.pop(0)()

        with (
            tc.tile_pool(name="wo8", bufs=1) as wo8_pool,
            tc.tile_pool(name="y8", bufs=1) as y8p,
            tc.tile_pool(name="osb", bufs=3) as o_pool,
        ):
            wo8h_sb = wo8_pool.tile([P, HG, D], e4, tag="wo8h")
            wo8l_sb = wo8_pool.tile([P, HG, D], e4, tag="wo8l")
            nc.sync.dma_start(wo8h_sb[:], wo8h[:])
            nc.gpsimd.dma_start(wo8l_sb[:], wo8l[:])
            y8h = y8p.tile([P, HG, N], e4, tag="y8h")
            y8l = y8p.tile([P, HG, N], e4, tag="y8l")
            rbz = {}

            def outproj_unit(a, grp):
                # grp 0: heads 0-1 partial -> outA ; grp 1: heads 2-3 -> out
                # grp 2: all heads (combined) -> out
                dst = outA3 if grp == 0 else out3
                ot = o_pool.tile([P, D], f16, tag="ot")
                for d4 in range(SP):
                    pool = ps_s if d4 % 2 else ps_b
                    ps = pool.tile([P, 512], f32, tag="s" if d4 % 2 else "b")
                    combos = []
                    for cc in ((0,), (2,), (0, 2))[grp]:
                        combos += [
                            (y8h, wo8h_sb, cc),
                            (y8h, wo8l_sb, cc),
                            (y8l, wo8h_sb, cc),
                        ]
                    for i, (yy, ww, cc) in enumerate(combos):
                        nc.tensor.matmul(
                            ps[:],
                            yy[:, cc:cc + 2, a * P:(a + 1) * P],
                            ww[:, cc:cc + 2, d4 * 512:(d4 + 1) * 512],
                            start=(i == 0),
                            stop=(i == len(combos) - 1),
                            perf_mode=DR,
                        )
                    if d4 % 2 == 0:
                        nc.vector.tensor_scalar_mul(
                            ot[:, d4 * 512:(d4 + 1) * 512], ps[:], 1.0 / 64.0
                        )
                    else:
                        nc.scalar.mul(
                            ot[:, d4 * 512:(d4 + 1) * 512], ps[:], 1.0 / 64.0
                        )
                        # stream each 1KB half out as soon as it's ready
                        half = d4 // 2
                        eng = nc.sync
                        eng.dma_start(
                            dst[:, a, half * 1024:(half + 1) * 1024],
                            ot[:, half * 1024:(half + 1) * 1024],
                        )

            def pv_mm(pc, s):
                # one span of YT via paired DR matmuls
                psy = ps_b.tile([P, 512], f32, tag="b", name=f"psy{pc}_{s}")
                lo_s = s * 512
                mms = []
                for t in range(2 * s):
                    mms.append((t, None, lo_s))
                mms.append((2 * s, 0, lo_s))                # solo chunk 4s
                mms.append((2 * s, None, lo_s + 128))       # pair (4s,4s+1)
                mms.append((2 * s + 1, 0, lo_s + 256))      # solo chunk 4s+2
                mms.append((2 * s + 1, None, lo_s + 384))   # pair (4s+2,4s+3)
                for i, (t, k, lo) in enumerate(mms):
                    st = (i == 0)
                    sp = (i == len(mms) - 1)
                    hi = (s + 1) * 512
                    if k is None:
                        nc.tensor.matmul(
                            psy[:, lo - lo_s:hi - lo_s],
                            qn8[:, t, :, pc, 0:P],
                            ess[pc][t][:, :, lo:hi],
                            start=st, stop=sp, perf_mode=DR,
                        )
                    else:
                        nc.tensor.matmul(
                            psy[:, lo - lo_s:lo - lo_s + 128],
                            qn8[:, t, k, pc, 0:P],
                            ess[pc][t][:, k, lo:lo + 128],
                            start=st, stop=sp,
                        )
                return psy

            def pv_evac(pc, s, psy, rbc, zbc):
                # yt = ((psy + yt_mirror) + q16*zrbc) * rbc  -> y8 hi/lo
                yt = yts[pc]
                sl = slice(s * 512, (s + 1) * 512)
                nc.vector.tensor_tensor(yt[:, sl], psy[:], yt[:, sl], ADD)
                nc.vector.tensor_tensor(yt[:, sl], yt[:, sl], rbc[:, sl], MUL)
                dg = tmp_pool.tile([P, 512], f16, tag="dg", name=f"dg{pc}_{s}")
                nc.vector.tensor_tensor(
                    dg[:], q16full[pc][:, sl], zbc[:, sl], MUL
                )
                nc.vector.tensor_tensor(yt[:, sl], yt[:, sl], dg[:], ADD)
                nc.gpsimd.tensor_copy(y8h[:, pc, sl], yt[:, sl])
                nc.gpsimd.tensor_tensor(
                    y8l[:, pc, sl], yt[:, sl], y8h[:, pc, sl], SUB
                )

            def pv_span(pc, s, rbc, zbc):
                pv_evac(pc, s, pv_mm(pc, s), rbc, zbc)

            # boot tail: drain head-0 stream, then overlap the head-1 exp
            # tail with PV(0) spans and the r-chains
            while sc:
                sc.pop(0)()
            rbz[0] = r_chain(0)
            s0 = 0
            while sc1:
                for _ in range(6):
                    if sc1:
                        sc1.pop(0)()
                if s0 < SP:
                    pv_span(0, s0, *rbz[0])
                    s0 += 1
            while s0 < SP:
                pv_span(0, s0, *rbz[0])
                s0 += 1
            rbz[1] = r_chain(1)

            # ============ stage 1: scores(2) || PV(1) ============
            # deferred qn8/S_nn prep for heads 2-3 (PE idle under exp here)
            while late_tr:
                late_tr.pop(0)()
            new_head_state(2)
            for v in range(NCH):
                scores_row(2, v)
                if v >= 4 and v % 2 == 0:
                    mirror_units(2, [v - 3, v - 2])
                    if v >= 8:
                        pv_span(1, (v - 8) // 2, *rbz[1])
            mirror_units(2, [NCH - 3, NCH - 2])
            mirror_units(2, [NCH - 1])
            rbz[2] = r_chain(2)

            # ===== stage 2: scores(3) || PV(2) + outproj-A (heads 0-1) =====
            new_head_state(3)
            for v in range(NCH):
                scores_row(3, v)
                if v >= 4 and v % 2 == 0:
                    mirror_units(3, [v - 3, v - 2])
                if v % 4 == 2:
                    pv_span(2, v // 4, *rbz[2])
            mirror_units(3, [NCH - 3, NCH - 2])
            mirror_units(3, [NCH - 1])
            rbz[3] = r_chain(3)

            # lull fill: heads-0/1 partials are independent of r_chain(3)
            for a in range(6):
                outproj_unit(a, 0)

            # == stage 3: PV(3) + outproj (B for a<6, combined for a>=6) ==
            psys = {0: pv_mm(3, 0), 1: pv_mm(3, 1)}
            for s in range(SP):
                pv_evac(3, s, psys[s], *rbz[3])
                if s + 2 < SP:
                    psys[s + 2] = pv_mm(3, s + 2)
                for a in range(4 * s, 4 * s + 4):
                    outproj_unit(a, 1 if a < 6 else 2)

    nc.compile()
    return nc


def _hi_lo(arr):
    hi = arr.astype(ml_dtypes.float8_e4m3)
    lo = (arr - hi.astype(np.float32)).astype(ml_dtypes.float8_e4m3)
    return hi, lo


def _pack_x(xt_hl):
    return np.ascontiguousarray(
        xt_hl.reshape(KP, 2, P, SP, 512).transpose(2, 3, 0, 1, 4)
    )


def kernel(x, Wq, Wo, bo):
    x = np.asarray(x, dtype=np.float32)
    Wq = np.asarray(Wq, dtype=np.float32)
    Wo = np.asarray(Wo, dtype=np.float32)
    bo = np.asarray(bo, dtype=np.float32)
    B = x.shape[0]
    assert B == 2 and x.shape == (B, N, D)
    assert Wq.shape == (D, D) and Wo.shape == (D, D)

    if "nc" not in _CACHE:
        _CACHE["nc"] = build_nc()
    nc = _CACHE["nc"]

    packed_x = []
    for b in range(B):
        hi, lo = _hi_lo(np.ascontiguousarray(x[b].T))
        packed_x.append((_pack_x(hi), _pack_x(lo)))
    in_maps = []
    for core in range(8):
        b, hg = core // 4, core % 4
        wq_s = WQS * Wq[:, hg * HCOLS:(hg + 1) * HCOLS]
        wqh, wql = _hi_lo(np.ascontiguousarray(wq_s))
        wo8h_a, wo8l_a = _hi_lo(
            np.ascontiguousarray(WQS * Wo[hg * HCOLS:(hg + 1) * HCOLS, :])
        )
        in_maps.append(
            {
                "xh": packed_x[b][0],
                "xl": packed_x[b][1],
                "wqh": np.ascontiguousarray(
                    wqh.reshape(KP, 2, P, HCOLS).transpose(2, 0, 1, 3)
                ),
                "wql": np.ascontiguousarray(
                    wql.reshape(KP, 2, P, HCOLS).transpose(2, 0, 1, 3)
                ),
                "wo8h": np.ascontiguousarray(
                    wo8h_a.reshape(HG, P, D).transpose(1, 0, 2)
                ),
                "wo8l": np.ascontiguousarray(
                    wo8l_a.reshape(HG, P, D).transpose(1, 0, 2)
                ),
            }
        )

    res = run_bass_kernel_spmd(nc, in_maps, list(range(8)))
    _CACHE["last_res"] = res
    out = np.zeros((B, N, D), dtype=np.float32)
    for core in range(8):
        b = core // 4
        out[b] += res.results[core]["out"].astype(np.float32)
        out[b][0:768] += res.results[core]["outA"][0:768].astype(np.float32)
    out += bo
    return out


# revision 6
# speedup vs baseline: 1.0691x; 1.0076x over previous
"""Trainium2 Bass kernel for nn_MultiHeadAttention_78864189489198 — v3.

fp8 paired-es variant.

E = exp(S - B8) is stored in fp8 (e4m3) chunk-PAIRED tiles es8[t] of
shape [P, 2, N] holding row-chunks (2t, 2t+1).  This lets both the PV
(attn @ V) and mirror matmuls run in DoubleRow perf mode (contraction
256 = two chunks at once, 0.5 cyc/col), cutting their PE cost ~4x/3.3x
vs the fp16 v2 path.

The huge diagonal E_nn (softmax is diagonally dominant here) cannot
live in fp8: the scores diagonal is masked to 0 in PSUM before exp
(DVE multiply by (1-I)), and the diagonal contribution is restored
exactly:  S_nn = rowsum(q16^2) via per-chunk ones-matmuls,
E_nn = exp(S_nn - 9), Y += (E_nn / r) * Q with r assembled from the
exp accumulators + mirror ones-column + E_nn (consistent numerator /
denominator, so S_nn quantization error cancels).

Everything else as v2: fp8 hi/lo DoubleRow qproj/outproj, host-side
batch x head-group sharding (8 cores), fp16 partial outputs summed on
host.
"""

import numpy as np
import ml_dtypes

import concourse.bass as bass
import concourse.mybir as mybir
import concourse.tile as tile
from concourse import bacc
from concourse.bass_utils import run_bass_kernel_spmd
from concourse.masks import make_identity
from contextlib import ExitStack

P = 128
N = 2048
D = 2048
KP = 8
HG = 4
HD = 128
HCOLS = HG * HD
SP = N // 512
NCH = N // P
NPAIR = NCH // 2
SCALE = HD ** -0.5
C_BIAS = 9.0
B8 = 3.5
WQS = 64.0

f32 = mybir.dt.float32
f32r = mybir.dt.float32r
f16 = mybir.dt.float16
e4 = mybir.dt.float8e4
DR = mybir.MatmulPerfMode.DoubleRow

_CACHE = {}

EXP = mybir.ActivationFunctionType.Exp
ADD = mybir.AluOpType.add
SUB = mybir.AluOpType.subtract
MUL = mybir.AluOpType.mult


def build_nc():
    nc = bacc.Bacc("TRN2", target_bir_lowering=False, debug=False)
    xh = nc.dram_tensor("xh", [P, SP, KP, 2, 512], e4, kind="ExternalInput")
    xl = nc.dram_tensor("xl", [P, SP, KP, 2, 512], e4, kind="ExternalInput")
    wqh = nc.dram_tensor("wqh", [P, KP, 2, HCOLS], e4, kind="ExternalInput")
    wql = nc.dram_tensor("wql", [P, KP, 2, HCOLS], e4, kind="ExternalInput")
    wo8h = nc.dram_tensor("wo8h", [P, HG, D], e4, kind="ExternalInput")
    wo8l = nc.dram_tensor("wo8l", [P, HG, D], e4, kind="ExternalInput")
    out = nc.dram_tensor("out", [N, D], f16, kind="ExternalOutput")
    out3 = out.rearrange("(a p) n -> p a n", p=P)
    outA = nc.dram_tensor("outA", [N, D], f16, kind="ExternalOutput")
    outA3 = outA.rearrange("(a p) n -> p a n", p=P)

    EB8 = float(np.exp(-B8))          # masked-diag residue per row
    EB9 = float(np.exp(B8 - C_BIAS))  # es8 scale -> e^-9 scale

    with (
        nc.allow_low_precision(reason="fp8/fp16 dataflow is intentional"),
        tile.TileContext(nc) as tc,
        ExitStack() as ctx,
    ):
        const_pool = ctx.enter_context(tc.tile_pool(name="const", bufs=1))
        qt8_pool = ctx.enter_context(tc.tile_pool(name="qt8", bufs=1))
        qn8_pool = ctx.enter_context(tc.tile_pool(name="qn8", bufs=1))
        q16_pool = ctx.enter_context(tc.tile_pool(name="q16f", bufs=4))
        es_pool = ctx.enter_context(tc.tile_pool(name="es8", bufs=16))
        rr_pool = ctx.enter_context(tc.tile_pool(name="rr", bufs=4))
        tmp_pool = ctx.enter_context(tc.tile_pool(name="tmp", bufs=3))
        snn_pool = ctx.enter_context(tc.tile_pool(name="snn", bufs=4))
        rbc_pool = ctx.enter_context(tc.tile_pool(name="rbc", bufs=4))
        yt_pool = ctx.enter_context(tc.tile_pool(name="yt", bufs=2))
        ynm_pool = ctx.enter_context(tc.tile_pool(name="ynm", bufs=4))
        q8st_pool = ctx.enter_context(tc.tile_pool(name="q8st", bufs=2))
        qn8st_pool = ctx.enter_context(tc.tile_pool(name="qn8st", bufs=2))
        ps_s = ctx.enter_context(tc.tile_pool(name="ps_s", bufs=2, space="PSUM"))
        ps_b = ctx.enter_context(tc.tile_pool(name="ps_b", bufs=4, space="PSUM"))

        idr = const_pool.tile([P, P], f32r, tag="idr")
        id16 = const_pool.tile([P, P], f16, tag="id16")
        # DR-layout fp8 identities: ia8p^T @ ia8n = -4096*I (diag canceller)
        ia8p = const_pool.tile([64, 2, P], e4, tag="ia8p")
        ia8n = const_pool.tile([64, 2, P], e4, tag="ia8n")
        nb8 = const_pool.tile([P, 1], f32, tag="nb8")
        nc.gpsimd.memset(nb8[:], -B8)
        nb9 = const_pool.tile([P, 1], f32, tag="nb9")
        nc.gpsimd.memset(nb9[:], -C_BIAS)
        ones1 = const_pool.tile([P, 1], f16, tag="ones1")
        nc.gpsimd.memset(ones1[:], 1.0)

        # qt8[p, r, c, n] : 4*Q^T head c, DR layout (hd = 2p+r-ish)
        qt8 = qt8_pool.tile([64, 2, HG, N], e4, tag="qt8")
        # qn8[p, t, k, c, :] = Q[(2t+k)*P+p, head c, :]/16
        # (128-multiple strides: fp8 Ldweights restriction)
        qn8 = qn8_pool.tile([P, NPAIR, 2, HG, P], e4, tag="qn8")
        # fp8 ones for the mirror r-column matmuls
        on8 = const_pool.tile([P, 2, 1], e4, tag="on8")
        nc.gpsimd.memset(on8[:], 1.0)
        # q16full[c] = 4*Q^T head c, f16, persistent (diag path + src of qt8/qn8)
        q16full = {}
        for c in range(HG):
            q16full[c] = q16_pool.tile([P, N], f16, tag="q16f", name=f"q16f{c}")

        yts = {}
        rrechs = {}
        rmirs = {}
        snns = {}
        ess = {}       # ess[c][t] = es8 pair tile [P, 2, N]

        def new_head_state(c):
            rrechs[c] = rr_pool.tile([P, NCH, 3], f32, tag="rrech", name=f"rr{c}")
            rmirs[c] = rr_pool.tile([P, NCH], f32, tag="rmir", name=f"rm{c}")
            nc.vector.memset(rrechs[c][:], 0.0)
            nc.vector.memset(rmirs[c][:], 0.0)
            yts[c] = yt_pool.tile([P, N], f16, tag="yt", name=f"yt{c}")
            nc.gpsimd.memset(yts[c][:, 0:P], 0.0)
            ess[c] = {}

        def scores_piece(c, v, h, es, k, lo=None, hi=None, slot=None):
            # scores cols [max(128v, 1024h), 1024(h+1)) for row-chunk v
            if lo is None:
                lo = max(P * v, 1024 * h)
            if hi is None:
                hi = 1024 * (h + 1)
            if slot is None:
                slot = h
            if lo >= hi:
                return
            ps = ps_s.tile([P, 1024], f32, tag="s")
            col = lo
            while col < hi:
                w = min(512 - col % 512, hi - col)
                nc.tensor.matmul(
                    ps[:, col - 1024 * h:col - 1024 * h + w],
                    qt8[:, :, c, v * P:(v + 1) * P],
                    qt8[:, :, c, col:col + w],
                    start=True,
                    stop=True,
                    perf_mode=DR,
                )
                col += w
            if lo == v * P:
                # diag block leads this piece: add -4096*I so exp of the
                # exact diagonal flushes to 0 in fp8 (residue ~1e-4, ignored)
                nc.tensor.matmul(
                    ps[:, lo - 1024 * h:lo - 1024 * h + P],
                    ia8p[:],
                    ia8n[:],
                    start=False,
                    stop=True,
                    perf_mode=DR,
                )
            nc.scalar.activation(
                es[:, k, lo:hi],
                ps[:, lo - 1024 * h:hi - 1024 * h],
                EXP,
                bias=nb8[:, 0:1],
                scale=SCALE / 16.0,
                accum_out=rrechs[c][:, v, slot:slot + 1],
            )

        def scores_row(c, v):
            t, k = v // 2, v % 2
            if k == 0:
                ess[c][t] = es_pool.tile([P, 2, N], e4, tag="es", name=f"e{c}_{t}")
            es = ess[c][t]
            scores_piece(c, v, 0, es, k)
            scores_piece(c, v, 1, es, k)

        def mirror_units(c, us):
            # Y[u-chunk, :] += sum_{v<u} E8[v, u-block]^T @ [Q_v/16 | 1]
            # us: 1-2 consecutive chunk indices sharing one PSUM tile
            nu = len(us)
            w = 128 * nu
            pm = ps_b.tile([P, w + nu], f32, tag="b", name=f"pm{c}_{us[0]}")
            for j, u in enumerate(us):
                npairs = u // 2
                tail = [(t, None) for t in range(npairs)]
                if u % 2 == 1:
                    tail.append((npairs, 0))
                for i, (t, k) in enumerate(tail):
                    st = (i == 0)
                    sp = (i == len(tail) - 1)
                    if k is None:
                        nc.tensor.matmul(
                            pm[:, j * P:(j + 1) * P],
                            ess[c][t][:, :, u * P:(u + 1) * P],
                            qn8[:, t, :, c, :],
                            start=st, stop=sp, perf_mode=DR,
                        )
                        nc.tensor.matmul(
                            pm[:, w + j:w + j + 1],
                            ess[c][t][:, :, u * P:(u + 1) * P],
                            on8[:],
                            start=st, stop=sp, perf_mode=DR,
                        )
                    else:
                        nc.tensor.matmul(
                            pm[:, j * P:(j + 1) * P],
                            ess[c][t][:, k, u * P:(u + 1) * P],
                            qn8[:, t, k, c, :],
                            start=st, stop=sp,
                        )
                        nc.tensor.matmul(
                            pm[:, w + j:w + j + 1],
                            ess[c][t][:, k, u * P:(u + 1) * P],
                            on8[:, 0],
                            start=st, stop=sp,
                        )
            ynm = ynm_pool.tile([P, w], f16, tag="ynm")
            nc.vector.tensor_copy(ynm[:], pm[:, 0:w])
            nc.vector.tensor_copy(rmirs[c][:, us[0]:us[0] + nu], pm[:, w:w + nu])
            ptm = ps_b.tile([P, w], f16, tag="b", name=f"ptm{c}_{us[0]}")
            for j in range(nu):
                nc.tensor.transpose(
                    ptm[:, j * P:(j + 1) * P], ynm[:, j * P:(j + 1) * P], id16[:]
                )
            nc.vector.tensor_copy(yts[c][:, us[0] * P:(us[0] + nu) * P], ptm[:])

        def snn_unit(c, t):
            # S_nn for chunks 4t..4t+3 of head c: colsums of (q16)^2 blocks
            qsq = tmp_pool.tile([P, 512], f16, tag="qsq")
            nc.vector.tensor_tensor(
                qsq[:], q16full[c][:, t * 512:(t + 1) * 512],
                q16full[c][:, t * 512:(t + 1) * 512], MUL,
            )
            pq = ps_b.tile([P, 4], f32, tag="b", name=f"pq{c}_{t}")
            for j in range(4):
                nc.tensor.matmul(
                    pq[:, j:j + 1],
                    qsq[:, j * P:(j + 1) * P],
                    ones1[:],
                    start=True, stop=True,
                )
            nc.vector.tensor_copy(snns[c][:, t * 4:(t + 1) * 4], pq[:])

        def r_chain(c):
            # r9 = (sum_h rrech + rmir - e^-B8) * e^(B8-9) + Enn9   [P, NCH]
            enn = tmp_pool.tile([P, NCH], f32, tag="enn")
            nc.scalar.activation(
                enn[:], snns[c][:], EXP, bias=nb9[:, 0:1], scale=SCALE / 16.0,
            )
            rr = tmp_pool.tile([P, NCH], f32, tag="rrec")
            nc.vector.tensor_reduce(
                rr[:], rrechs[c][:], mybir.AxisListType.X, ADD
            )
            nc.vector.tensor_tensor(rr[:], rr[:], rmirs[c][:], ADD)
            nc.vector.tensor_scalar_mul(rr[:], rr[:], EB9)
            nc.vector.tensor_tensor(rr[:], rr[:], enn[:], ADD)
            rcp = tmp_pool.tile([P, NCH], f32r, tag="rcp")
            nc.vector.reciprocal(rcp[:], rr[:])
            # broadcast vectors: rbc = 16*e^(B8-9)/r9 ; zbc = Enn9/(4*r9)
            # (evac: yt = (psy + yt_mirror)*rbc + q16*zbc ; zbc <= 0.25)
            offv = tmp_pool.tile([P, 2, NCH], f32r, tag="offv")
            nc.vector.tensor_scalar_mul(offv[:, 0], rcp[:], 16.0 * EB9)
            nc.vector.tensor_tensor(offv[:, 1], rcp[:], enn[:], MUL)
            nc.vector.tensor_scalar_mul(offv[:, 1], offv[:, 1], 0.25)
            pt = ps_b.tile([NCH, 2 * P], f32r, tag="b", name=f"rt{c}")
            nc.tensor.transpose(pt[:, 0:P], offv[:, 0], idr[:])
            nc.tensor.transpose(pt[:, P:2 * P], offv[:, 1], idr[:])
            both16 = tmp_pool.tile([NCH, 2 * P], f16, tag="both16")
            nc.vector.tensor_copy(both16[:], pt[:])
            rbc = rbc_pool.tile([P, N], f16, tag="rbc", name=f"rb{c}")
            zbc = rbc_pool.tile([P, N], f16, tag="zbc", name=f"zb{c}")
            nc.sync.dma_start(rbc[0:1, :], both16[:, 0:P])
            nc.sync.dma_start(zbc[0:1, :], both16[:, P:2 * P])
            nc.gpsimd.partition_broadcast(rbc[:], rbc[0:1, :])
            nc.gpsimd.partition_broadcast(zbc[:], zbc[0:1, :])
            return rbc, zbc

        with (
            tc.tile_pool(name="xt", bufs=4) as xt_pool,
            tc.tile_pool(name="wq", bufs=1) as wq_pool,
        ):
            make_identity(nc, id16[:])
            nc.vector.tensor_copy(idr[:], id16[:])
            i8st = q8st_pool.tile([P, P], e4, tag="q8s", name="i8p")
            nc.vector.tensor_scalar_mul(i8st[:], id16[:], 64.0)
            nc.sync.dma_start(ia8p[:], i8st[:])
            i8st2 = q8st_pool.tile([P, P], e4, tag="q8s", name="i8n")
            nc.vector.tensor_scalar_mul(i8st2[:], id16[:], -64.0)
            nc.sync.dma_start(ia8n[:], i8st2[:])

            for c in range(HG):
                snns[c] = snn_pool.tile([P, NCH], f32, tag="snn", name=f"sn{c}")

            wqh_sb = wq_pool.tile([P, KP, 2, HCOLS], e4, tag="wqh")
            wql_sb = wq_pool.tile([P, KP, 2, HCOLS], e4, tag="wql")
            xsp = {}

            def load_x(t, engs):
                xsp[t] = (
                    xt_pool.tile([P, KP, 2, 512], e4, tag="xt", name=f"xh{t}"),
                    xt_pool.tile([P, KP, 2, 512], e4, tag="xt", name=f"xl{t}"),
                )
                engs[0].dma_start(xsp[t][0][:], xh[:, t])
                engs[1].dma_start(xsp[t][1][:], xl[:, t])

            # prologue loads: fine-grained halves so qproj starts early
            xsp[0] = (
                xt_pool.tile([P, KP, 2, 512], e4, tag="xt", name="xh0"),
                xt_pool.tile([P, KP, 2, 512], e4, tag="xt", name="xl0"),
            )
            nc.sync.dma_start(wqh_sb[:, 0:2], wqh[:, 0:2])
            nc.scalar.dma_start(xsp[0][0][:, 0:2], xh[:, 0, 0:2])
            nc.gpsimd.dma_start(xsp[0][1][:, 0:2], xl[:, 0, 0:2])
            nc.sync.dma_start(wqh_sb[:, 2:5], wqh[:, 2:5])
            nc.scalar.dma_start(xsp[0][0][:, 2:5], xh[:, 0, 2:5])
            nc.gpsimd.dma_start(xsp[0][1][:, 2:5], xl[:, 0, 2:5])
            nc.sync.dma_start(wqh_sb[:, 5:8], wqh[:, 5:8])
            nc.scalar.dma_start(xsp[0][0][:, 5:8], xh[:, 0, 5:8])
            nc.gpsimd.dma_start(xsp[0][1][:, 5:8], xl[:, 0, 5:8])
            nc.sync.dma_start(wql_sb[:, 0:4], wql[:, 0:4])
            nc.scalar.dma_start(wql_sb[:, 4:8], wql[:, 4:8])
            load_x(1, (nc.scalar, nc.gpsimd))
            edum = rr_pool.tile([P, 1], f32, tag="edum")
            nc.scalar.activation(edum[:], nb8[:, 0:1], EXP)

            pend_tr = []
            late_tr = []

            def qproj_unit(c, t):
                ps = ps_b.tile([P, 512], f32, tag="b")
                th, tl = xsp[t]
                combos = [(wqh_sb, th, kp) for kp in range(KP)]
                combos += [(wql_sb, th, kp) for kp in range(KP)]
                combos += [(wqh_sb, tl, kp) for kp in range(KP)]
                for i, (w, x, kp) in enumerate(combos):
                    nc.tensor.matmul(
                        ps[:],
                        w[:, kp, :, c * P:(c + 1) * P],
                        x[:, kp],
                        start=(i == 0),
                        stop=(i == len(combos) - 1),
                        perf_mode=DR,
                    )
                # q16full stores 4*Q^T (f16)
                nc.vector.tensor_scalar_mul(
                    q16full[c][:, t * 512:(t + 1) * 512], ps[:], 1.0 / 16.0
                )
                # qt8 = e4m3(4*Q^T), staged then DR-repacked via DMA
                q8s = q8st_pool.tile([P, 512], e4, tag="q8s")
                nc.gpsimd.tensor_copy(q8s[:], q16full[c][:, t * 512:(t + 1) * 512])
                nc.sync.dma_start(qt8[:, :, c, t * 512:(t + 1) * 512], q8s[:])
                if pend_tr:
                    pend_tr.pop(0)()

                def transposes(c=c, t=t):
                    # Q natural fp8 (Q/16) into paired DR layout
                    pt_all = ps_b.tile([P, 512], f16, tag="b")
                    for j in range(4):
                        nc.tensor.transpose(
                            pt_all[:, j * P:(j + 1) * P],
                            q16full[c][:, (t * 4 + j) * P:(t * 4 + j + 1) * P],
                            id16[:],
                        )
                    qn8s = qn8st_pool.tile([P, 512], e4, tag="qn8s")
                    nc.vector.tensor_scalar_mul(qn8s[:], pt_all[:], 1.0 / 64.0)
                    nc.sync.dma_start(
                        qn8[:, 2 * t:2 * t + 2, :, c, :], qn8s[:]
                    )
                    snn_unit(c, t)

                if c < 2:
                    pend_tr.append(transposes)
                else:
                    late_tr.append(transposes)

            # Boot interleave: stream head-0 and head-1 score pieces (feeding
            # the ACT exp pipeline) between qproj units so ACT never starves.
            sc = []
            sc1 = []

            def emit(n):
                for _ in range(n):
                    if sc:
                        sc.pop(0)()
                    elif sc1:
                        sc1.pop(0)()

            def ph_piece(c, v, h, **kw):
                t, k = v // 2, v % 2
                if k == 0 and t not in ess[c]:
                    ess[c][t] = es_pool.tile(
                        [P, 2, N], e4, tag="es", name=f"e{c}_{t}"
                    )
                scores_piece(c, v, h, ess[c][t], k, **kw)

            qproj_unit(0, 0)
            new_head_state(0)
            # rows 0-3 cols [vP,512): only needs the (0,0) qt8 fold -> ACT
            # exp stream starts while qproj is still warming up
            for v in range(4):
                sc.append(lambda v=v: ph_piece(0, v, 0, hi=512, slot=2))
            qproj_unit(1, 0)
            emit(1)
            qproj_unit(2, 0)
            emit(1)
            qproj_unit(3, 0)
            emit(2)
            load_x(2, (nc.sync, nc.sync))
            qproj_unit(0, 1)
            for v in range(8):
                sc.append(lambda v=v: ph_piece(0, v, 0, lo=max(v * P, 512)))
            qproj_unit(1, 1)
            new_head_state(1)
            for v in range(8):
                sc.append(lambda v=v: ph_piece(1, v, 0))
            emit(2)
            qproj_unit(2, 1)
            emit(2)
            qproj_unit(3, 1)
            emit(2)
            load_x(3, (nc.sync, nc.sync))
            qproj_unit(0, 2)
            emit(2)
            qproj_unit(1, 2)
            emit(2)
            qproj_unit(0, 3)
            # head-0 phase 2 (qt8[0] complete)
            for v in range(NCH):
                sc.append(lambda v=v: ph_piece(0, v, 1))
                if v >= 4 and v % 2 == 0:
                    sc.append(lambda v=v: mirror_units(0, [v - 3, v - 2]))
            sc.append(lambda: mirror_units(0, [NCH - 3, NCH - 2]))
            sc.append(lambda: mirror_units(0, [NCH - 1]))
            qproj_unit(1, 3)
            # head 1 (qt8[1] complete)
            for v in range(NCH):
                if v >= 8:
                    sc1.append(lambda v=v: ph_piece(1, v, 0))
                sc1.append(lambda v=v: ph_piece(1, v, 1))
                if v >= 4 and v % 2 == 0:
                    sc1.append(lambda v=v: mirror_units(1, [v - 3, v - 2]))
            sc1.append(lambda: mirror_units(1, [NCH - 3, NCH - 2]))
            sc1.append(lambda: mirror_units(1, [NCH - 1]))
            qproj_unit(2, 2)
            emit(4)
            qproj_unit(2, 3)
            emit(4)
            qproj_unit(3, 2)
            emit(4)
            qproj_unit(3, 3)
            while pend_tr:
                pend_tr.pop(0)()

        with (
            tc.tile_pool(name="wo8", bufs=1) as wo8_pool,
            tc.tile_pool(name="y8", bufs=1) as y8p,
            tc.tile_pool(name="osb", bufs=3) as o_pool,
        ):
            wo8h_sb = wo8_pool.tile([P, HG, D], e4, tag="wo8h")
            wo8l_sb = wo8_pool.tile([P, HG, D], e4, tag="wo8l")
            nc.sync.dma_start(wo8h_sb[:], wo8h[:])
            nc.gpsimd.dma_start(wo8l_sb[:], wo8l[:])
            y8h = y8p.tile([P, HG, N], e4, tag="y8h")
            y8l = y8p.tile([P, HG, N], e4, tag="y8l")
            rbz = {}

            def outproj_unit(a, grp):
                # grp 0: heads 0-1 partial -> outA ; grp 1: heads 2-3 -> out
                # grp 2: all heads (combined) -> out
                dst = outA3 if grp == 0 else out3
                ot = o_pool.tile([P, D], f16, tag="ot")
                for d4 in range(SP):
                    pool = ps_s if d4 % 2 else ps_b
                    ps = pool.tile([P, 512], f32, tag="s" if d4 % 2 else "b")
                    combos = []
                    for cc in ((0,), (2,), (0, 2))[grp]:
                        combos += [
                            (y8h, wo8h_sb, cc),
                            (y8h, wo8l_sb, cc),
                            (y8l, wo8h_sb, cc),
                        ]
                    for i, (yy, ww, cc) in enumerate(combos):
                        nc.tensor.matmul(
                            ps[:],
                            yy[:, cc:cc + 2, a * P:(a + 1) * P],
                            ww[:, cc:cc + 2, d4 * 512:(d4 + 1) * 512],
                            start=(i == 0),
                            stop=(i == len(combos) - 1),
                            perf_mode=DR,
                        )
                    if d4 % 2 == 0:
                        nc.vector.tensor_scalar_mul(
                            ot[:, d4 * 512:(d4 + 1) * 512], ps[:], 1.0 / 64.0
                        )
                    else:
                        nc.scalar.mul(
                            ot[:, d4 * 512:(d4 + 1) * 512], ps[:], 1.0 / 64.0
                        )
                        # stream each 1KB half out as soon as it's ready
                        half = d4 // 2
                        eng = nc.sync
                        eng.dma_start(
                            dst[:, a, half * 1024:(half + 1) * 1024],
                            ot[:, half * 1024:(half + 1) * 1024],
                        )

            def pv_mm(pc, s):
                # one span of YT via paired DR matmuls
                psy = ps_b.tile([P, 512], f32, tag="b", name=f"psy{pc}_{s}")
                lo_s = s * 512
                mms = []
                for t in range(2 * s):
                    mms.append((t, None, lo_s))
                mms.append((2 * s, 0, lo_s))                # solo chunk 4s
                mms.append((2 * s, None, lo_s + 128))       # pair (4s,4s+1)
                mms.append((2 * s + 1, 0, lo_s + 256))      # solo chunk 4s+2
                mms.append((2 * s + 1, None, lo_s + 384))   # pair (4s+2,4s+3)
                for i, (t, k, lo) in enumerate(mms):
                    st = (i == 0)
                    sp = (i == len(mms) - 1)
                    hi = (s + 1) * 512
                    if k is None:
                        nc.tensor.matmul(
                            psy[:, lo - lo_s:hi - lo_s],
                            qn8[:, t, :, pc, 0:P],
                            ess[pc][t][:, :, lo:hi],
                            start=st, stop=sp, perf_mode=DR,
                        )
                    else:
                        nc.tensor.matmul(
                            psy[:, lo - lo_s:lo - lo_s + 128],
                            qn8[:, t, k, pc, 0:P],
                            ess[pc][t][:, k, lo:lo + 128],
                            start=st, stop=sp,
                        )
                return psy

            def pv_evac(pc, s, psy, rbc, zbc):
                # yt = ((psy + yt_mirror) + q16*zrbc) * rbc  -> y8 hi/lo
                yt = yts[pc]
                sl = slice(s * 512, (s + 1) * 512)
                nc.vector.tensor_tensor(yt[:, sl], psy[:], yt[:, sl], ADD)
                nc.vector.tensor_tensor(yt[:, sl], yt[:, sl], rbc[:, sl], MUL)
                dg = tmp_pool.tile([P, 512], f16, tag="dg", name=f"dg{pc}_{s}")
                nc.vector.tensor_tensor(
                    dg[:], q16full[pc][:, sl], zbc[:, sl], MUL
                )
                nc.vector.tensor_tensor(yt[:, sl], yt[:, sl], dg[:], ADD)
                nc.gpsimd.tensor_copy(y8h[:, pc, sl], yt[:, sl])
                nc.gpsimd.tensor_tensor(
                    y8l[:, pc, sl], yt[:, sl], y8h[:, pc, sl], SUB
                )

            def pv_span(pc, s, rbc, zbc):
                pv_evac(pc, s, pv_mm(pc, s), rbc, zbc)

            # boot tail: drain head-0 stream, then overlap the head-1 exp
            # tail with PV(0) spans and the r-chains
            while sc:
                sc.pop(0)()
            rbz[0] = r_chain(0)
            s0 = 0
            while sc1:
                for _ in range(6):
                    if sc1:
                        sc1.pop(0)()
                if s0 < SP:
                    pv_span(0, s0, *rbz[0])
                    s0 += 1
            while s0 < SP:
                pv_span(0, s0, *rbz[0])
                s0 += 1
            rbz[1] = r_chain(1)

            # ============ stage 1: scores(2) || PV(1) ============
            # deferred qn8/S_nn prep for heads 2-3 (PE idle under exp here)
            while late_tr:
                late_tr.pop(0)()
            new_head_state(2)
            for v in range(NCH):
                scores_row(2, v)
                if v >= 4 and v % 2 == 0:
                    mirror_units(2, [v - 3, v - 2])
                    if v >= 8:
                        pv_span(1, (v - 8) // 2, *rbz[1])
            mirror_units(2, [NCH - 3, NCH - 2])
            mirror_units(2, [NCH - 1])
            rbz[2] = r_chain(2)

            # ===== stage 2: scores(3) || PV(2) + outproj-A (heads 0-1) =====
            new_head_state(3)
            for v in range(NCH):
                scores_row(3, v)
                if v >= 4 and v % 2 == 0:
                    mirror_units(3, [v - 3, v - 2])
                if v % 4 == 2:
                    pv_span(2, v // 4, *rbz[2])
            mirror_units(3, [NCH - 3, NCH - 2])
            mirror_units(3, [NCH - 1])
            rbz[3] = r_chain(3)

            # lull fill: heads-0/1 partials are independent of r_chain(3)
            for a in range(8):
                outproj_unit(a, 0)

            # == stage 3: PV(3) + outproj (B for a<6, combined for a>=6) ==
            psys = {0: pv_mm(3, 0), 1: pv_mm(3, 1)}
            for s in range(SP):
                pv_evac(3, s, psys[s], *rbz[3])
                if s + 2 < SP:
                    psys[s + 2] = pv_mm(3, s + 2)
                for a in range(4 * s, 4 * s + 4):
                    outproj_unit(a, 1 if a < 8 else 2)

    nc.compile()
    return nc


def _hi_lo(arr):
    hi = arr.astype(ml_dtypes.float8_e4m3)
    lo = (arr - hi.astype(np.float32)).astype(ml_dtypes.float8_e4m3)
    return hi, lo


def _pack_x(xt_hl):
    return np.ascontiguousarray(
        xt_hl.reshape(KP, 2, P, SP, 512).transpose(2, 3, 0, 1, 4)
    )


def kernel(x, Wq, Wo, bo):
    x = np.asarray(x, dtype=np.float32)
    Wq = np.asarray(Wq, dtype=np.float32)
    Wo = np.asarray(Wo, dtype=np.float32)
    bo = np.asarray(bo, dtype=np.float32)
    B = x.shape[0]
    assert B == 2 and x.shape == (B, N, D)
    assert Wq.shape == (D, D) and Wo.shape == (D, D)

    if "nc" not in _CACHE:
        _CACHE["nc"] = build_nc()
    nc = _CACHE["nc"]

    packed_x = []
    for b in range(B):
        hi, lo = _hi_lo(np.ascontiguousarray(x[b].T))
        packed_x.append((_pack_x(hi), _pack_x(lo)))
    in_maps = []
    for core in range(8):
        b, hg = core // 4, core % 4
        wq_s = WQS * Wq[:, hg * HCOLS:(hg + 1) * HCOLS]
        wqh, wql = _hi_lo(np.ascontiguousarray(wq_s))
        wo8h_a, wo8l_a = _hi_lo(
            np.ascontiguousarray(WQS * Wo[hg * HCOLS:(hg + 1) * HCOLS, :])
        )
        in_maps.append(
            {
                "xh": packed_x[b][0],
                "xl": packed_x[b][1],
                "wqh": np.ascontiguousarray(
                    wqh.reshape(KP, 2, P, HCOLS).transpose(2, 0, 1, 3)
                ),
                "wql": np.ascontiguousarray(
                    wql.reshape(KP, 2, P, HCOLS).transpose(2, 0, 1, 3)
                ),
                "wo8h": np.ascontiguousarray(
                    wo8h_a.reshape(HG, P, D).transpose(1, 0, 2)
                ),
                "wo8l": np.ascontiguousarray(
                    wo8l_a.reshape(HG, P, D).transpose(1, 0, 2)
                ),
            }
        )

    res = run_bass_kernel_spmd(nc, in_maps, list(range(8)))
    _CACHE["last_res"] = res
    out = np.zeros((B, N, D), dtype=np.float32)
    for core in range(8):
        b = core // 4
        out[b] += res.results[core]["out"].astype(np.float32)
        out[b][0:1024] += res.results[core]["outA"][0:1024].astype(np.float32)
    out += bo
    return out


# revision 7
# speedup vs baseline: 1.0935x; 1.0228x over previous
"""Trainium2 Bass kernel for nn_MultiHeadAttention_78864189489198 — v3.

fp8 paired-es variant.

E = exp(S - B8) is stored in fp8 (e4m3) chunk-PAIRED tiles es8[t] of
shape [P, 2, N] holding row-chunks (2t, 2t+1).  This lets both the PV
(attn @ V) and mirror matmuls run in DoubleRow perf mode (contraction
256 = two chunks at once, 0.5 cyc/col), cutting their PE cost ~4x/3.3x
vs the fp16 v2 path.

The huge diagonal E_nn (softmax is diagonally dominant here) cannot
live in fp8: the scores diagonal is masked to 0 in PSUM before exp
(DVE multiply by (1-I)), and the diagonal contribution is restored
exactly:  S_nn = rowsum(q16^2) via per-chunk ones-matmuls,
E_nn = exp(S_nn - 9), Y += (E_nn / r) * Q with r assembled from the
exp accumulators + mirror ones-column + E_nn (consistent numerator /
denominator, so S_nn quantization error cancels).

Everything else as v2: fp8 hi/lo DoubleRow qproj/outproj, host-side
batch x head-group sharding (8 cores), fp16 partial outputs summed on
host.
"""

import numpy as np
import ml_dtypes

import concourse.bass as bass
import concourse.mybir as mybir
import concourse.tile as tile
from concourse import bacc
from concourse.bass_utils import run_bass_kernel_spmd
from concourse.masks import make_identity
from contextlib import ExitStack

P = 128
N = 2048
D = 2048
KP = 8
HG = 4
HD = 128
HCOLS = HG * HD
SP = N // 512
NCH = N // P
NPAIR = NCH // 2
SCALE = HD ** -0.5
C_BIAS = 9.0
B8 = 3.5
WQS = 64.0

f32 = mybir.dt.float32
f32r = mybir.dt.float32r
f16 = mybir.dt.float16
e4 = mybir.dt.float8e4
DR = mybir.MatmulPerfMode.DoubleRow

_CACHE = {}

EXP = mybir.ActivationFunctionType.Exp
ADD = mybir.AluOpType.add
SUB = mybir.AluOpType.subtract
MUL = mybir.AluOpType.mult


def build_nc():
    nc = bacc.Bacc("TRN2", target_bir_lowering=False, debug=False)
    xh = nc.dram_tensor("xh", [P, SP, KP, 2, 512], e4, kind="ExternalInput")
    xl = nc.dram_tensor("xl", [P, SP, KP, 2, 512], e4, kind="ExternalInput")
    wqh = nc.dram_tensor("wqh", [P, KP, 2, HCOLS], e4, kind="ExternalInput")
    wql = nc.dram_tensor("wql", [P, KP, 2, HCOLS], e4, kind="ExternalInput")
    wo8h = nc.dram_tensor("wo8h", [P, HG, D], e4, kind="ExternalInput")
    wo8l = nc.dram_tensor("wo8l", [P, HG, D], e4, kind="ExternalInput")
    out = nc.dram_tensor("out", [N, D], f16, kind="ExternalOutput")
    out3 = out.rearrange("(a p) n -> p a n", p=P)
    outA = nc.dram_tensor("outA", [N, D], f16, kind="ExternalOutput")
    outA3 = outA.rearrange("(a p) n -> p a n", p=P)

    EB8 = float(np.exp(-B8))          # masked-diag residue per row
    EB9 = float(np.exp(B8 - C_BIAS))  # es8 scale -> e^-9 scale

    with (
        nc.allow_low_precision(reason="fp8/fp16 dataflow is intentional"),
        tile.TileContext(nc) as tc,
        ExitStack() as ctx,
    ):
        const_pool = ctx.enter_context(tc.tile_pool(name="const", bufs=1))
        qt8_pool = ctx.enter_context(tc.tile_pool(name="qt8", bufs=1))
        qn8_pool = ctx.enter_context(tc.tile_pool(name="qn8", bufs=1))
        q16_pool = ctx.enter_context(tc.tile_pool(name="q16f", bufs=4))
        es_pool = ctx.enter_context(tc.tile_pool(name="es8", bufs=16))
        rr_pool = ctx.enter_context(tc.tile_pool(name="rr", bufs=4))
        tmp_pool = ctx.enter_context(tc.tile_pool(name="tmp", bufs=3))
        snn_pool = ctx.enter_context(tc.tile_pool(name="snn", bufs=4))
        rbc_pool = ctx.enter_context(tc.tile_pool(name="rbc", bufs=4))
        yt_pool = ctx.enter_context(tc.tile_pool(name="yt", bufs=2))
        ynm_pool = ctx.enter_context(tc.tile_pool(name="ynm", bufs=4))
        q8st_pool = ctx.enter_context(tc.tile_pool(name="q8st", bufs=2))
        qn8st_pool = ctx.enter_context(tc.tile_pool(name="qn8st", bufs=2))
        ps_s = ctx.enter_context(tc.tile_pool(name="ps_s", bufs=2, space="PSUM"))
        ps_b = ctx.enter_context(tc.tile_pool(name="ps_b", bufs=4, space="PSUM"))

        idr = const_pool.tile([P, P], f32r, tag="idr")
        id16 = const_pool.tile([P, P], f16, tag="id16")
        # DR-layout fp8 identities: ia8p^T @ ia8n = -4096*I (diag canceller)
        ia8p = const_pool.tile([64, 2, P], e4, tag="ia8p")
        ia8n = const_pool.tile([64, 2, P], e4, tag="ia8n")
        nb8 = const_pool.tile([P, 1], f32, tag="nb8")
        nc.gpsimd.memset(nb8[:], -B8)
        nb9 = const_pool.tile([P, 1], f32, tag="nb9")
        nc.gpsimd.memset(nb9[:], -C_BIAS)
        ones1 = const_pool.tile([P, 1], f16, tag="ones1")
        nc.gpsimd.memset(ones1[:], 1.0)

        # qt8[p, r, c, n] : 4*Q^T head c, DR layout (hd = 2p+r-ish)
        qt8 = qt8_pool.tile([64, 2, HG, N], e4, tag="qt8")
        # qn8[p, t, k, c, :] = Q[(2t+k)*P+p, head c, :]/16
        # (128-multiple strides: fp8 Ldweights restriction)
        qn8 = qn8_pool.tile([P, NPAIR, 2, HG, P], e4, tag="qn8")
        # fp8 ones for the mirror r-column matmuls
        on8 = const_pool.tile([P, 2, 1], e4, tag="on8")
        nc.gpsimd.memset(on8[:], 1.0)
        # q16full[c] = 4*Q^T head c, f16, persistent (diag path + src of qt8/qn8)
        q16full = {}
        for c in range(HG):
            q16full[c] = q16_pool.tile([P, N], f16, tag="q16f", name=f"q16f{c}")

        yts = {}
        rrechs = {}
        rmirs = {}
        snns = {}
        ess = {}       # ess[c][t] = es8 pair tile [P, 2, N]

        def new_head_state(c):
            rrechs[c] = rr_pool.tile([P, NCH, 3], f32, tag="rrech", name=f"rr{c}")
            rmirs[c] = rr_pool.tile([P, NCH], f32, tag="rmir", name=f"rm{c}")
            nc.vector.memset(rrechs[c][:], 0.0)
            nc.vector.memset(rmirs[c][:], 0.0)
            yts[c] = yt_pool.tile([P, N], f16, tag="yt", name=f"yt{c}")
            nc.gpsimd.memset(yts[c][:, 0:P], 0.0)
            ess[c] = {}

        def scores_piece(c, v, h, es, k, lo=None, hi=None, slot=None):
            # scores cols [max(128v, 1024h), 1024(h+1)) for row-chunk v
            if lo is None:
                lo = max(P * v, 1024 * h)
            if hi is None:
                hi = 1024 * (h + 1)
            if slot is None:
                slot = h
            if lo >= hi:
                return
            ps = ps_s.tile([P, 1024], f32, tag="s")
            col = lo
            while col < hi:
                w = min(512 - col % 512, hi - col)
                nc.tensor.matmul(
                    ps[:, col - 1024 * h:col - 1024 * h + w],
                    qt8[:, :, c, v * P:(v + 1) * P],
                    qt8[:, :, c, col:col + w],
                    start=True,
                    stop=True,
                    perf_mode=DR,
                )
                col += w
            if lo == v * P:
                # diag block leads this piece: add -4096*I so exp of the
                # exact diagonal flushes to 0 in fp8 (residue ~1e-4, ignored)
                nc.tensor.matmul(
                    ps[:, lo - 1024 * h:lo - 1024 * h + P],
                    ia8p[:],
                    ia8n[:],
                    start=False,
                    stop=True,
                    perf_mode=DR,
                )
            nc.scalar.activation(
                es[:, k, lo:hi],
                ps[:, lo - 1024 * h:hi - 1024 * h],
                EXP,
                bias=nb8[:, 0:1],
                scale=SCALE / 16.0,
                accum_out=rrechs[c][:, v, slot:slot + 1],
            )

        def scores_row(c, v):
            t, k = v // 2, v % 2
            if k == 0:
                ess[c][t] = es_pool.tile([P, 2, N], e4, tag="es", name=f"e{c}_{t}")
            es = ess[c][t]
            scores_piece(c, v, 0, es, k)
            scores_piece(c, v, 1, es, k)

        def mirror_units(c, us):
            # Y[u-chunk, :] += sum_{v<u} E8[v, u-block]^T @ [Q_v/16 | 1]
            # us: 1-2 consecutive chunk indices sharing one PSUM tile
            nu = len(us)
            w = 128 * nu
            pm = ps_b.tile([P, w + nu], f32, tag="b", name=f"pm{c}_{us[0]}")
            for j, u in enumerate(us):
                npairs = u // 2
                tail = [(t, None) for t in range(npairs)]
                if u % 2 == 1:
                    tail.append((npairs, 0))
                for i, (t, k) in enumerate(tail):
                    st = (i == 0)
                    sp = (i == len(tail) - 1)
                    if k is None:
                        nc.tensor.matmul(
                            pm[:, j * P:(j + 1) * P],
                            ess[c][t][:, :, u * P:(u + 1) * P],
                            qn8[:, t, :, c, :],
                            start=st, stop=sp, perf_mode=DR,
                        )
                        nc.tensor.matmul(
                            pm[:, w + j:w + j + 1],
                            ess[c][t][:, :, u * P:(u + 1) * P],
                            on8[:],
                            start=st, stop=sp, perf_mode=DR,
                        )
                    else:
                        nc.tensor.matmul(
                            pm[:, j * P:(j + 1) * P],
                            ess[c][t][:, k, u * P:(u + 1) * P],
                            qn8[:, t, k, c, :],
                            start=st, stop=sp,
                        )
                        nc.tensor.matmul(
                            pm[:, w + j:w + j + 1],
                            ess[c][t][:, k, u * P:(u + 1) * P],
                            on8[:, 0],
                            start=st, stop=sp,
                        )
            ynm = ynm_pool.tile([P, w], f16, tag="ynm")
            nc.vector.tensor_copy(ynm[:], pm[:, 0:w])
            nc.vector.tensor_copy(rmirs[c][:, us[0]:us[0] + nu], pm[:, w:w + nu])
            ptm = ps_b.tile([P, w], f16, tag="b", name=f"ptm{c}_{us[0]}")
            for j in range(nu):
                nc.tensor.transpose(
                    ptm[:, j * P:(j + 1) * P], ynm[:, j * P:(j + 1) * P], id16[:]
                )
            nc.vector.tensor_copy(yts[c][:, us[0] * P:(us[0] + nu) * P], ptm[:])

        def snn_unit(c, t):
            # S_nn for chunks 4t..4t+3 of head c: colsums of (q16)^2 blocks
            qsq = tmp_pool.tile([P, 512], f16, tag="qsq")
            nc.vector.tensor_tensor(
                qsq[:], q16full[c][:, t * 512:(t + 1) * 512],
                q16full[c][:, t * 512:(t + 1) * 512], MUL,
            )
            pq = ps_b.tile([P, 4], f32, tag="b", name=f"pq{c}_{t}")
            for j in range(4):
                nc.tensor.matmul(
                    pq[:, j:j + 1],
                    qsq[:, j * P:(j + 1) * P],
                    ones1[:],
                    start=True, stop=True,
                )
            nc.vector.tensor_copy(snns[c][:, t * 4:(t + 1) * 4], pq[:])

        def r_chain(c):
            # r9 = (sum_h rrech + rmir - e^-B8) * e^(B8-9) + Enn9   [P, NCH]
            enn = tmp_pool.tile([P, NCH], f32, tag="enn")
            nc.scalar.activation(
                enn[:], snns[c][:], EXP, bias=nb9[:, 0:1], scale=SCALE / 16.0,
            )
            rr = tmp_pool.tile([P, NCH], f32, tag="rrec")
            nc.vector.tensor_reduce(
                rr[:], rrechs[c][:], mybir.AxisListType.X, ADD
            )
            nc.vector.tensor_tensor(rr[:], rr[:], rmirs[c][:], ADD)
            nc.vector.tensor_scalar_mul(rr[:], rr[:], EB9)
            nc.vector.tensor_tensor(rr[:], rr[:], enn[:], ADD)
            rcp = tmp_pool.tile([P, NCH], f32r, tag="rcp")
            nc.vector.reciprocal(rcp[:], rr[:])
            # broadcast vectors: rbc = 16*e^(B8-9)/r9 ; zbc = Enn9/(4*r9)
            # (evac: yt = (psy + yt_mirror)*rbc + q16*zbc ; zbc <= 0.25)
            offv = tmp_pool.tile([P, 2, NCH], f32r, tag="offv")
            nc.vector.tensor_scalar_mul(offv[:, 0], rcp[:], 16.0 * EB9)
            nc.vector.tensor_tensor(offv[:, 1], rcp[:], enn[:], MUL)
            nc.vector.tensor_scalar_mul(offv[:, 1], offv[:, 1], 0.25)
            pt = ps_b.tile([NCH, 2 * P], f32r, tag="b", name=f"rt{c}")
            nc.tensor.transpose(pt[:, 0:P], offv[:, 0], idr[:])
            nc.tensor.transpose(pt[:, P:2 * P], offv[:, 1], idr[:])
            both16 = tmp_pool.tile([NCH, 2 * P], f16, tag="both16")
            nc.vector.tensor_copy(both16[:], pt[:])
            rbc = rbc_pool.tile([P, N], f16, tag="rbc", name=f"rb{c}")
            zbc = rbc_pool.tile([P, N], f16, tag="zbc", name=f"zb{c}")
            nc.sync.dma_start(rbc[0:1, :], both16[:, 0:P])
            nc.sync.dma_start(zbc[0:1, :], both16[:, P:2 * P])
            nc.gpsimd.partition_broadcast(rbc[:], rbc[0:1, :])
            nc.gpsimd.partition_broadcast(zbc[:], zbc[0:1, :])
            return rbc, zbc

        with (
            tc.tile_pool(name="xt", bufs=4) as xt_pool,
            tc.tile_pool(name="wq", bufs=1) as wq_pool,
        ):
            for c in range(HG):
                snns[c] = snn_pool.tile([P, NCH], f32, tag="snn", name=f"sn{c}")

            wqh_sb = wq_pool.tile([P, KP, 2, HCOLS], e4, tag="wqh")
            wql_sb = wq_pool.tile([P, KP, 2, HCOLS], e4, tag="wql")
            xsp = {}

            def load_x(t, engs):
                xsp[t] = (
                    xt_pool.tile([P, KP, 2, 512], e4, tag="xt", name=f"xh{t}"),
                    xt_pool.tile([P, KP, 2, 512], e4, tag="xt", name=f"xl{t}"),
                )
                engs[0].dma_start(xsp[t][0][:], xh[:, t])
                engs[1].dma_start(xsp[t][1][:], xl[:, t])

            # prologue loads: fine-grained halves so qproj starts early
            xsp[0] = (
                xt_pool.tile([P, KP, 2, 512], e4, tag="xt", name="xh0"),
                xt_pool.tile([P, KP, 2, 512], e4, tag="xt", name="xl0"),
            )
            nc.sync.dma_start(wqh_sb[:, 0:2], wqh[:, 0:2])
            nc.scalar.dma_start(xsp[0][0][:, 0:2], xh[:, 0, 0:2])
            nc.gpsimd.dma_start(wql_sb[:, 0:4], wql[:, 0:4])
            nc.sync.dma_start(wqh_sb[:, 2:5], wqh[:, 2:5])
            nc.scalar.dma_start(xsp[0][0][:, 2:5], xh[:, 0, 2:5])
            nc.gpsimd.dma_start(xsp[0][1][:, 0:2], xl[:, 0, 0:2])
            nc.sync.dma_start(wqh_sb[:, 5:8], wqh[:, 5:8])
            nc.scalar.dma_start(xsp[0][0][:, 5:8], xh[:, 0, 5:8])
            nc.gpsimd.dma_start(xsp[0][1][:, 2:5], xl[:, 0, 2:5])
            nc.sync.dma_start(wql_sb[:, 4:8], wql[:, 4:8])
            nc.gpsimd.dma_start(xsp[0][1][:, 5:8], xl[:, 0, 5:8])
            load_x(1, (nc.scalar, nc.sync))
            edum = rr_pool.tile([P, 1], f32, tag="edum")
            nc.scalar.activation(edum[:], nb8[:, 0:1], EXP)
            make_identity(nc, id16[:])
            nc.vector.tensor_copy(idr[:], id16[:])
            i8st = q8st_pool.tile([P, P], e4, tag="q8s", name="i8p")
            nc.vector.tensor_scalar_mul(i8st[:], id16[:], 64.0)
            nc.sync.dma_start(ia8p[:], i8st[:])
            i8st2 = q8st_pool.tile([P, P], e4, tag="q8s", name="i8n")
            nc.vector.tensor_scalar_mul(i8st2[:], id16[:], -64.0)
            nc.sync.dma_start(ia8n[:], i8st2[:])

            pend_tr = []
            late_tr = []

            def qproj_unit(c, t):
                ps = ps_b.tile([P, 512], f32, tag="b")
                th, tl = xsp[t]
                combos = [(wqh_sb, th, kp) for kp in range(KP)]
                combos += [(wql_sb, th, kp) for kp in range(KP)]
                combos += [(wqh_sb, tl, kp) for kp in range(KP)]
                for i, (w, x, kp) in enumerate(combos):
                    nc.tensor.matmul(
                        ps[:],
                        w[:, kp, :, c * P:(c + 1) * P],
                        x[:, kp],
                        start=(i == 0),
                        stop=(i == len(combos) - 1),
                        perf_mode=DR,
                    )
                # q16full stores 4*Q^T (f16)
                nc.vector.tensor_scalar_mul(
                    q16full[c][:, t * 512:(t + 1) * 512], ps[:], 1.0 / 16.0
                )
                # qt8 = e4m3(4*Q^T), staged then DR-repacked via DMA
                q8s = q8st_pool.tile([P, 512], e4, tag="q8s")
                nc.gpsimd.tensor_copy(q8s[:], q16full[c][:, t * 512:(t + 1) * 512])
                nc.sync.dma_start(qt8[:, :, c, t * 512:(t + 1) * 512], q8s[:])
                if pend_tr:
                    pend_tr.pop(0)()

                def transposes(c=c, t=t):
                    # Q natural fp8 (Q/16) into paired DR layout
                    pt_all = ps_b.tile([P, 512], f16, tag="b")
                    for j in range(4):
                        nc.tensor.transpose(
                            pt_all[:, j * P:(j + 1) * P],
                            q16full[c][:, (t * 4 + j) * P:(t * 4 + j + 1) * P],
                            id16[:],
                        )
                    qn8s = qn8st_pool.tile([P, 512], e4, tag="qn8s")
                    nc.vector.tensor_scalar_mul(qn8s[:], pt_all[:], 1.0 / 64.0)
                    nc.sync.dma_start(
                        qn8[:, 2 * t:2 * t + 2, :, c, :], qn8s[:]
                    )
                    snn_unit(c, t)

                if c < 2:
                    pend_tr.append(transposes)
                else:
                    late_tr.append(transposes)

            # Boot interleave: stream head-0 and head-1 score pieces (feeding
            # the ACT exp pipeline) between qproj units so ACT never starves.
            sc = []
            sc1 = []

            def emit(n):
                for _ in range(n):
                    if sc:
                        sc.pop(0)()
                    elif sc1:
                        sc1.pop(0)()

            def ph_piece(c, v, h, **kw):
                t, k = v // 2, v % 2
                if k == 0 and t not in ess[c]:
                    ess[c][t] = es_pool.tile(
                        [P, 2, N], e4, tag="es", name=f"e{c}_{t}"
                    )
                scores_piece(c, v, h, ess[c][t], k, **kw)

            qproj_unit(0, 0)
            new_head_state(0)
            # rows 0-3 cols [vP,512): only needs the (0,0) qt8 fold -> ACT
            # exp stream starts while qproj is still warming up
            for v in range(4):
                sc.append(lambda v=v: ph_piece(0, v, 0, hi=512, slot=2))
            qproj_unit(1, 0)
            emit(1)
            qproj_unit(2, 0)
            emit(1)
            qproj_unit(3, 0)
            emit(2)
            load_x(2, (nc.sync, nc.sync))
            qproj_unit(0, 1)
            for v in range(8):
                sc.append(lambda v=v: ph_piece(0, v, 0, lo=max(v * P, 512)))
            qproj_unit(1, 1)
            new_head_state(1)
            for v in range(8):
                sc.append(lambda v=v: ph_piece(1, v, 0))
            emit(2)
            qproj_unit(2, 1)
            emit(2)
            qproj_unit(3, 1)
            emit(2)
            load_x(3, (nc.sync, nc.sync))
            qproj_unit(0, 2)
            emit(2)
            qproj_unit(1, 2)
            emit(2)
            qproj_unit(0, 3)
            # head-0 phase 2 (qt8[0] complete)
            for v in range(NCH):
                sc.append(lambda v=v: ph_piece(0, v, 1))
                if v >= 4 and v % 2 == 0:
                    sc.append(lambda v=v: mirror_units(0, [v - 3, v - 2]))
            sc.append(lambda: mirror_units(0, [NCH - 3, NCH - 2]))
            sc.append(lambda: mirror_units(0, [NCH - 1]))
            qproj_unit(1, 3)
            # head 1 (qt8[1] complete)
            for v in range(NCH):
                if v >= 8:
                    sc1.append(lambda v=v: ph_piece(1, v, 0))
                sc1.append(lambda v=v: ph_piece(1, v, 1))
                if v >= 4 and v % 2 == 0:
                    sc1.append(lambda v=v: mirror_units(1, [v - 3, v - 2]))
            sc1.append(lambda: mirror_units(1, [NCH - 3, NCH - 2]))
            sc1.append(lambda: mirror_units(1, [NCH - 1]))
            qproj_unit(2, 2)
            emit(4)
            qproj_unit(2, 3)
            emit(4)
            qproj_unit(3, 2)
            emit(4)
            qproj_unit(3, 3)
            while pend_tr:
                pend_tr.pop(0)()

        with (
            tc.tile_pool(name="wo8", bufs=1) as wo8_pool,
            tc.tile_pool(name="y8", bufs=1) as y8p,
            tc.tile_pool(name="osb", bufs=3) as o_pool,
        ):
            wo8h_sb = wo8_pool.tile([P, HG, D], e4, tag="wo8h")
            wo8l_sb = wo8_pool.tile([P, HG, D], e4, tag="wo8l")
            nc.sync.dma_start(wo8h_sb[:], wo8h[:])
            nc.gpsimd.dma_start(wo8l_sb[:], wo8l[:])
            y8h = y8p.tile([P, HG, N], e4, tag="y8h")
            y8l = y8p.tile([P, HG, N], e4, tag="y8l")
            rbz = {}

            def outproj_unit(a, grp):
                # grp 0: heads 0-1 partial -> outA ; grp 1: heads 2-3 -> out
                # grp 2: all heads (combined) -> out
                dst = outA3 if grp == 0 else out3
                ot = o_pool.tile([P, D], f16, tag="ot")
                for d4 in range(SP):
                    pool = ps_s if d4 % 2 else ps_b
                    ps = pool.tile([P, 512], f32, tag="s" if d4 % 2 else "b")
                    combos = []
                    for cc in ((0,), (2,), (0, 2))[grp]:
                        combos += [
                            (y8h, wo8h_sb, cc),
                            (y8h, wo8l_sb, cc),
                            (y8l, wo8h_sb, cc),
                        ]
                    for i, (yy, ww, cc) in enumerate(combos):
                        nc.tensor.matmul(
                            ps[:],
                            yy[:, cc:cc + 2, a * P:(a + 1) * P],
                            ww[:, cc:cc + 2, d4 * 512:(d4 + 1) * 512],
                            start=(i == 0),
                            stop=(i == len(combos) - 1),
                            perf_mode=DR,
                        )
                    if d4 % 2 == 0:
                        nc.vector.tensor_scalar_mul(
                            ot[:, d4 * 512:(d4 + 1) * 512], ps[:], 1.0 / 64.0
                        )
                    else:
                        nc.scalar.mul(
                            ot[:, d4 * 512:(d4 + 1) * 512], ps[:], 1.0 / 64.0
                        )
                        # stream each 1KB half out as soon as it's ready
                        half = d4 // 2
                        eng = nc.sync
                        eng.dma_start(
                            dst[:, a, half * 1024:(half + 1) * 1024],
                            ot[:, half * 1024:(half + 1) * 1024],
                        )

            def pv_mm(pc, s):
                # one span of YT via paired DR matmuls
                psy = ps_b.tile([P, 512], f32, tag="b", name=f"psy{pc}_{s}")
                lo_s = s * 512
                mms = []
                for t in range(2 * s):
                    mms.append((t, None, lo_s))
                mms.append((2 * s, 0, lo_s))                # solo chunk 4s
                mms.append((2 * s, None, lo_s + 128))       # pair (4s,4s+1)
                mms.append((2 * s + 1, 0, lo_s + 256))      # solo chunk 4s+2
                mms.append((2 * s + 1, None, lo_s + 384))   # pair (4s+2,4s+3)
                for i, (t, k, lo) in enumerate(mms):
                    st = (i == 0)
                    sp = (i == len(mms) - 1)
                    hi = (s + 1) * 512
                    if k is None:
                        nc.tensor.matmul(
                            psy[:, lo - lo_s:hi - lo_s],
                            qn8[:, t, :, pc, 0:P],
                            ess[pc][t][:, :, lo:hi],
                            start=st, stop=sp, perf_mode=DR,
                        )
                    else:
                        nc.tensor.matmul(
                            psy[:, lo - lo_s:lo - lo_s + 128],
                            qn8[:, t, k, pc, 0:P],
                            ess[pc][t][:, k, lo:lo + 128],
                            start=st, stop=sp,
                        )
                return psy

            def pv_evac(pc, s, psy, rbc, zbc):
                # yt = ((psy + yt_mirror) + q16*zrbc) * rbc  -> y8 hi/lo
                yt = yts[pc]
                sl = slice(s * 512, (s + 1) * 512)
                nc.vector.tensor_tensor(yt[:, sl], psy[:], yt[:, sl], ADD)
                nc.vector.tensor_tensor(yt[:, sl], yt[:, sl], rbc[:, sl], MUL)
                dg = tmp_pool.tile([P, 512], f16, tag="dg", name=f"dg{pc}_{s}")
                nc.vector.tensor_tensor(
                    dg[:], q16full[pc][:, sl], zbc[:, sl], MUL
                )
                nc.vector.tensor_tensor(yt[:, sl], yt[:, sl], dg[:], ADD)
                nc.gpsimd.tensor_copy(y8h[:, pc, sl], yt[:, sl])
                nc.gpsimd.tensor_tensor(
                    y8l[:, pc, sl], yt[:, sl], y8h[:, pc, sl], SUB
                )

            def pv_span(pc, s, rbc, zbc):
                pv_evac(pc, s, pv_mm(pc, s), rbc, zbc)

            # boot tail: drain head-0 stream, then overlap the head-1 exp
            # tail with PV(0) spans and the r-chains
            while sc:
                sc.pop(0)()
            rbz[0] = r_chain(0)
            s0 = 0
            while sc1:
                for _ in range(6):
                    if sc1:
                        sc1.pop(0)()
                if s0 < SP:
                    pv_span(0, s0, *rbz[0])
                    s0 += 1
            while s0 < SP:
                pv_span(0, s0, *rbz[0])
                s0 += 1
            rbz[1] = r_chain(1)

            # ============ stage 1: scores(2) || PV(1) ============
            # deferred qn8/S_nn prep for heads 2-3 (PE idle under exp here)
            while late_tr:
                late_tr.pop(0)()
            new_head_state(2)
            for v in range(NCH):
                scores_row(2, v)
                if v >= 4 and v % 2 == 0:
                    mirror_units(2, [v - 3, v - 2])
                    if v >= 8:
                        pv_span(1, (v - 8) // 2, *rbz[1])
            mirror_units(2, [NCH - 3, NCH - 2])
            mirror_units(2, [NCH - 1])
            rbz[2] = r_chain(2)

            # ===== stage 2: scores(3) || PV(2) + outproj-A (heads 0-1) =====
            new_head_state(3)
            for v in range(NCH):
                scores_row(3, v)
                if v >= 4 and v % 2 == 0:
                    mirror_units(3, [v - 3, v - 2])
                if v % 4 == 2:
                    pv_span(2, v // 4, *rbz[2])
            mirror_units(3, [NCH - 3, NCH - 2])
            mirror_units(3, [NCH - 1])
            rbz[3] = r_chain(3)

            # lull fill: heads-0/1 partials are independent of r_chain(3)
            for a in range(8):
                outproj_unit(a, 0)

            # == stage 3: PV(3) + outproj (B for a<6, combined for a>=6) ==
            psys = {0: pv_mm(3, 0), 1: pv_mm(3, 1)}
            for s in range(SP):
                pv_evac(3, s, psys[s], *rbz[3])
                if s + 2 < SP:
                    psys[s + 2] = pv_mm(3, s + 2)
                for a in range(4 * s, 4 * s + 4):
                    outproj_unit(a, 1 if a < 8 else 2)

    nc.compile()
    return nc


def _hi_lo(arr):
    hi = arr.astype(ml_dtypes.float8_e4m3)
    lo = (arr - hi.astype(np.float32)).astype(ml_dtypes.float8_e4m3)
    return hi, lo


def _pack_x(xt_hl):
    return np.ascontiguousarray(
        xt_hl.reshape(KP, 2, P, SP, 512).transpose(2, 3, 0, 1, 4)
    )


def kernel(x, Wq, Wo, bo):
    x = np.asarray(x, dtype=np.float32)
    Wq = np.asarray(Wq, dtype=np.float32)
    Wo = np.asarray(Wo, dtype=np.float32)
    bo = np.asarray(bo, dtype=np.float32)
    B = x.shape[0]
    assert B == 2 and x.shape == (B, N, D)
    assert Wq.shape == (D, D) and Wo.shape == (D, D)

    if "nc" not in _CACHE:
        _CACHE["nc"] = build_nc()
    nc = _CACHE["nc"]

    packed_x = []
    for b in range(B):
        hi, lo = _hi_lo(np.ascontiguousarray(x[b].T))
        packed_x.append((_pack_x(hi), _pack_x(lo)))
    in_maps = []
    for core in range(8):
        b, hg = core // 4, core % 4
        wq_s = WQS * Wq[:, hg * HCOLS:(hg + 1) * HCOLS]
        wqh, wql = _hi_lo(np.ascontiguousarray(wq_s))
        wo8h_a, wo8l_a = _hi_lo(
            np.ascontiguousarray(WQS * Wo[hg * HCOLS:(hg + 1) * HCOLS, :])
        )
        in_maps.append(
            {
                "xh": packed_x[b][0],
                "xl": packed_x[b][1],
                "wqh": np.ascontiguousarray(
                    wqh.reshape(KP, 2, P, HCOLS).transpose(2, 0, 1, 3)
                ),
                "wql": np.ascontiguousarray(
                    wql.reshape(KP, 2, P, HCOLS).transpose(2, 0, 1, 3)
                ),
                "wo8h": np.ascontiguousarray(
                    wo8h_a.reshape(HG, P, D).transpose(1, 0, 2)
                ),
                "wo8l": np.ascontiguousarray(
                    wo8l_a.reshape(HG, P, D).transpose(1, 0, 2)
                ),
            }
        )

    res = run_bass_kernel_spmd(nc, in_maps, list(range(8)))
    _CACHE["last_res"] = res
    out = np.zeros((B, N, D), dtype=np.float32)
    for core in range(8):
        b = core // 4
        out[b] += res.results[core]["out"].astype(np.float32)
        out[b][0:1024] += res.results[core]["outA"][0:1024].astype(np.float32)
    out += bo
    return out


# revision 8
# speedup vs baseline: 1.0937x; 1.0002x over previous
"""Trainium2 Bass kernel for nn_MultiHeadAttention_78864189489198 — v3.

fp8 paired-es variant.

E = exp(S - B8) is stored in fp8 (e4m3) chunk-PAIRED tiles es8[t] of
shape [P, 2, N] holding row-chunks (2t, 2t+1).  This lets both the PV
(attn @ V) and mirror matmuls run in DoubleRow perf mode (contraction
256 = two chunks at once, 0.5 cyc/col), cutting their PE cost ~4x/3.3x
vs the fp16 v2 path.

The huge diagonal E_nn (softmax is diagonally dominant here) cannot
live in fp8: the scores diagonal is masked to 0 in PSUM before exp
(DVE multiply by (1-I)), and the diagonal contribution is restored
exactly:  S_nn = rowsum(q16^2) via per-chunk ones-matmuls,
E_nn = exp(S_nn - 9), Y += (E_nn / r) * Q with r assembled from the
exp accumulators + mirror ones-column + E_nn (consistent numerator /
denominator, so S_nn quantization error cancels).

Everything else as v2: fp8 hi/lo DoubleRow qproj/outproj, host-side
batch x head-group sharding (8 cores), fp16 partial outputs summed on
host.
"""

import numpy as np
import ml_dtypes

import concourse.bass as bass
import concourse.mybir as mybir
import concourse.tile as tile
from concourse import bacc
from concourse.bass_utils import run_bass_kernel_spmd
from concourse.masks import make_identity
from contextlib import ExitStack

P = 128
N = 2048
D = 2048
KP = 8
HG = 4
HD = 128
HCOLS = HG * HD
SP = N // 512
NCH = N // P
NPAIR = NCH // 2
SCALE = HD ** -0.5
C_BIAS = 9.0
B8 = 3.5
WQS = 64.0

f32 = mybir.dt.float32
f32r = mybir.dt.float32r
f16 = mybir.dt.float16
e4 = mybir.dt.float8e4
DR = mybir.MatmulPerfMode.DoubleRow

_CACHE = {}

EXP = mybir.ActivationFunctionType.Exp
ADD = mybir.AluOpType.add
SUB = mybir.AluOpType.subtract
MUL = mybir.AluOpType.mult


def build_nc():
    nc = bacc.Bacc("TRN2", target_bir_lowering=False, debug=False)
    xh = nc.dram_tensor("xh", [P, SP, KP, 2, 512], e4, kind="ExternalInput")
    xl = nc.dram_tensor("xl", [P, SP, KP, 2, 512], e4, kind="ExternalInput")
    wqh = nc.dram_tensor("wqh", [P, KP, 2, HCOLS], e4, kind="ExternalInput")
    wql = nc.dram_tensor("wql", [P, KP, 2, HCOLS], e4, kind="ExternalInput")
    wo8h = nc.dram_tensor("wo8h", [P, HG, D], e4, kind="ExternalInput")
    wo8l = nc.dram_tensor("wo8l", [P, HG, D], e4, kind="ExternalInput")
    out = nc.dram_tensor("out", [N, D], f16, kind="ExternalOutput")
    out3 = out.rearrange("(a p) n -> p a n", p=P)
    outA = nc.dram_tensor("outA", [N, D], f16, kind="ExternalOutput")
    outA3 = outA.rearrange("(a p) n -> p a n", p=P)

    EB8 = float(np.exp(-B8))          # masked-diag residue per row
    EB9 = float(np.exp(B8 - C_BIAS))  # es8 scale -> e^-9 scale

    with (
        nc.allow_low_precision(reason="fp8/fp16 dataflow is intentional"),
        tile.TileContext(nc) as tc,
        ExitStack() as ctx,
    ):
        const_pool = ctx.enter_context(tc.tile_pool(name="const", bufs=1))
        qt8_pool = ctx.enter_context(tc.tile_pool(name="qt8", bufs=1))
        qn8_pool = ctx.enter_context(tc.tile_pool(name="qn8", bufs=1))
        q16_pool = ctx.enter_context(tc.tile_pool(name="q16f", bufs=4))
        es_pool = ctx.enter_context(tc.tile_pool(name="es8", bufs=16))
        rr_pool = ctx.enter_context(tc.tile_pool(name="rr", bufs=4))
        tmp_pool = ctx.enter_context(tc.tile_pool(name="tmp", bufs=3))
        snn_pool = ctx.enter_context(tc.tile_pool(name="snn", bufs=4))
        rbc_pool = ctx.enter_context(tc.tile_pool(name="rbc", bufs=4))
        yt_pool = ctx.enter_context(tc.tile_pool(name="yt", bufs=2))
        ynm_pool = ctx.enter_context(tc.tile_pool(name="ynm", bufs=4))
        q8st_pool = ctx.enter_context(tc.tile_pool(name="q8st", bufs=2))
        qn8st_pool = ctx.enter_context(tc.tile_pool(name="qn8st", bufs=2))
        ps_s = ctx.enter_context(tc.tile_pool(name="ps_s", bufs=2, space="PSUM"))
        ps_b = ctx.enter_context(tc.tile_pool(name="ps_b", bufs=4, space="PSUM"))

        idr = const_pool.tile([P, P], f32r, tag="idr")
        id16 = const_pool.tile([P, P], f16, tag="id16")
        # DR-layout fp8 identities: ia8p^T @ ia8n = -4096*I (diag canceller)
        ia8p = const_pool.tile([64, 2, P], e4, tag="ia8p")
        ia8n = const_pool.tile([64, 2, P], e4, tag="ia8n")
        nb8 = const_pool.tile([P, 1], f32, tag="nb8")
        nb9 = const_pool.tile([P, 1], f32, tag="nb9")
        ones1 = const_pool.tile([P, 1], f16, tag="ones1")

        # qt8[p, r, c, n] : 4*Q^T head c, DR layout (hd = 2p+r-ish)
        qt8 = qt8_pool.tile([64, 2, HG, N], e4, tag="qt8")
        # qn8[p, t, k, c, :] = Q[(2t+k)*P+p, head c, :]/16
        # (128-multiple strides: fp8 Ldweights restriction)
        qn8 = qn8_pool.tile([P, NPAIR, 2, HG, P], e4, tag="qn8")
        # fp8 ones for the mirror r-column matmuls
        on8 = const_pool.tile([P, 2, 1], e4, tag="on8")
        # q16full[c] = 4*Q^T head c, f16, persistent (diag path + src of qt8/qn8)
        q16full = {}
        for c in range(HG):
            q16full[c] = q16_pool.tile([P, N], f16, tag="q16f", name=f"q16f{c}")

        yts = {}
        rrechs = {}
        rmirs = {}
        snns = {}
        ess = {}       # ess[c][t] = es8 pair tile [P, 2, N]

        def new_head_state(c):
            rrechs[c] = rr_pool.tile([P, NCH, 3], f32, tag="rrech", name=f"rr{c}")
            rmirs[c] = rr_pool.tile([P, NCH], f32, tag="rmir", name=f"rm{c}")
            nc.vector.memset(rrechs[c][:], 0.0)
            nc.vector.memset(rmirs[c][:], 0.0)
            yts[c] = yt_pool.tile([P, N], f16, tag="yt", name=f"yt{c}")
            nc.gpsimd.memset(yts[c][:, 0:P], 0.0)
            ess[c] = {}

        def scores_piece(c, v, h, es, k, lo=None, hi=None, slot=None):
            # scores cols [max(128v, 1024h), 1024(h+1)) for row-chunk v
            if lo is None:
                lo = max(P * v, 1024 * h)
            if hi is None:
                hi = 1024 * (h + 1)
            if slot is None:
                slot = h
            if lo >= hi:
                return
            ps = ps_s.tile([P, 1024], f32, tag="s")
            col = lo
            while col < hi:
                w = min(512 - col % 512, hi - col)
                nc.tensor.matmul(
                    ps[:, col - 1024 * h:col - 1024 * h + w],
                    qt8[:, :, c, v * P:(v + 1) * P],
                    qt8[:, :, c, col:col + w],
                    start=True,
                    stop=True,
                    perf_mode=DR,
                )
                col += w
            if lo == v * P:
                # diag block leads this piece: add -4096*I so exp of the
                # exact diagonal flushes to 0 in fp8 (residue ~1e-4, ignored)
                nc.tensor.matmul(
                    ps[:, lo - 1024 * h:lo - 1024 * h + P],
                    ia8p[:],
                    ia8n[:],
                    start=False,
                    stop=True,
                    perf_mode=DR,
                )
            nc.scalar.activation(
                es[:, k, lo:hi],
                ps[:, lo - 1024 * h:hi - 1024 * h],
                EXP,
                bias=nb8[:, 0:1],
                scale=SCALE / 16.0,
                accum_out=rrechs[c][:, v, slot:slot + 1],
            )

        def scores_row(c, v):
            t, k = v // 2, v % 2
            if k == 0:
                ess[c][t] = es_pool.tile([P, 2, N], e4, tag="es", name=f"e{c}_{t}")
            es = ess[c][t]
            scores_piece(c, v, 0, es, k)
            scores_piece(c, v, 1, es, k)

        def mirror_units(c, us):
            # Y[u-chunk, :] += sum_{v<u} E8[v, u-block]^T @ [Q_v/16 | 1]
            # us: 1-2 consecutive chunk indices sharing one PSUM tile
            nu = len(us)
            w = 128 * nu
            pm = ps_b.tile([P, w + nu], f32, tag="b", name=f"pm{c}_{us[0]}")
            for j, u in enumerate(us):
                npairs = u // 2
                tail = [(t, None) for t in range(npairs)]
                if u % 2 == 1:
                    tail.append((npairs, 0))
                for i, (t, k) in enumerate(tail):
                    st = (i == 0)
                    sp = (i == len(tail) - 1)
                    if k is None:
                        nc.tensor.matmul(
                            pm[:, j * P:(j + 1) * P],
                            ess[c][t][:, :, u * P:(u + 1) * P],
                            qn8[:, t, :, c, :],
                            start=st, stop=sp, perf_mode=DR,
                        )
                        nc.tensor.matmul(
                            pm[:, w + j:w + j + 1],
                            ess[c][t][:, :, u * P:(u + 1) * P],
                            on8[:],
                            start=st, stop=sp, perf_mode=DR,
                        )
                    else:
                        nc.tensor.matmul(
                            pm[:, j * P:(j + 1) * P],
                            ess[c][t][:, k, u * P:(u + 1) * P],
                            qn8[:, t, k, c, :],
                            start=st, stop=sp,
                        )
                        nc.tensor.matmul(
                            pm[:, w + j:w + j + 1],
                            ess[c][t][:, k, u * P:(u + 1) * P],
                            on8[:, 0],
                            start=st, stop=sp,
                        )
            ynm = ynm_pool.tile([P, w], f16, tag="ynm")
            nc.vector.tensor_copy(ynm[:], pm[:, 0:w])
            nc.vector.tensor_copy(rmirs[c][:, us[0]:us[0] + nu], pm[:, w:w + nu])
            ptm = ps_b.tile([P, w], f16, tag="b", name=f"ptm{c}_{us[0]}")
            for j in range(nu):
                nc.tensor.transpose(
                    ptm[:, j * P:(j + 1) * P], ynm[:, j * P:(j + 1) * P], id16[:]
                )
            nc.vector.tensor_copy(yts[c][:, us[0] * P:(us[0] + nu) * P], ptm[:])

        def snn_unit(c, t):
            # S_nn for chunks 4t..4t+3 of head c: colsums of (q16)^2 blocks
            qsq = tmp_pool.tile([P, 512], f16, tag="qsq")
            nc.vector.tensor_tensor(
                qsq[:], q16full[c][:, t * 512:(t + 1) * 512],
                q16full[c][:, t * 512:(t + 1) * 512], MUL,
            )
            pq = ps_b.tile([P, 4], f32, tag="b", name=f"pq{c}_{t}")
            for j in range(4):
                nc.tensor.matmul(
                    pq[:, j:j + 1],
                    qsq[:, j * P:(j + 1) * P],
                    ones1[:],
                    start=True, stop=True,
                )
            nc.vector.tensor_copy(snns[c][:, t * 4:(t + 1) * 4], pq[:])

        def r_chain(c):
            # r9 = (sum_h rrech + rmir - e^-B8) * e^(B8-9) + Enn9   [P, NCH]
            enn = tmp_pool.tile([P, NCH], f32, tag="enn")
            nc.scalar.activation(
                enn[:], snns[c][:], EXP, bias=nb9[:, 0:1], scale=SCALE / 16.0,
            )
            rr = tmp_pool.tile([P, NCH], f32, tag="rrec")
            nc.vector.tensor_reduce(
                rr[:], rrechs[c][:], mybir.AxisListType.X, ADD
            )
            nc.vector.tensor_tensor(rr[:], rr[:], rmirs[c][:], ADD)
            nc.vector.tensor_scalar_mul(rr[:], rr[:], EB9)
            nc.vector.tensor_tensor(rr[:], rr[:], enn[:], ADD)
            rcp = tmp_pool.tile([P, NCH], f32r, tag="rcp")
            nc.vector.reciprocal(rcp[:], rr[:])
            # broadcast vectors: rbc = 16*e^(B8-9)/r9 ; zbc = Enn9/(4*r9)
            # (evac: yt = (psy + yt_mirror)*rbc + q16*zbc ; zbc <= 0.25)
            offv = tmp_pool.tile([P, 2, NCH], f32r, tag="offv")
            nc.vector.tensor_scalar_mul(offv[:, 0], rcp[:], 16.0 * EB9)
            nc.vector.tensor_tensor(offv[:, 1], rcp[:], enn[:], MUL)
            nc.vector.tensor_scalar_mul(offv[:, 1], offv[:, 1], 0.25)
            pt = ps_b.tile([NCH, 2 * P], f32r, tag="b", name=f"rt{c}")
            nc.tensor.transpose(pt[:, 0:P], offv[:, 0], idr[:])
            nc.tensor.transpose(pt[:, P:2 * P], offv[:, 1], idr[:])
            both16 = tmp_pool.tile([NCH, 2 * P], f16, tag="both16")
            nc.vector.tensor_copy(both16[:], pt[:])
            rbc = rbc_pool.tile([P, N], f16, tag="rbc", name=f"rb{c}")
            zbc = rbc_pool.tile([P, N], f16, tag="zbc", name=f"zb{c}")
            nc.sync.dma_start(rbc[0:1, :], both16[:, 0:P])
            nc.sync.dma_start(zbc[0:1, :], both16[:, P:2 * P])
            nc.gpsimd.partition_broadcast(rbc[:], rbc[0:1, :])
            nc.gpsimd.partition_broadcast(zbc[:], zbc[0:1, :])
            return rbc, zbc

        with (
            tc.tile_pool(name="xt", bufs=4) as xt_pool,
            tc.tile_pool(name="wq", bufs=1) as wq_pool,
        ):
            for c in range(HG):
                snns[c] = snn_pool.tile([P, NCH], f32, tag="snn", name=f"sn{c}")

            wqh_sb = wq_pool.tile([P, KP, 2, HCOLS], e4, tag="wqh")
            wql_sb = wq_pool.tile([P, KP, 2, HCOLS], e4, tag="wql")
            xsp = {}

            def load_x(t, engs):
                xsp[t] = (
                    xt_pool.tile([P, KP, 2, 512], e4, tag="xt", name=f"xh{t}"),
                    xt_pool.tile([P, KP, 2, 512], e4, tag="xt", name=f"xl{t}"),
                )
                engs[0].dma_start(xsp[t][0][:], xh[:, t])
                engs[1].dma_start(xsp[t][1][:], xl[:, t])

            # prologue loads: fine-grained halves so qproj starts early
            xsp[0] = (
                xt_pool.tile([P, KP, 2, 512], e4, tag="xt", name="xh0"),
                xt_pool.tile([P, KP, 2, 512], e4, tag="xt", name="xl0"),
            )
            nc.sync.dma_start(wqh_sb[:, 0:2], wqh[:, 0:2])
            nc.scalar.dma_start(xsp[0][0][:, 0:2], xh[:, 0, 0:2])
            nc.gpsimd.dma_start(wql_sb[:, 0:4], wql[:, 0:4])
            nc.sync.dma_start(wqh_sb[:, 2:5], wqh[:, 2:5])
            nc.scalar.dma_start(xsp[0][0][:, 2:5], xh[:, 0, 2:5])
            nc.gpsimd.dma_start(xsp[0][1][:, 0:2], xl[:, 0, 0:2])
            nc.sync.dma_start(wqh_sb[:, 5:8], wqh[:, 5:8])
            nc.scalar.dma_start(xsp[0][0][:, 5:8], xh[:, 0, 5:8])
            nc.gpsimd.dma_start(xsp[0][1][:, 2:5], xl[:, 0, 2:5])
            nc.sync.dma_start(wql_sb[:, 4:8], wql[:, 4:8])
            nc.gpsimd.dma_start(xsp[0][1][:, 5:8], xl[:, 0, 5:8])
            load_x(1, (nc.scalar, nc.sync))
            nc.gpsimd.memset(nb8[:], -B8)
            nc.gpsimd.memset(nb9[:], -C_BIAS)
            nc.gpsimd.memset(ones1[:], 1.0)
            nc.gpsimd.memset(on8[:], 1.0)
            edum = rr_pool.tile([P, 1], f32, tag="edum")
            nc.scalar.activation(edum[:], nb8[:, 0:1], EXP)
            make_identity(nc, id16[:])
            nc.vector.tensor_copy(idr[:], id16[:])
            i8st = q8st_pool.tile([P, P], e4, tag="q8s", name="i8p")
            nc.vector.tensor_scalar_mul(i8st[:], id16[:], 64.0)
            nc.sync.dma_start(ia8p[:], i8st[:])
            i8st2 = q8st_pool.tile([P, P], e4, tag="q8s", name="i8n")
            nc.vector.tensor_scalar_mul(i8st2[:], id16[:], -64.0)
            nc.sync.dma_start(ia8n[:], i8st2[:])

            pend_tr = []
            late_tr = []

            def qproj_unit(c, t):
                ps = ps_b.tile([P, 512], f32, tag="b")
                th, tl = xsp[t]
                combos = [(wqh_sb, th, kp) for kp in range(KP)]
                combos += [(wql_sb, th, kp) for kp in range(KP)]
                combos += [(wqh_sb, tl, kp) for kp in range(KP)]
                for i, (w, x, kp) in enumerate(combos):
                    nc.tensor.matmul(
                        ps[:],
                        w[:, kp, :, c * P:(c + 1) * P],
                        x[:, kp],
                        start=(i == 0),
                        stop=(i == len(combos) - 1),
                        perf_mode=DR,
                    )
                # q16full stores 4*Q^T (f16)
                nc.vector.tensor_scalar_mul(
                    q16full[c][:, t * 512:(t + 1) * 512], ps[:], 1.0 / 16.0
                )
                # qt8 = e4m3(4*Q^T), staged then DR-repacked via DMA
                q8s = q8st_pool.tile([P, 512], e4, tag="q8s")
                nc.gpsimd.tensor_copy(q8s[:], q16full[c][:, t * 512:(t + 1) * 512])
                nc.sync.dma_start(qt8[:, :, c, t * 512:(t + 1) * 512], q8s[:])
                if pend_tr:
                    pend_tr.pop(0)()

                def transposes(c=c, t=t):
                    # Q natural fp8 (Q/16) into paired DR layout
                    pt_all = ps_b.tile([P, 512], f16, tag="b")
                    for j in range(4):
                        nc.tensor.transpose(
                            pt_all[:, j * P:(j + 1) * P],
                            q16full[c][:, (t * 4 + j) * P:(t * 4 + j + 1) * P],
                            id16[:],
                        )
                    qn8s = qn8st_pool.tile([P, 512], e4, tag="qn8s")
                    nc.vector.tensor_scalar_mul(qn8s[:], pt_all[:], 1.0 / 64.0)
                    nc.sync.dma_start(
                        qn8[:, 2 * t:2 * t + 2, :, c, :], qn8s[:]
                    )
                    snn_unit(c, t)

                if c < 2:
                    pend_tr.append(transposes)
                else:
                    late_tr.append(transposes)

            # Boot interleave: stream head-0 and head-1 score pieces (feeding
            # the ACT exp pipeline) between qproj units so ACT never starves.
            sc = []
            sc1 = []

            def emit(n):
                for _ in range(n):
                    if sc:
                        sc.pop(0)()
                    elif sc1:
                        sc1.pop(0)()

            def ph_piece(c, v, h, **kw):
                t, k = v // 2, v % 2
                if k == 0 and t not in ess[c]:
                    ess[c][t] = es_pool.tile(
                        [P, 2, N], e4, tag="es", name=f"e{c}_{t}"
                    )
                scores_piece(c, v, h, ess[c][t], k, **kw)

            qproj_unit(0, 0)
            new_head_state(0)
            # rows 0-3 cols [vP,512): only needs the (0,0) qt8 fold -> ACT
            # exp stream starts while qproj is still warming up
            for v in range(4):
                sc.append(lambda v=v: ph_piece(0, v, 0, hi=512, slot=2))
            qproj_unit(1, 0)
            emit(1)
            qproj_unit(2, 0)
            emit(1)
            qproj_unit(3, 0)
            emit(2)
            load_x(2, (nc.sync, nc.sync))
            qproj_unit(0, 1)
            for v in range(8):
                sc.append(lambda v=v: ph_piece(0, v, 0, lo=max(v * P, 512)))
            qproj_unit(1, 1)
            new_head_state(1)
            for v in range(8):
                sc.append(lambda v=v: ph_piece(1, v, 0))
            emit(2)
            qproj_unit(2, 1)
            emit(2)
            qproj_unit(3, 1)
            emit(2)
            load_x(3, (nc.sync, nc.sync))
            qproj_unit(0, 2)
            emit(2)
            qproj_unit(1, 2)
            emit(2)
            qproj_unit(0, 3)
            # head-0 phase 2 (qt8[0] complete)
            for v in range(NCH):
                sc.append(lambda v=v: ph_piece(0, v, 1))
                if v >= 4 and v % 2 == 0:
                    sc.append(lambda v=v: mirror_units(0, [v - 3, v - 2]))
            sc.append(lambda: mirror_units(0, [NCH - 3, NCH - 2]))
            sc.append(lambda: mirror_units(0, [NCH - 1]))
            qproj_unit(1, 3)
            # head 1 (qt8[1] complete)
            for v in range(NCH):
                if v >= 8:
                    sc1.append(lambda v=v: ph_piece(1, v, 0))
                sc1.append(lambda v=v: ph_piece(1, v, 1))
                if v >= 4 and v % 2 == 0:
                    sc1.append(lambda v=v: mirror_units(1, [v - 3, v - 2]))
            sc1.append(lambda: mirror_units(1, [NCH - 3, NCH - 2]))
            sc1.append(lambda: mirror_units(1, [NCH - 1]))
            qproj_unit(2, 2)
            emit(4)
            qproj_unit(2, 3)
            emit(4)
            qproj_unit(3, 2)
            emit(4)
            qproj_unit(3, 3)
            while pend_tr:
                pend_tr.pop(0)()

        with (
            tc.tile_pool(name="wo8", bufs=1) as wo8_pool,
            tc.tile_pool(name="y8", bufs=1) as y8p,
            tc.tile_pool(name="osb", bufs=3) as o_pool,
        ):
            wo8h_sb = wo8_pool.tile([P, HG, D], e4, tag="wo8h")
            wo8l_sb = wo8_pool.tile([P, HG, D], e4, tag="wo8l")
            nc.sync.dma_start(wo8h_sb[:], wo8h[:])
            nc.gpsimd.dma_start(wo8l_sb[:], wo8l[:])
            y8h = y8p.tile([P, HG, N], e4, tag="y8h")
            y8l = y8p.tile([P, HG, N], e4, tag="y8l")
            rbz = {}

            def outproj_unit(a, grp):
                # grp 0: heads 0-1 partial -> outA ; grp 1: heads 2-3 -> out
                # grp 2: all heads (combined) -> out
                dst = outA3 if grp == 0 else out3
                ot = o_pool.tile([P, D], f16, tag="ot")
                for d4 in range(SP):
                    pool = ps_s if d4 % 2 else ps_b
                    ps = pool.tile([P, 512], f32, tag="s" if d4 % 2 else "b")
                    combos = []
                    for cc in ((0,), (2,), (0, 2))[grp]:
                        combos += [
                            (y8h, wo8h_sb, cc),
                            (y8h, wo8l_sb, cc),
                            (y8l, wo8h_sb, cc),
                        ]
                    for i, (yy, ww, cc) in enumerate(combos):
                        nc.tensor.matmul(
                            ps[:],
                            yy[:, cc:cc + 2, a * P:(a + 1) * P],
                            ww[:, cc:cc + 2, d4 * 512:(d4 + 1) * 512],
                            start=(i == 0),
                            stop=(i == len(combos) - 1),
                            perf_mode=DR,
                        )
                    if d4 % 2 == 0:
                        nc.vector.tensor_scalar_mul(
                            ot[:, d4 * 512:(d4 + 1) * 512], ps[:], 1.0 / 64.0
                        )
                    else:
                        nc.scalar.mul(
                            ot[:, d4 * 512:(d4 + 1) * 512], ps[:], 1.0 / 64.0
                        )
                        # stream each 1KB half out as soon as it's ready
                        half = d4 // 2
                        eng = nc.sync
                        eng.dma_start(
                            dst[:, a, half * 1024:(half + 1) * 1024],
                            ot[:, half * 1024:(half + 1) * 1024],
                        )

            def pv_mm(pc, s):
                # one span of YT via paired DR matmuls
                psy = ps_b.tile([P, 512], f32, tag="b", name=f"psy{pc}_{s}")
                lo_s = s * 512
                mms = []
                for t in range(2 * s):
                    mms.append((t, None, lo_s))
                mms.append((2 * s, 0, lo_s))                # solo chunk 4s
                mms.append((2 * s, None, lo_s + 128))       # pair (4s,4s+1)
                mms.append((2 * s + 1, 0, lo_s + 256))      # solo chunk 4s+2
                mms.append((2 * s + 1, None, lo_s + 384))   # pair (4s+2,4s+3)
                for i, (t, k, lo) in enumerate(mms):
                    st = (i == 0)
                    sp = (i == len(mms) - 1)
                    hi = (s + 1) * 512
                    if k is None:
                        nc.tensor.matmul(
                            psy[:, lo - lo_s:hi - lo_s],
                            qn8[:, t, :, pc, 0:P],
                            ess[pc][t][:, :, lo:hi],
                            start=st, stop=sp, perf_mode=DR,
                        )
                    else:
                        nc.tensor.matmul(
                            psy[:, lo - lo_s:lo - lo_s + 128],
                            qn8[:, t, k, pc, 0:P],
                            ess[pc][t][:, k, lo:lo + 128],
                            start=st, stop=sp,
                        )
                return psy

            def pv_evac(pc, s, psy, rbc, zbc):
                # yt = ((psy + yt_mirror) + q16*zrbc) * rbc  -> y8 hi/lo
                yt = yts[pc]
                sl = slice(s * 512, (s + 1) * 512)
                nc.vector.tensor_tensor(yt[:, sl], psy[:], yt[:, sl], ADD)
                nc.vector.tensor_tensor(yt[:, sl], yt[:, sl], rbc[:, sl], MUL)
                dg = tmp_pool.tile([P, 512], f16, tag="dg", name=f"dg{pc}_{s}")
                nc.vector.tensor_tensor(
                    dg[:], q16full[pc][:, sl], zbc[:, sl], MUL
                )
                nc.vector.tensor_tensor(yt[:, sl], yt[:, sl], dg[:], ADD)
                nc.gpsimd.tensor_copy(y8h[:, pc, sl], yt[:, sl])
                nc.gpsimd.tensor_tensor(
                    y8l[:, pc, sl], yt[:, sl], y8h[:, pc, sl], SUB
                )

            def pv_span(pc, s, rbc, zbc):
                pv_evac(pc, s, pv_mm(pc, s), rbc, zbc)

            # boot tail: drain head-0 stream, then overlap the head-1 exp
            # tail with PV(0) spans and the r-chains
            while sc:
                sc.pop(0)()
            rbz[0] = r_chain(0)
            s0 = 0
            while sc1:
                for _ in range(6):
                    if sc1:
                        sc1.pop(0)()
                if s0 < SP:
                    pv_span(0, s0, *rbz[0])
                    s0 += 1
            while s0 < SP:
                pv_span(0, s0, *rbz[0])
                s0 += 1
            rbz[1] = r_chain(1)

            # ============ stage 1: scores(2) || PV(1) ============
            # deferred qn8/S_nn prep for heads 2-3 (PE idle under exp here)
            while late_tr:
                late_tr.pop(0)()
            new_head_state(2)
            for v in range(NCH):
                scores_row(2, v)
                if v >= 4 and v % 2 == 0:
                    mirror_units(2, [v - 3, v - 2])
                    if v >= 8:
                        pv_span(1, (v - 8) // 2, *rbz[1])
            mirror_units(2, [NCH - 3, NCH - 2])
            mirror_units(2, [NCH - 1])
            rbz[2] = r_chain(2)

            # ===== stage 2: scores(3) || PV(2) + outproj-A (heads 0-1) =====
            new_head_state(3)
            for v in range(NCH):
                scores_row(3, v)
                if v >= 4 and v % 2 == 0:
                    mirror_units(3, [v - 3, v - 2])
                if v % 4 == 2:
                    pv_span(2, v // 4, *rbz[2])
            mirror_units(3, [NCH - 3, NCH - 2])
            mirror_units(3, [NCH - 1])
            rbz[3] = r_chain(3)

            # lull fill: heads-0/1 partials are independent of r_chain(3)
            for a in range(8):
                outproj_unit(a, 0)

            # == stage 3: PV(3) + outproj (B for a<6, combined for a>=6) ==
            psys = {0: pv_mm(3, 0), 1: pv_mm(3, 1)}
            for s in range(SP):
                pv_evac(3, s, psys[s], *rbz[3])
                if s + 2 < SP:
                    psys[s + 2] = pv_mm(3, s + 2)
                for a in range(4 * s, 4 * s + 4):
                    outproj_unit(a, 1 if a < 8 else 2)

    nc.compile()
    return nc


def _hi_lo(arr):
    hi = arr.astype(ml_dtypes.float8_e4m3)
    lo = (arr - hi.astype(np.float32)).astype(ml_dtypes.float8_e4m3)
    return hi, lo


def _pack_x(xt_hl):
    return np.ascontiguousarray(
        xt_hl.reshape(KP, 2, P, SP, 512).transpose(2, 3, 0, 1, 4)
    )


def kernel(x, Wq, Wo, bo):
    x = np.asarray(x, dtype=np.float32)
    Wq = np.asarray(Wq, dtype=np.float32)
    Wo = np.asarray(Wo, dtype=np.float32)
    bo = np.asarray(bo, dtype=np.float32)
    B = x.shape[0]
    assert B == 2 and x.shape == (B, N, D)
    assert Wq.shape == (D, D) and Wo.shape == (D, D)

    if "nc" not in _CACHE:
        _CACHE["nc"] = build_nc()
    nc = _CACHE["nc"]

    packed_x = []
    for b in range(B):
        hi, lo = _hi_lo(np.ascontiguousarray(x[b].T))
        packed_x.append((_pack_x(hi), _pack_x(lo)))
    in_maps = []
    for core in range(8):
        b, hg = core // 4, core % 4
        wq_s = WQS * Wq[:, hg * HCOLS:(hg + 1) * HCOLS]
        wqh, wql = _hi_lo(np.ascontiguousarray(wq_s))
        wo8h_a, wo8l_a = _hi_lo(
            np.ascontiguousarray(WQS * Wo[hg * HCOLS:(hg + 1) * HCOLS, :])
        )
        in_maps.append(
            {
                "xh": packed_x[b][0],
                "xl": packed_x[b][1],
                "wqh": np.ascontiguousarray(
                    wqh.reshape(KP, 2, P, HCOLS).transpose(2, 0, 1, 3)
                ),
                "wql": np.ascontiguousarray(
                    wql.reshape(KP, 2, P, HCOLS).transpose(2, 0, 1, 3)
                ),
                "wo8h": np.ascontiguousarray(
                    wo8h_a.reshape(HG, P, D).transpose(1, 0, 2)
                ),
                "wo8l": np.ascontiguousarray(
                    wo8l_a.reshape(HG, P, D).transpose(1, 0, 2)
                ),
            }
        )

    res = run_bass_kernel_spmd(nc, in_maps, list(range(8)))
    _CACHE["last_res"] = res
    out = np.zeros((B, N, D), dtype=np.float32)
    for core in range(8):
        b = core // 4
        out[b] += res.results[core]["out"].astype(np.float32)
        out[b][0:1024] += res.results[core]["outA"][0:1024].astype(np.float32)
    out += bo
    return out


# revision 9
# speedup vs baseline: 1.0953x; 1.0014x over previous
"""Trainium2 Bass kernel for nn_MultiHeadAttention_78864189489198 — v3.

fp8 paired-es variant.

E = exp(S - B8) is stored in fp8 (e4m3) chunk-PAIRED tiles es8[t] of
shape [P, 2, N] holding row-chunks (2t, 2t+1).  This lets both the PV
(attn @ V) and mirror matmuls run in DoubleRow perf mode (contraction
256 = two chunks at once, 0.5 cyc/col), cutting their PE cost ~4x/3.3x
vs the fp16 v2 path.

The huge diagonal E_nn (softmax is diagonally dominant here) cannot
live in fp8: the scores diagonal is masked to 0 in PSUM before exp
(DVE multiply by (1-I)), and the diagonal contribution is restored
exactly:  S_nn = rowsum(q16^2) via per-chunk ones-matmuls,
E_nn = exp(S_nn - 9), Y += (E_nn / r) * Q with r assembled from the
exp accumulators + mirror ones-column + E_nn (consistent numerator /
denominator, so S_nn quantization error cancels).

Everything else as v2: fp8 hi/lo DoubleRow qproj/outproj, host-side
batch x head-group sharding (8 cores), fp16 partial outputs summed on
host.
"""

import numpy as np
import ml_dtypes

import concourse.bass as bass
import concourse.mybir as mybir
import concourse.tile as tile
from concourse import bacc
from concourse.bass_utils import run_bass_kernel_spmd
from concourse.masks import make_identity
from contextlib import ExitStack

P = 128
N = 2048
D = 2048
KP = 8
HG = 4
HD = 128
HCOLS = HG * HD
SP = N // 512
NCH = N // P
NPAIR = NCH // 2
SCALE = HD ** -0.5
C_BIAS = 9.0
B8 = 3.5
WQS = 64.0

f32 = mybir.dt.float32
f32r = mybir.dt.float32r
f16 = mybir.dt.float16
e4 = mybir.dt.float8e4
DR = mybir.MatmulPerfMode.DoubleRow

_CACHE = {}

EXP = mybir.ActivationFunctionType.Exp
ADD = mybir.AluOpType.add
SUB = mybir.AluOpType.subtract
MUL = mybir.AluOpType.mult


def build_nc():
    nc = bacc.Bacc("TRN2", target_bir_lowering=False, debug=False)
    xh = nc.dram_tensor("xh", [P, SP, KP, 2, 512], e4, kind="ExternalInput")
    xl = nc.dram_tensor("xl", [P, SP, KP, 2, 512], e4, kind="ExternalInput")
    wqh = nc.dram_tensor("wqh", [P, KP, 2, HCOLS], e4, kind="ExternalInput")
    wql = nc.dram_tensor("wql", [P, KP, 2, HCOLS], e4, kind="ExternalInput")
    wo8h = nc.dram_tensor("wo8h", [P, HG, D], e4, kind="ExternalInput")
    wo8l = nc.dram_tensor("wo8l", [P, HG, D], e4, kind="ExternalInput")
    out = nc.dram_tensor("out", [N, D], f16, kind="ExternalOutput")
    out3 = out.rearrange("(a p) n -> p a n", p=P)
    outA = nc.dram_tensor("outA", [N, D], f16, kind="ExternalOutput")
    outA3 = outA.rearrange("(a p) n -> p a n", p=P)

    EB8 = float(np.exp(-B8))          # masked-diag residue per row
    EB9 = float(np.exp(B8 - C_BIAS))  # es8 scale -> e^-9 scale

    with (
        nc.allow_low_precision(reason="fp8/fp16 dataflow is intentional"),
        tile.TileContext(nc) as tc,
        ExitStack() as ctx,
    ):
        const_pool = ctx.enter_context(tc.tile_pool(name="const", bufs=1))
        qt8_pool = ctx.enter_context(tc.tile_pool(name="qt8", bufs=1))
        qn8_pool = ctx.enter_context(tc.tile_pool(name="qn8", bufs=1))
        q16_pool = ctx.enter_context(tc.tile_pool(name="q16f", bufs=4))
        es_pool = ctx.enter_context(tc.tile_pool(name="es8", bufs=16))
        rr_pool = ctx.enter_context(tc.tile_pool(name="rr", bufs=4))
        tmp_pool = ctx.enter_context(tc.tile_pool(name="tmp", bufs=3))
        snn_pool = ctx.enter_context(tc.tile_pool(name="snn", bufs=4))
        rbc_pool = ctx.enter_context(tc.tile_pool(name="rbc", bufs=4))
        yt_pool = ctx.enter_context(tc.tile_pool(name="yt", bufs=2))
        ynm_pool = ctx.enter_context(tc.tile_pool(name="ynm", bufs=4))
        q8st_pool = ctx.enter_context(tc.tile_pool(name="q8st", bufs=2))
        qn8st_pool = ctx.enter_context(tc.tile_pool(name="qn8st", bufs=2))
        ps_s = ctx.enter_context(tc.tile_pool(name="ps_s", bufs=2, space="PSUM"))
        ps_b = ctx.enter_context(tc.tile_pool(name="ps_b", bufs=4, space="PSUM"))

        idr = const_pool.tile([P, P], f32r, tag="idr")
        id16 = const_pool.tile([P, P], f16, tag="id16")
        # DR-layout fp8 identities: ia8p^T @ ia8n = -4096*I (diag canceller)
        ia8p = const_pool.tile([64, 2, P], e4, tag="ia8p")
        ia8n = const_pool.tile([64, 2, P], e4, tag="ia8n")
        nb8 = const_pool.tile([P, 1], f32, tag="nb8")
        nb9 = const_pool.tile([P, 1], f32, tag="nb9")
        ones1 = const_pool.tile([P, 1], f16, tag="ones1")

        # qt8[p, r, c, n] : 4*Q^T head c, DR layout (hd = 2p+r-ish)
        qt8 = qt8_pool.tile([64, 2, HG, N], e4, tag="qt8")
        # qn8[p, t, k, c, :] = Q[(2t+k)*P+p, head c, :]/16
        # (128-multiple strides: fp8 Ldweights restriction)
        qn8 = qn8_pool.tile([P, NPAIR, 2, HG, P], e4, tag="qn8")
        # fp8 ones for the mirror r-column matmuls
        on8 = const_pool.tile([P, 2, 1], e4, tag="on8")
        # q16full[c] = 4*Q^T head c, f16, persistent (diag path + src of qt8/qn8)
        q16full = {}
        for c in range(HG):
            q16full[c] = q16_pool.tile([P, N], f16, tag="q16f", name=f"q16f{c}")

        yts = {}
        rrechs = {}
        rmirs = {}
        snns = {}
        ess = {}       # ess[c][t] = es8 pair tile [P, 2, N]

        def new_head_state(c):
            rrechs[c] = rr_pool.tile([P, NCH, 3], f32, tag="rrech", name=f"rr{c}")
            rmirs[c] = rr_pool.tile([P, NCH], f32, tag="rmir", name=f"rm{c}")
            nc.vector.memset(rrechs[c][:], 0.0)
            nc.vector.memset(rmirs[c][:], 0.0)
            yts[c] = yt_pool.tile([P, N], f16, tag="yt", name=f"yt{c}")
            nc.gpsimd.memset(yts[c][:, 0:P], 0.0)
            ess[c] = {}

        def scores_piece(c, v, h, es, k, lo=None, hi=None, slot=None):
            # scores cols [max(128v, 1024h), 1024(h+1)) for row-chunk v
            if lo is None:
                lo = max(P * v, 1024 * h)
            if hi is None:
                hi = 1024 * (h + 1)
            if slot is None:
                slot = h
            if lo >= hi:
                return
            ps = ps_s.tile([P, 1024], f32, tag="s")
            col = lo
            while col < hi:
                w = min(512 - col % 512, hi - col)
                nc.tensor.matmul(
                    ps[:, col - 1024 * h:col - 1024 * h + w],
                    qt8[:, :, c, v * P:(v + 1) * P],
                    qt8[:, :, c, col:col + w],
                    start=True,
                    stop=True,
                    perf_mode=DR,
                )
                col += w
            if lo == v * P:
                # diag block leads this piece: add -4096*I so exp of the
                # exact diagonal flushes to 0 in fp8 (residue ~1e-4, ignored)
                nc.tensor.matmul(
                    ps[:, lo - 1024 * h:lo - 1024 * h + P],
                    ia8p[:],
                    ia8n[:],
                    start=False,
                    stop=True,
                    perf_mode=DR,
                )
            nc.scalar.activation(
                es[:, k, lo:hi],
                ps[:, lo - 1024 * h:hi - 1024 * h],
                EXP,
                bias=nb8[:, 0:1],
                scale=SCALE / 16.0,
                accum_out=rrechs[c][:, v, slot:slot + 1],
            )

        def scores_row(c, v):
            t, k = v // 2, v % 2
            if k == 0:
                ess[c][t] = es_pool.tile([P, 2, N], e4, tag="es", name=f"e{c}_{t}")
            es = ess[c][t]
            scores_piece(c, v, 0, es, k)
            scores_piece(c, v, 1, es, k)

        def mirror_units(c, us):
            # Y[u-chunk, :] += sum_{v<u} E8[v, u-block]^T @ [Q_v/16 | 1]
            # us: 1-2 consecutive chunk indices sharing one PSUM tile
            nu = len(us)
            w = 128 * nu
            pm = ps_b.tile([P, w + nu], f32, tag="b", name=f"pm{c}_{us[0]}")
            for j, u in enumerate(us):
                npairs = u // 2
                tail = [(t, None) for t in range(npairs)]
                if u % 2 == 1:
                    tail.append((npairs, 0))
                for i, (t, k) in enumerate(tail):
                    st = (i == 0)
                    sp = (i == len(tail) - 1)
                    if k is None:
                        nc.tensor.matmul(
                            pm[:, j * P:(j + 1) * P],
                            ess[c][t][:, :, u * P:(u + 1) * P],
                            qn8[:, t, :, c, :],
                            start=st, stop=sp, perf_mode=DR,
                        )
                        nc.tensor.matmul(
                            pm[:, w + j:w + j + 1],
                            ess[c][t][:, :, u * P:(u + 1) * P],
                            on8[:],
                            start=st, stop=sp, perf_mode=DR,
                        )
                    else:
                        nc.tensor.matmul(
                            pm[:, j * P:(j + 1) * P],
                            ess[c][t][:, k, u * P:(u + 1) * P],
                            qn8[:, t, k, c, :],
                            start=st, stop=sp,
                        )
                        nc.tensor.matmul(
                            pm[:, w + j:w + j + 1],
                            ess[c][t][:, k, u * P:(u + 1) * P],
                            on8[:, 0],
                            start=st, stop=sp,
                        )
            ynm = ynm_pool.tile([P, w], f16, tag="ynm")
            nc.vector.tensor_copy(ynm[:], pm[:, 0:w])
            nc.vector.tensor_copy(rmirs[c][:, us[0]:us[0] + nu], pm[:, w:w + nu])
            ptm = ps_b.tile([P, w], f16, tag="b", name=f"ptm{c}_{us[0]}")
            for j in range(nu):
                nc.tensor.transpose(
                    ptm[:, j * P:(j + 1) * P], ynm[:, j * P:(j + 1) * P], id16[:]
                )
            nc.vector.tensor_copy(yts[c][:, us[0] * P:(us[0] + nu) * P], ptm[:])

        def snn_unit(c, t):
            # S_nn for chunks 4t..4t+3 of head c: colsums of (q16)^2 blocks
            qsq = tmp_pool.tile([P, 512], f16, tag="qsq")
            nc.vector.tensor_tensor(
                qsq[:], q16full[c][:, t * 512:(t + 1) * 512],
                q16full[c][:, t * 512:(t + 1) * 512], MUL,
            )
            pq = ps_b.tile([P, 4], f32, tag="b", name=f"pq{c}_{t}")
            for j in range(4):
                nc.tensor.matmul(
                    pq[:, j:j + 1],
                    qsq[:, j * P:(j + 1) * P],
                    ones1[:],
                    start=True, stop=True,
                )
            nc.vector.tensor_copy(snns[c][:, t * 4:(t + 1) * 4], pq[:])

        def r_chain(c):
            # r9 = (sum_h rrech + rmir - e^-B8) * e^(B8-9) + Enn9   [P, NCH]
            enn = tmp_pool.tile([P, NCH], f32, tag="enn")
            nc.scalar.activation(
                enn[:], snns[c][:], EXP, bias=nb9[:, 0:1], scale=SCALE / 16.0,
            )
            rr = tmp_pool.tile([P, NCH], f32, tag="rrec")
            nc.vector.tensor_reduce(
                rr[:], rrechs[c][:], mybir.AxisListType.X, ADD
            )
            nc.vector.tensor_tensor(rr[:], rr[:], rmirs[c][:], ADD)
            nc.vector.tensor_scalar_mul(rr[:], rr[:], EB9)
            nc.vector.tensor_tensor(rr[:], rr[:], enn[:], ADD)
            rcp = tmp_pool.tile([P, NCH], f32r, tag="rcp")
            nc.vector.reciprocal(rcp[:], rr[:])
            # broadcast vectors: rbc = 16*e^(B8-9)/r9 ; zbc = Enn9/(4*r9)
            # (evac: yt = (psy + yt_mirror)*rbc + q16*zbc ; zbc <= 0.25)
            offv = tmp_pool.tile([P, 2, NCH], f32r, tag="offv")
            nc.vector.tensor_scalar_mul(offv[:, 0], rcp[:], 16.0 * EB9)
            nc.vector.tensor_tensor(offv[:, 1], rcp[:], enn[:], MUL)
            nc.vector.tensor_scalar_mul(offv[:, 1], offv[:, 1], 0.25)
            pt = ps_b.tile([NCH, 2 * P], f32r, tag="b", name=f"rt{c}")
            nc.tensor.transpose(pt[:, 0:P], offv[:, 0], idr[:])
            nc.tensor.transpose(pt[:, P:2 * P], offv[:, 1], idr[:])
            both16 = tmp_pool.tile([NCH, 2 * P], f16, tag="both16")
            nc.vector.tensor_copy(both16[:], pt[:])
            rbc = rbc_pool.tile([P, N], f16, tag="rbc", name=f"rb{c}")
            zbc = rbc_pool.tile([P, N], f16, tag="zbc", name=f"zb{c}")
            nc.sync.dma_start(rbc[0:1, :], both16[:, 0:P])
            nc.sync.dma_start(zbc[0:1, :], both16[:, P:2 * P])
            nc.gpsimd.partition_broadcast(rbc[:], rbc[0:1, :])
            nc.gpsimd.partition_broadcast(zbc[:], zbc[0:1, :])
            return rbc, zbc

        with (
            tc.tile_pool(name="xt", bufs=4) as xt_pool,
            tc.tile_pool(name="wq", bufs=1) as wq_pool,
        ):
            for c in range(HG):
                snns[c] = snn_pool.tile([P, NCH], f32, tag="snn", name=f"sn{c}")

            wqh_sb = wq_pool.tile([P, KP, 2, HCOLS], e4, tag="wqh")
            wql_sb = wq_pool.tile([P, KP, 2, HCOLS], e4, tag="wql")
            xsp = {}

            def load_x(t, engs):
                xsp[t] = (
                    xt_pool.tile([P, KP, 2, 512], e4, tag="xt", name=f"xh{t}"),
                    xt_pool.tile([P, KP, 2, 512], e4, tag="xt", name=f"xl{t}"),
                )
                engs[0].dma_start(xsp[t][0][:], xh[:, t])
                engs[1].dma_start(xsp[t][1][:], xl[:, t])

            # prologue loads: fine-grained halves so qproj starts early
            xsp[0] = (
                xt_pool.tile([P, KP, 2, 512], e4, tag="xt", name="xh0"),
                xt_pool.tile([P, KP, 2, 512], e4, tag="xt", name="xl0"),
            )
            nc.sync.dma_start(wqh_sb[:, 0:2], wqh[:, 0:2])
            nc.scalar.dma_start(xsp[0][0][:, 0:2], xh[:, 0, 0:2])
            nc.gpsimd.dma_start(wql_sb[:, 0:4], wql[:, 0:4])
            nc.sync.dma_start(wqh_sb[:, 2:5], wqh[:, 2:5])
            nc.scalar.dma_start(xsp[0][0][:, 2:5], xh[:, 0, 2:5])
            nc.gpsimd.dma_start(xsp[0][1][:, 0:2], xl[:, 0, 0:2])
            nc.sync.dma_start(wqh_sb[:, 5:8], wqh[:, 5:8])
            nc.scalar.dma_start(xsp[0][0][:, 5:8], xh[:, 0, 5:8])
            nc.gpsimd.dma_start(xsp[0][1][:, 2:5], xl[:, 0, 2:5])
            nc.sync.dma_start(wql_sb[:, 4:8], wql[:, 4:8])
            nc.gpsimd.dma_start(xsp[0][1][:, 5:8], xl[:, 0, 5:8])
            load_x(1, (nc.scalar, nc.gpsimd))
            nc.gpsimd.memset(nb8[:], -B8)
            nc.gpsimd.memset(nb9[:], -C_BIAS)
            nc.gpsimd.memset(ones1[:], 1.0)
            nc.gpsimd.memset(on8[:], 1.0)
            edum = rr_pool.tile([P, 1], f32, tag="edum")
            nc.scalar.activation(edum[:], nb8[:, 0:1], EXP)
            make_identity(nc, id16[:])
            nc.vector.tensor_copy(idr[:], id16[:])
            i8st = q8st_pool.tile([P, P], e4, tag="q8s", name="i8p")
            nc.vector.tensor_scalar_mul(i8st[:], id16[:], 64.0)
            nc.sync.dma_start(ia8p[:], i8st[:])
            i8st2 = q8st_pool.tile([P, P], e4, tag="q8s", name="i8n")
            nc.vector.tensor_scalar_mul(i8st2[:], id16[:], -64.0)
            nc.sync.dma_start(ia8n[:], i8st2[:])

            pend_tr = []
            late_tr = []

            def qproj_unit(c, t):
                ps = ps_b.tile([P, 512], f32, tag="b")
                th, tl = xsp[t]
                combos = [(wqh_sb, th, kp) for kp in range(KP)]
                combos += [(wql_sb, th, kp) for kp in range(KP)]
                combos += [(wqh_sb, tl, kp) for kp in range(KP)]
                for i, (w, x, kp) in enumerate(combos):
                    nc.tensor.matmul(
                        ps[:],
                        w[:, kp, :, c * P:(c + 1) * P],
                        x[:, kp],
                        start=(i == 0),
                        stop=(i == len(combos) - 1),
                        perf_mode=DR,
                    )
                # q16full stores 4*Q^T (f16)
                nc.vector.tensor_scalar_mul(
                    q16full[c][:, t * 512:(t + 1) * 512], ps[:], 1.0 / 16.0
                )
                # qt8 = e4m3(4*Q^T), staged then DR-repacked via DMA
                q8s = q8st_pool.tile([P, 512], e4, tag="q8s")
                nc.gpsimd.tensor_copy(q8s[:], q16full[c][:, t * 512:(t + 1) * 512])
                nc.sync.dma_start(qt8[:, :, c, t * 512:(t + 1) * 512], q8s[:])
                if pend_tr:
                    pend_tr.pop(0)()

                def transposes(c=c, t=t):
                    # Q natural fp8 (Q/16) into paired DR layout
                    pt_all = ps_b.tile([P, 512], f16, tag="b")
                    for j in range(4):
                        nc.tensor.transpose(
                            pt_all[:, j * P:(j + 1) * P],
                            q16full[c][:, (t * 4 + j) * P:(t * 4 + j + 1) * P],
                            id16[:],
                        )
                    qn8s = qn8st_pool.tile([P, 512], e4, tag="qn8s")
                    nc.vector.tensor_scalar_mul(qn8s[:], pt_all[:], 1.0 / 64.0)
                    nc.sync.dma_start(
                        qn8[:, 2 * t:2 * t + 2, :, c, :], qn8s[:]
                    )
                    snn_unit(c, t)

                if c < 2:
                    pend_tr.append(transposes)
                else:
                    late_tr.append(transposes)

            # Boot interleave: stream head-0 and head-1 score pieces (feeding
            # the ACT exp pipeline) between qproj units so ACT never starves.
            sc = []
            sc1 = []

            def emit(n):
                for _ in range(n):
                    if sc:
                        sc.pop(0)()
                    elif sc1:
                        sc1.pop(0)()

            def ph_piece(c, v, h, **kw):
                t, k = v // 2, v % 2
                if k == 0 and t not in ess[c]:
                    ess[c][t] = es_pool.tile(
                        [P, 2, N], e4, tag="es", name=f"e{c}_{t}"
                    )
                scores_piece(c, v, h, ess[c][t], k, **kw)

            qproj_unit(0, 0)
            new_head_state(0)
            # rows 0-3 cols [vP,512): only needs the (0,0) qt8 fold -> ACT
            # exp stream starts while qproj is still warming up
            for v in range(4):
                sc.append(lambda v=v: ph_piece(0, v, 0, hi=512, slot=2))
            qproj_unit(1, 0)
            emit(1)
            qproj_unit(2, 0)
            emit(1)
            qproj_unit(3, 0)
            emit(2)
            load_x(2, (nc.sync, nc.sync))
            qproj_unit(0, 1)
            for v in range(8):
                sc.append(lambda v=v: ph_piece(0, v, 0, lo=max(v * P, 512)))
            qproj_unit(1, 1)
            new_head_state(1)
            for v in range(8):
                sc.append(lambda v=v: ph_piece(1, v, 0))
            emit(2)
            qproj_unit(2, 1)
            emit(2)
            qproj_unit(3, 1)
            emit(2)
            load_x(3, (nc.sync, nc.sync))
            qproj_unit(0, 2)
            emit(2)
            qproj_unit(1, 2)
            emit(2)
            qproj_unit(0, 3)
            # head-0 phase 2 (qt8[0] complete)
            for v in range(NCH):
                sc.append(lambda v=v: ph_piece(0, v, 1))
                if v >= 4 and v % 2 == 0:
                    sc.append(lambda v=v: mirror_units(0, [v - 3, v - 2]))
            sc.append(lambda: mirror_units(0, [NCH - 3, NCH - 2]))
            sc.append(lambda: mirror_units(0, [NCH - 1]))
            qproj_unit(1, 3)
            # head 1 (qt8[1] complete)
            for v in range(NCH):
                if v >= 8:
                    sc1.append(lambda v=v: ph_piece(1, v, 0))
                sc1.append(lambda v=v: ph_piece(1, v, 1))
                if v >= 4 and v % 2 == 0:
                    sc1.append(lambda v=v: mirror_units(1, [v - 3, v - 2]))
            sc1.append(lambda: mirror_units(1, [NCH - 3, NCH - 2]))
            sc1.append(lambda: mirror_units(1, [NCH - 1]))
            qproj_unit(2, 2)
            emit(4)
            qproj_unit(2, 3)
            emit(4)
            qproj_unit(3, 2)
            emit(4)
            qproj_unit(3, 3)
            while pend_tr:
                pend_tr.pop(0)()

        with (
            tc.tile_pool(name="wo8", bufs=1) as wo8_pool,
            tc.tile_pool(name="y8", bufs=1) as y8p,
            tc.tile_pool(name="osb", bufs=3) as o_pool,
        ):
            wo8h_sb = wo8_pool.tile([P, HG, D], e4, tag="wo8h")
            wo8l_sb = wo8_pool.tile([P, HG, D], e4, tag="wo8l")
            nc.sync.dma_start(wo8h_sb[:], wo8h[:])
            nc.gpsimd.dma_start(wo8l_sb[:], wo8l[:])
            y8h = y8p.tile([P, HG, N], e4, tag="y8h")
            y8l = y8p.tile([P, HG, N], e4, tag="y8l")
            rbz = {}

            def outproj_unit(a, grp):
                # grp 0: heads 0-1 partial -> outA ; grp 1: heads 2-3 -> out
                # grp 2: all heads (combined) -> out
                dst = outA3 if grp == 0 else out3
                ot = o_pool.tile([P, D], f16, tag="ot")
                for d4 in range(SP):
                    pool = ps_s if d4 % 2 else ps_b
                    ps = pool.tile([P, 512], f32, tag="s" if d4 % 2 else "b")
                    combos = []
                    for cc in ((0,), (2,), (0, 2))[grp]:
                        combos += [
                            (y8h, wo8h_sb, cc),
                            (y8h, wo8l_sb, cc),
                            (y8l, wo8h_sb, cc),
                        ]
                    for i, (yy, ww, cc) in enumerate(combos):
                        nc.tensor.matmul(
                            ps[:],
                            yy[:, cc:cc + 2, a * P:(a + 1) * P],
                            ww[:, cc:cc + 2, d4 * 512:(d4 + 1) * 512],
                            start=(i == 0),
                            stop=(i == len(combos) - 1),
                            perf_mode=DR,
                        )
                    if d4 % 2 == 0:
                        nc.vector.tensor_scalar_mul(
                            ot[:, d4 * 512:(d4 + 1) * 512], ps[:], 1.0 / 64.0
                        )
                    else:
                        nc.scalar.mul(
                            ot[:, d4 * 512:(d4 + 1) * 512], ps[:], 1.0 / 64.0
                        )
                        # stream each 1KB half out as soon as it's ready
                        half = d4 // 2
                        eng = nc.sync
                        eng.dma_start(
                            dst[:, a, half * 1024:(half + 1) * 1024],
                            ot[:, half * 1024:(half + 1) * 1024],
                        )

            def pv_mm(pc, s):
                # one span of YT via paired DR matmuls
                psy = ps_b.tile([P, 512], f32, tag="b", name=f"psy{pc}_{s}")
                lo_s = s * 512
                mms = []
                for t in range(2 * s):
                    mms.append((t, None, lo_s))
                mms.append((2 * s, 0, lo_s))                # solo chunk 4s
                mms.append((2 * s, None, lo_s + 128))       # pair (4s,4s+1)
                mms.append((2 * s + 1, 0, lo_s + 256))      # solo chunk 4s+2
                mms.append((2 * s + 1, None, lo_s + 384))   # pair (4s+2,4s+3)
                for i, (t, k, lo) in enumerate(mms):
                    st = (i == 0)
                    sp = (i == len(mms) - 1)
                    hi = (s + 1) * 512
                    if k is None:
                        nc.tensor.matmul(
                            psy[:, lo - lo_s:hi - lo_s],
                            qn8[:, t, :, pc, 0:P],
                            ess[pc][t][:, :, lo:hi],
                            start=st, stop=sp, perf_mode=DR,
                        )
                    else:
                        nc.tensor.matmul(
                            psy[:, lo - lo_s:lo - lo_s + 128],
                            qn8[:, t, k, pc, 0:P],
                            ess[pc][t][:, k, lo:lo + 128],
                            start=st, stop=sp,
                        )
                return psy

            def pv_evac(pc, s, psy, rbc, zbc):
                # yt = ((psy + yt_mirror) + q16*zrbc) * rbc  -> y8 hi/lo
                yt = yts[pc]
                sl = slice(s * 512, (s + 1) * 512)
                nc.vector.tensor_tensor(yt[:, sl], psy[:], yt[:, sl], ADD)
                nc.vector.tensor_tensor(yt[:, sl], yt[:, sl], rbc[:, sl], MUL)
                dg = tmp_pool.tile([P, 512], f16, tag="dg", name=f"dg{pc}_{s}")
                nc.vector.tensor_tensor(
                    dg[:], q16full[pc][:, sl], zbc[:, sl], MUL
                )
                nc.vector.tensor_tensor(yt[:, sl], yt[:, sl], dg[:], ADD)
                nc.gpsimd.tensor_copy(y8h[:, pc, sl], yt[:, sl])
                nc.gpsimd.tensor_tensor(
                    y8l[:, pc, sl], yt[:, sl], y8h[:, pc, sl], SUB
                )

            def pv_span(pc, s, rbc, zbc):
                pv_evac(pc, s, pv_mm(pc, s), rbc, zbc)

            # boot tail: drain head-0 stream, then overlap the head-1 exp
            # tail with PV(0) spans and the r-chains
            while sc:
                sc.pop(0)()
            rbz[0] = r_chain(0)
            s0 = 0
            while sc1:
                for _ in range(6):
                    if sc1:
                        sc1.pop(0)()
                if s0 < SP:
                    pv_span(0, s0, *rbz[0])
                    s0 += 1
            while s0 < SP:
                pv_span(0, s0, *rbz[0])
                s0 += 1
            rbz[1] = r_chain(1)

            # ============ stage 1: scores(2) || PV(1) ============
            # deferred qn8/S_nn prep for heads 2-3 (PE idle under exp here)
            while late_tr:
                late_tr.pop(0)()
            new_head_state(2)
            for v in range(NCH):
                scores_row(2, v)
                if v >= 4 and v % 2 == 0:
                    mirror_units(2, [v - 3, v - 2])
                    if v >= 8:
                        pv_span(1, (v - 8) // 2, *rbz[1])
            mirror_units(2, [NCH - 3, NCH - 2])
            mirror_units(2, [NCH - 1])
            rbz[2] = r_chain(2)

            # ===== stage 2: scores(3) || PV(2) + outproj-A (heads 0-1) =====
            new_head_state(3)
            for v in range(NCH):
                scores_row(3, v)
                if v >= 4 and v % 2 == 0:
                    mirror_units(3, [v - 3, v - 2])
                if v % 4 == 2:
                    pv_span(2, v // 4, *rbz[2])
            mirror_units(3, [NCH - 3, NCH - 2])
            mirror_units(3, [NCH - 1])
            rbz[3] = r_chain(3)

            # lull fill: heads-0/1 partials are independent of r_chain(3)
            for a in range(8):
                outproj_unit(a, 0)

            # == stage 3: PV(3) + outproj (B for a<6, combined for a>=6) ==
            psys = {0: pv_mm(3, 0), 1: pv_mm(3, 1)}
            for s in range(SP):
                pv_evac(3, s, psys[s], *rbz[3])
                if s + 2 < SP:
                    psys[s + 2] = pv_mm(3, s + 2)
                for a in range(4 * s, 4 * s + 4):
                    outproj_unit(a, 1 if a < 8 else 2)

    nc.compile()
    return nc


def _hi_lo(arr):
    hi = arr.astype(ml_dtypes.float8_e4m3)
    lo = (arr - hi.astype(np.float32)).astype(ml_dtypes.float8_e4m3)
    return hi, lo


def _pack_x(xt_hl):
    return np.ascontiguousarray(
        xt_hl.reshape(KP, 2, P, SP, 512).transpose(2, 3, 0, 1, 4)
    )


def kernel(x, Wq, Wo, bo):
    x = np.asarray(x, dtype=np.float32)
    Wq = np.asarray(Wq, dtype=np.float32)
    Wo = np.asarray(Wo, dtype=np.float32)
    bo = np.asarray(bo, dtype=np.float32)
    B = x.shape[0]
    assert B == 2 and x.shape == (B, N, D)
    assert Wq.shape == (D, D) and Wo.shape == (D, D)

    if "nc" not in _CACHE:
        _CACHE["nc"] = build_nc()
    nc = _CACHE["nc"]

    packed_x = []
    for b in range(B):
        hi, lo = _hi_lo(np.ascontiguousarray(x[b].T))
        packed_x.append((_pack_x(hi), _pack_x(lo)))
    in_maps = []
    for core in range(8):
        b, hg = core // 4, core % 4
        wq_s = WQS * Wq[:, hg * HCOLS:(hg + 1) * HCOLS]
        wqh, wql = _hi_lo(np.ascontiguousarray(wq_s))
        wo8h_a, wo8l_a = _hi_lo(
            np.ascontiguousarray(WQS * Wo[hg * HCOLS:(hg + 1) * HCOLS, :])
        )
        in_maps.append(
            {
                "xh": packed_x[b][0],
                "xl": packed_x[b][1],
                "wqh": np.ascontiguousarray(
                    wqh.reshape(KP, 2, P, HCOLS).transpose(2, 0, 1, 3)
                ),
                "wql": np.ascontiguousarray(
                    wql.reshape(KP, 2, P, HCOLS).transpose(2, 0, 1, 3)
                ),
                "wo8h": np.ascontiguousarray(
                    wo8h_a.reshape(HG, P, D).transpose(1, 0, 2)
                ),
                "wo8l": np.ascontiguousarray(
                    wo8l_a.reshape(HG, P, D).transpose(1, 0, 2)
                ),
            }
        )

    res = run_bass_kernel_spmd(nc, in_maps, list(range(8)))
    _CACHE["last_res"] = res
    out = np.zeros((B, N, D), dtype=np.float32)
    for core in range(8):
        b = core // 4
        out[b] += res.results[core]["out"].astype(np.float32)
        out[b][0:1024] += res.results[core]["outA"][0:1024].astype(np.float32)
    out += bo
    return out
